# revision 2
# baseline (speedup 1.0000x reference)
"""Trainium2 Bass kernel for nn_MultiHeadAttention_75548474736720.

Linear-attention-style MHA with two causal prefix-sum bilinear forms,
evaluated with a chunked (linear-attention) reformulation instead of the
naive O(S^2)-blocks triangular matmuls:
  qh/kh/vh = projections, ph = split_heads(p)
  A1 = elu(qh ph^T) + 1
  U[t,j] = sum_{s<=t} Sq[t,s] A1[s,j],  Sq = qh kh^T  (1/(t+1) in exp scale)
  W' = exp(U/(t+1)), den = sum_j W'
  out2[t,d] = (1/((t+1) den[t])) sum_{s<=t} (W'[t].A1[s]) vh[s,d]

Chunked evaluation (128-row chunks, 256-row state snapshots):
  U:  M[d,j] = cumsum_s kh[s,d] A1[s,j] held in f32 PSUM, snapshotted to
      bf16 SBUF every 256 rows; U(i) = SqT-strips @ A1 + qh_i @ M_snap.
  S2: N[j,d] = cumsum_s A1[s,j] vh[s,d] as a bf16 SBUF running sum;
      D_ii = masked(A1_i W'_i^T) via transposed strips;
      out2(i) = W'_i @ N_{<i} + D_ii-contract vh_i, ACT-scaled by
      1/((t+1) den) so W' is never normalized explicitly.

All [row,col]->[col,row] layout changes (W'^T, A1^T, kh, oN->oT) run on the
DMA crossbar (dma_start_transpose), costing no PE/ACT/DVE time.  The four
heads run through one software-pipelined loop (stages lag 0/2/3/4) so each
head's S2 drain overlaps the next head's U phase; A1 generation for head
h+1 is pulled two units per step into head h's loop.

Sharding: 8 cores = (batch b in 0..1) x (head-group hg in 0..3, 4 heads
each).  Each core computes its 4 heads end-to-end (wq/wk/wv column-sliced,
wc row-sliced) and returns a partial [S, Dm] output in bf16; the host sums
partials per batch and adds the wc bias.
"""

import sys

sys.path.insert(0, "/opt/trn_rl_repo")

import ml_dtypes
import numpy as np

import concourse.bass as bass  # noqa: F401  (registers AP machinery)
import concourse.mybir as mybir
from concourse import bacc
from concourse.tile import TileContext
from concourse.bass_utils import run_bass_kernel_spmd

F32 = mybir.dt.float32
BF16 = mybir.dt.bfloat16
ACTF = mybir.ActivationFunctionType
ALU = mybir.AluOpType
NPBF = ml_dtypes.bfloat16

B, S, DM, H = 2, 1024, 1024, 16
D = DM // H            # 64, head dim
HG = 4                 # heads per core
DL = HG * D            # 256, local dm slice
NB = S // 128          # 8 s-blocks
NORM_D = 0.125         # 1/sqrt(D)

DEBUG = False


def _build_program():
    nc = bacc.Bacc(None, target_bir_lowering=False)

    qT_in = nc.declare_dram_parameter("qT", [DM, S], BF16, isOutput=False)
    kT_in = nc.declare_dram_parameter("kT", [DM, S], BF16, isOutput=False)
    vT_in = nc.declare_dram_parameter("vT", [DM, S], BF16, isOutput=False)
    pT_in = nc.declare_dram_parameter("pT", [DL, S], BF16, isOutput=False)
    wq_in = nc.declare_dram_parameter("wq", [DM, DL], BF16, isOutput=False)
    wk_in = nc.declare_dram_parameter("wk", [DM, DL], BF16, isOutput=False)
    wv_in = nc.declare_dram_parameter("wv", [DM, DL], BF16, isOutput=False)
    wc_in = nc.declare_dram_parameter("wc", [DL, S], BF16, isOutput=False)
    wqb_in = nc.declare_dram_parameter("wqb", [128, 2], F32, isOutput=False)
    wkb_in = nc.declare_dram_parameter("wkb", [128, 2], F32, isOutput=False)
    wvb_in = nc.declare_dram_parameter("wvb", [1, DL], BF16, isOutput=False)
    ones_in = nc.declare_dram_parameter("ones1", [1, 128], BF16, isOutput=False)
    mask_in = nc.declare_dram_parameter("maskLE", [128, 128], BF16, isOutput=False)
    inv_in = nc.declare_dram_parameter("invidx", [128, NB], F32, isOutput=False)
    out_d = nc.declare_dram_parameter("out", [S, DM], BF16, isOutput=True)
    dbg = {}
    if DEBUG:
        dbg["a1"] = nc.declare_dram_parameter("d_a1", [128, NB * S], F32, isOutput=True)
        dbg["den"] = nc.declare_dram_parameter("d_den", [128, NB], F32, isOutput=True)
        dbg["wtT"] = nc.declare_dram_parameter("d_wtT", [128, NB * S], F32, isOutput=True)
        dbg["nsb"] = nc.declare_dram_parameter("d_nsb", [128, NB * 64], F32, isOutput=True)
        dbg["msb"] = nc.declare_dram_parameter("d_msb", [128, 2 * 512], F32, isOutput=True)
        dbg["oT"] = nc.declare_dram_parameter("d_oT", [64, HG * S], F32, isOutput=True)

    with TileContext(nc) as tc:
        with tc.tile_pool(name="persist", bufs=1) as cp, \
             tc.tile_pool(name="pm", bufs=4, space="PSUM") as pm, \
             tc.tile_pool(name="scr", bufs=2) as sp:

            maskLE = cp.tile([128, 128], BF16)
            invidx = cp.tile([128, NB], F32)
            wqb = cp.tile([128, 2], F32)
            wkb = cp.tile([128, 2], F32)
            wvb = cp.tile([1, DL], BF16)
            ones1 = cp.tile([1, 128], BF16)
            pTt = cp.tile([128, 2, S], BF16)
            qhT = cp.tile([128, 2, S], BF16)
            khT = cp.tile([128, 2, S], BF16)
            vh = cp.tile([128, NB, DL], BF16)
            oT = cp.tile([128, 2, S], BF16)
            wct = cp.tile([128, 2, S], BF16)
            # double-buffered big per-head tensors
            a1s = [cp.tile([128, NB, S], BF16, name=f"a1_{x}") for x in range(2)]
            wtTs = [cp.tile([128, NB, S], BF16, name=f"wtT_{x}") for x in range(2)]

            # PSUM cumulative state (persists across the per-head loops);
            # N is accumulated in SBUF bf16 snapshots instead (value path)
            Mps = [pm.tile([128, 512], F32, tag=f"Mps{c}", bufs=1, name=f"Mps{c}")
                   for c in range(2)]

            # ---------------- projections ----------------
            vp_cm = tc.tile_pool(name="vproj", bufs=1)
            vp = vp_cm.__enter__()
            wvt = vp.tile([128, NB, DL], BF16)
            vTt = vp.tile([128, NB, S], BF16)
            with tc.tile_pool(name="proj", bufs=1) as jp:
                wqt = jp.tile([128, NB, DL], BF16)
                wkt = jp.tile([128, NB, DL], BF16)
                qTt = jp.tile([128, NB, S], BF16)
                kTt = jp.tile([128, NB, S], BF16)
                for wt_, wsrc, xt_, xsrc in ((wqt, wq_in, qTt, qT_in),
                                             (wkt, wk_in, kTt, kT_in),
                                             (wvt, wv_in, vTt, vT_in)):
                    for q4 in range(4):
                        kb = 2 * q4
                        nc.sync.dma_start(
                            out=wt_[:, kb:kb + 2, :],
                            in_=wsrc[kb * 128:(kb + 2) * 128, :].rearrange(
                                "(a p) d -> p a d", p=128))
                        nc.sync.dma_start(
                            out=xt_[:, kb:kb + 2, :],
                            in_=xsrc[kb * 128:(kb + 2) * 128, :].rearrange(
                                "(a p) t -> p a t", p=128))
                    if wt_ is wqt:
                        nc.sync.dma_start(
                            out=pTt[:], in_=pT_in.rearrange("(g p) t -> p g t", p=128))
                        nc.sync.dma_start(out=wqb[:], in_=wqb_in[:])
                        nc.sync.dma_start(out=invidx[:], in_=inv_in[:])
                    elif wt_ is wkt:
                        nc.sync.dma_start(out=maskLE[:], in_=mask_in[:])
                        nc.sync.dma_start(out=wkb[:], in_=wkb_in[:])
                    else:
                        nc.sync.dma_start(out=wvb[:], in_=wvb_in[:])
                        nc.sync.dma_start(out=ones1[:], in_=ones_in[:])
                        nc.sync.dma_start(
                            out=wct[:], in_=wc_in.rearrange("(a p) t -> p a t", p=128))

                # qhT[dm, t] = sum_c wq[c, dm] qT[c, t]  (+bias, * 1/sqrt(D))
                for wt_, xt_, dst, bias_t, scale in (
                    (wqt, qTt, qhT, wqb, NORM_D),
                    (wkt, kTt, khT, wkb, 1.0),
                ):
                    for g in range(2):
                        for n in range(2):
                            ps = pm.tile([128, 512], F32, tag="mm", name="ps_proj")
                            for kb in range(NB):
                                nc.tensor.matmul(
                                    ps[:], wt_[:, kb, g * 128:(g + 1) * 128],
                                    xt_[:, kb, n * 512:(n + 1) * 512],
                                    start=(kb == 0), stop=(kb == NB - 1))
                            nc.scalar.activation(
                                dst[:, g, n * 512:(n + 1) * 512], ps[:],
                                ACTF.Identity, bias=bias_t[:, g:g + 1], scale=scale)

            # ---------------- attention (4 heads, chunked) ----------------
            st_sq = {}      # (h,i) -> masked SqT_ii strip
            st_wb = {}      # (h,i) -> W' block (exp, unnormalized)
            st_gsc = {}     # (h,i) -> 1/((t+1) den) column
            st_a1t = {}     # (h,i) -> A1^T strip
            st_dsb = {}     # (h,i) -> masked D_ii^T
            st_nsb = {}     # (h,i) -> N snapshot through chunk i
            msbs = {}       # (c, i%2) -> M snapshot half
            khSs = {}       # h -> kh [s,d] strips
            oNs = {}

            def a1_gen(h):
                """A1 = elu(qh ph^T)+1 = min(exp(x),1) + relu(x); 16 units."""
                g, p0 = h // 2, (h % 2) * 64
                a1 = a1s[h % 2]
                for m in range(NB):
                    for c in range(2):
                        ps = pm.tile([128, 512], F32, tag="a1ps", bufs=2,
                                     name="ps_a1")
                        nc.tensor.matmul(
                            ps[:], qhT[p0:p0 + 64, g, m * 128:(m + 1) * 128],
                            pTt[p0:p0 + 64, g, c * 512:(c + 1) * 512],
                            start=True, stop=True)
                        e = sp.tile([128, 512], BF16, tag="e", bufs=6, name="e")
                        nc.scalar.activation(e[:], ps[:], ACTF.Exp)
                        e1 = sp.tile([128, 512], BF16, tag="e1", bufs=6, name="e1")
                        nc.gpsimd.tensor_scalar_min(e1[:], e[:], 1.0)
                        nc.vector.scalar_tensor_tensor(
                            a1[:, m, c * 512:(c + 1) * 512], ps[:], 0.0, e1[:],
                            ALU.max, ALU.add)
                        yield

            def emit_khS(h):
                g, p0 = h // 2, (h % 2) * 64
                khS = sp.tile([128, NB, 64], BF16, tag="khS", bufs=3, name="khS")
                nc.sync.dma_start_transpose(out=khS[:], in_=khT[p0:p0 + 64, g, :])
                khSs[h] = khS

            def emit_sq(h, i):
                # SqT strip [s in block si, t in block i]: si = i (masked diag)
                # plus si = i-1 (unmasked) for odd i, whose M snapshot lags.
                g, p0 = h // 2, (h % 2) * 64
                for si in ([i - 1, i] if i % 2 == 1 else [i]):
                    ps = pm.tile([128, 128], F32, tag="mm", name="ps_sq")
                    nc.tensor.matmul(
                        ps[:], khT[p0:p0 + 64, g, si * 128:(si + 1) * 128],
                        qhT[p0:p0 + 64, g, i * 128:(i + 1) * 128],
                        start=True, stop=True)
                    sq = sp.tile([128, 128], BF16, tag="sq", bufs=4, name="sq")
                    if si == i:
                        nc.vector.tensor_tensor(sq[:], ps[:], maskLE[:], ALU.mult)
                    else:
                        nc.vector.tensor_copy(sq[:], ps[:])
                    st_sq[(h, i, si)] = sq

            def emit_u(h, i):
                # U(i) = SqT_ii @ A1_i + qh_i @ M_{<i};  W' = exp(U/(t+1))
                # M[d,j] += kh_i^T A1_i afterwards (PSUM accum, snapshot to bf16)
                g, p0 = h // 2, (h % 2) * 64
                a1 = a1s[h % 2]
                wb = sp.tile([128, S], BF16, tag="wblk", bufs=4, name="wb")
                st_wb[(h, i)] = wb
                strips = [st_sq.pop(k) for k in
                          ([(h, i, i - 1), (h, i, i)] if i % 2 == 1
                           else [(h, i, i)])]
                mlag = 2 * (i // 2) - 1   # M snapshot (odd index) U(i) reads
                dps = []
                for c in range(2):
                    ps = pm.tile([128, 512], F32, tag="mm", name="ps_u")
                    for z, sq in enumerate(strips):
                        si = i - (len(strips) - 1 - z)
                        nc.tensor.matmul(ps[:], sq[:],
                                         a1[:, si, c * 512:(c + 1) * 512],
                                         start=(z == 0),
                                         stop=(z == len(strips) - 1 and mlag < 0))
                    if mlag >= 0:
                        nc.tensor.matmul(
                            ps[:], qhT[p0:p0 + 64, g, i * 128:(i + 1) * 128],
                            msbs[(c, (mlag // 2) % 2)][p0:p0 + 64, :],
                            start=False, stop=True)
                    dp = sp.tile([128, 1], F32, tag="dp", bufs=6, name="dp")
                    nc.scalar.activation(
                        wb[:, c * 512:(c + 1) * 512], ps[:], ACTF.Exp,
                        scale=invidx[:, i:i + 1], accum_out=dp[:])
                    dps.append(dp)
                # M update for chunk i (after U used M_{<i})
                for c in range(2):
                    nc.tensor.matmul(
                        Mps[c][p0:p0 + 64, :], khSs[h][:, i, :],
                        a1[:, i, c * 512:(c + 1) * 512],
                        start=(i == 0), stop=True)
                if i % 2 == 1 and i < NB - 1:
                    for c in range(2):
                        msb = sp.tile([128, 512], BF16, tag=f"msb{c}", bufs=3,
                                      name="msb")
                        nc.vector.tensor_copy(msb[p0:p0 + 64, :],
                                              Mps[c][p0:p0 + 64, :])
                        msbs[(c, (i // 2) % 2)] = msb
                # denominator -> gsc = 1/((t+1) den)
                dsum = sp.tile([128, 1], F32, tag="dsum", bufs=2, name="dsum")
                nc.vector.tensor_tensor(dsum[:], dps[0][:], dps[1][:], ALU.add)
                rec = sp.tile([128, 1], F32, tag="rec", bufs=2, name="rec")
                nc.vector.reciprocal(rec[:], dsum[:])
                gsc = sp.tile([128, 1], F32, tag="gsc", bufs=8, name="gsc")
                nc.vector.tensor_tensor(gsc[:], rec[:], invidx[:, i:i + 1],
                                        ALU.mult)
                st_gsc[(h, i)] = gsc
                if DEBUG and h == 0:
                    nc.vector.tensor_copy(dbg_den[:, i:i + 1], dsum[:])

            def emit_wt(h, i):
                wb = st_wb.pop((h, i))
                nc.sync.dma_start_transpose(
                    out=wtTs[h % 2][:, :, i * 128:(i + 1) * 128], in_=wb[:])

            def emit_a1t(h, i):
                a1 = a1s[h % 2]
                at = sp.tile([128, NB, 128], BF16, tag="a1T", bufs=5, name="a1T")
                nc.sync.dma_start_transpose(out=at[:], in_=a1[:, i, :])
                st_a1t[(h, i)] = at

            def emit_d(h, i):
                # D_ii^T[s',t'] = sum_j A1[s,j] W'[t,j], masked to s<=t
                at = st_a1t.pop((h, i))
                ps = pm.tile([128, 128], F32, tag="mm", name="ps_d")
                for k in range(NB):
                    nc.tensor.matmul(
                        ps[:], at[:, k, :],
                        wtTs[h % 2][:, k, i * 128:(i + 1) * 128],
                        start=(k == 0), stop=(k == NB - 1))
                dsb = sp.tile([128, 128], BF16, tag="dsb", bufs=4, name="dsb")
                nc.vector.tensor_tensor(dsb[:], ps[:], maskLE[:], ALU.mult)
                st_dsb[(h, i)] = dsb

            def emit_nupd(h, i):
                # N_i[j,d] = N_{i-1} + A1_i^T vh_i  (SBUF bf16 running sum)
                a1 = a1s[h % 2]
                d0 = h * 64
                ps = pm.tile([128, NB, 64], F32, tag="mm", name="ps_n")
                for k in range(NB):
                    nc.tensor.matmul(
                        ps[:, k, :], a1[:, i, k * 128:(k + 1) * 128],
                        vh[:, i, d0:d0 + 64], start=True, stop=True)
                nsb = sp.tile([128, NB, 64], BF16, tag="nsb", bufs=5, name="nsb")
                if i == 0:
                    nc.vector.tensor_copy(nsb[:], ps[:])
                else:
                    nc.vector.tensor_tensor(nsb[:], ps[:],
                                            st_nsb[(h, i - 1)][:], ALU.add)
                st_nsb[(h, i)] = nsb
                if DEBUG and h == 0 and i == NB - 1:
                    nc.sync.dma_start(out=dbg["nsb"].rearrange(
                        "p (a b) -> p a b", a=NB), in_=nsb[:])

            def emit_o2(h, i):
                # out2(i) = (W'_i @ N_{<i} + D^T-contract vh_i) * gsc
                d0 = h * 64
                if h % 2 == 0 and i == 0:
                    oNs[h // 2] = sp.tile([128, NB, 128], BF16, tag="oN",
                                          bufs=2, name="oN")
                oN = oNs[h // 2]
                ps = pm.tile([128, 64], F32, tag="mm", name="ps_o2")
                dsb = st_dsb.pop((h, i))
                if i > 0:
                    nsb = st_nsb[(h, i - 1)]
                    for k in range(NB):
                        nc.tensor.matmul(
                            ps[:], wtTs[h % 2][:, k, i * 128:(i + 1) * 128],
                            nsb[:, k, :], start=(k == 0), stop=False)
                    nc.tensor.matmul(ps[:], dsb[:], vh[:, i, d0:d0 + 64],
                                     start=False, stop=True)
                else:
                    nc.tensor.matmul(ps[:], dsb[:], vh[:, i, d0:d0 + 64],
                                     start=True, stop=True)
                if i >= 2:
                    st_nsb.pop((h, i - 2), None)
                nc.scalar.activation(
                    oN[:, i, (h % 2) * 64:(h % 2) * 64 + 64], ps[:],
                    ACTF.Copy, scale=st_gsc.pop((h, i))[:])
                if h % 2 == 1:
                    nc.sync.dma_start_transpose(
                        out=oT[:, h // 2, i * 128:(i + 1) * 128],
                        in_=oN[:, i, :])

            def emit_final_tile(i):
                for c in range(2):
                    ps = pm.tile([128, 512], F32, tag="mm", name="ps_fin")
                    for g2 in range(2):
                        nc.tensor.matmul(
                            ps[:], oT[:, g2, i * 128:(i + 1) * 128],
                            wct[:, g2, c * 512:(c + 1) * 512],
                            start=(g2 == 0), stop=(g2 == 1))
                    ot = sp.tile([128, 512], BF16, tag="ot", bufs=6, name="ot")
                    if (i + c) % 2 == 0:
                        nc.scalar.activation(ot[:], ps[:], ACTF.Copy)
                    else:
                        nc.vector.tensor_copy(ot[:], ps[:])
                    nc.sync.dma_start(
                        out=out_d[i * 128:(i + 1) * 128, c * 512:(c + 1) * 512],
                        in_=ot[:])

            if DEBUG:
                dbg_den = sp.tile([128, NB], F32, tag="dbgden", bufs=1,
                                  name="dbgden")

            # vh[s, d] = sum_c vT[c, s] wv[c, d] + wv_b[d], interleaved with
            # head 0's A1 so PE has work while vT streams in
            gens = {hh: a1_gen(hh) for hh in range(HG)}

            def pull(h, n):
                if h < HG:
                    for _ in range(n):
                        if next(gens[h], "done") == "done":
                            break

            for m in range(NB):
                ps = pm.tile([128, DL], F32, tag="mm", name="ps_vh")
                for kb in range(NB):
                    nc.tensor.matmul(
                        ps[:], vTt[:, kb, m * 128:(m + 1) * 128], wvt[:, kb, :],
                        start=(kb == 0), stop=False)
                nc.tensor.matmul(ps[:], ones1[:], wvb[:], start=False, stop=True)
                nc.scalar.activation(vh[:, m, :], ps[:], ACTF.Copy)
                pull(0, 2)
            pull(0, 16)
            if DEBUG:
                nc.sync.dma_start(out=dbg["a1"].rearrange("p (a b) -> p a b", a=NB),
                                  in_=a1s[0][:])

            def hi(tau):
                # map absolute pipeline time to (head, iter), None past the end
                h, i = divmod(tau, NB)
                return (h, i) if 0 <= h < HG else None

            emit_khS(0)
            emit_sq(0, 0)
            for tau in range(HG * NB + 4):
                cur = hi(tau)
                if cur:
                    emit_u(*cur)
                    nxt = hi(tau + 1)
                    if nxt:
                        if nxt[1] == 0:
                            emit_khS(nxt[0])
                        emit_sq(*nxt)
                    emit_wt(*cur)
                pull(tau // NB + 1, 1)
                if cur:
                    emit_a1t(*cur)
                if hi(tau - 2):
                    emit_nupd(*hi(tau - 2))
                pull(tau // NB + 1, 1)
                if hi(tau - 3):
                    emit_d(*hi(tau - 3))
                if hi(tau - 4):
                    emit_o2(*hi(tau - 4))
            st_nsb.clear()
            oNs.clear()
            for i in range(NB):
                emit_final_tile(i)
            if DEBUG:
                nc.sync.dma_start(
                    out=dbg["oT"].rearrange("p (a b) -> p a b", a=HG), in_=oT[:])

            vp_cm.__exit__(None, None, None)

    nc.finalize()
    return nc


_CACHE = {}


def _get_program():
    if "nc" not in _CACHE:
        _CACHE["nc"] = _build_program()
    return _CACHE["nc"]


def _consts():
    if "consts" not in _CACHE:
        p_ = np.arange(128, dtype=np.float32)[:, None]
        c_ = np.arange(128, dtype=np.float32)[None, :]
        maskLE = (p_ <= c_).astype(NPBF)
        blk = np.arange(NB, dtype=np.float32)[None, :]
        invidx = (1.0 / (blk * 128.0 + p_ + 1.0)).astype(np.float32)
        ones1 = np.ones((1, 128), NPBF)
        _CACHE["consts"] = (maskLE, invidx, ones1)
    return _CACHE["consts"]


PROFILE = False
LAST_RESULTS = None


def kernel(v, k, q, p, wq_k, wq_b, wk_k, wk_b, wv_k, wv_b, wc_k, wc_b):
    global LAST_RESULTS
    nc = _get_program()
    maskLE, invidx, ones1 = _consts()

    qT = [np.ascontiguousarray(q[b].T).astype(NPBF) for b in range(B)]
    kT = [np.ascontiguousarray(k[b].T).astype(NPBF) for b in range(B)]
    vT = [np.ascontiguousarray(v[b].T).astype(NPBF) for b in range(B)]
    pT = [np.ascontiguousarray(p[b].T).astype(NPBF) for b in range(B)]
    wqc = wq_k.astype(NPBF)
    wkc = wk_k.astype(NPBF)
    wvc = wv_k.astype(NPBF)
    wcc = wc_k.astype(NPBF)

    in_maps = []
    for c in range(8):
        b, hg = c // 4, c % 4
        c0 = hg * DL
        wqb = np.ascontiguousarray(
            (wq_b[c0:c0 + DL].reshape(2, 128).T * NORM_D).astype(np.float32))
        wkb = np.ascontiguousarray(wk_b[c0:c0 + DL].reshape(2, 128).T.astype(np.float32))
        in_maps.append({
            "qT": qT[b], "kT": kT[b], "vT": vT[b],
            "pT": np.ascontiguousarray(pT[b][c0:c0 + DL]),
            "wq": np.ascontiguousarray(wqc[:, c0:c0 + DL]),
            "wk": np.ascontiguousarray(wkc[:, c0:c0 + DL]),
            "wv": np.ascontiguousarray(wvc[:, c0:c0 + DL]),
            "wc": np.ascontiguousarray(wcc[c0:c0 + DL, :]),
            "wqb": wqb, "wkb": wkb,
            "wvb": np.ascontiguousarray(wv_b[c0:c0 + DL].reshape(1, DL).astype(NPBF)),
            "ones1": ones1, "maskLE": maskLE, "invidx": invidx,
        })

    res = run_bass_kernel_spmd(
        nc, in_maps, core_ids=list(range(8)), trace=PROFILE)
    LAST_RESULTS = res

    out = np.zeros((B, S, DM), np.float32)
    for c in range(8):
        out[c // 4] += res.results[c]["out"].astype(np.float32)
    out += wc_b[None, None, :].astype(np.float32)
    return out


# revision 3
# speedup vs baseline: 1.0058x; 1.0058x over previous
"""Trainium2 Bass kernel for nn_MultiHeadAttention_75548474736720.

Linear-attention-style MHA with two causal prefix-sum bilinear forms,
evaluated with a chunked (linear-attention) reformulation instead of the
naive O(S^2)-blocks triangular matmuls:
  qh/kh/vh = projections, ph = split_heads(p)
  A1 = elu(qh ph^T) + 1
  U[t,j] = sum_{s<=t} Sq[t,s] A1[s,j],  Sq = qh kh^T  (1/(t+1) in exp scale)
  W' = exp(U/(t+1)), den = sum_j W'
  out2[t,d] = (1/((t+1) den[t])) sum_{s<=t} (W'[t].A1[s]) vh[s,d]

Chunked evaluation (128-row chunks, 256-row state snapshots):
  U:  M[d,j] = cumsum_s kh[s,d] A1[s,j] held in f32 PSUM, snapshotted to
      bf16 SBUF every 256 rows; U(i) = SqT-strips @ A1 + qh_i @ M_snap.
  S2: N[j,d] = cumsum_s A1[s,j] vh[s,d] as a bf16 SBUF running sum;
      D_ii = masked(A1_i W'_i^T) via transposed strips;
      out2(i) = W'_i @ N_{<i} + D_ii-contract vh_i, ACT-scaled by
      1/((t+1) den) so W' is never normalized explicitly.

All [row,col]->[col,row] layout changes (W'^T, A1^T, kh, oN->oT) run on the
DMA crossbar (dma_start_transpose), costing no PE/ACT/DVE time.  The four
heads run through one software-pipelined loop (stages lag 0/2/3/4) so each
head's S2 drain overlaps the next head's U phase; A1 generation for head
h+1 is pulled two units per step into head h's loop.

Sharding: 8 cores = (batch b in 0..1) x (head-group hg in 0..3, 4 heads
each).  Each core computes its 4 heads end-to-end (wq/wk/wv column-sliced,
wc row-sliced) and returns a partial [S, Dm] output in bf16; the host sums
partials per batch and adds the wc bias.
"""

import sys

sys.path.insert(0, "/opt/trn_rl_repo")

import ml_dtypes
import numpy as np

import concourse.bass as bass  # noqa: F401  (registers AP machinery)
import concourse.mybir as mybir
from concourse import bacc
from concourse.tile import TileContext
from concourse.bass_utils import run_bass_kernel_spmd

F32 = mybir.dt.float32
BF16 = mybir.dt.bfloat16
ACTF = mybir.ActivationFunctionType
ALU = mybir.AluOpType
NPBF = ml_dtypes.bfloat16

B, S, DM, H = 2, 1024, 1024, 16
D = DM // H            # 64, head dim
HG = 4                 # heads per core
DL = HG * D            # 256, local dm slice
NB = S // 128          # 8 s-blocks
NORM_D = 0.125         # 1/sqrt(D)

DEBUG = False


def _build_program():
    nc = bacc.Bacc(None, target_bir_lowering=False)

    qT_in = nc.declare_dram_parameter("qT", [DM, S], BF16, isOutput=False)
    kT_in = nc.declare_dram_parameter("kT", [DM, S], BF16, isOutput=False)
    vT_in = nc.declare_dram_parameter("vT", [DM, S], BF16, isOutput=False)
    pT_in = nc.declare_dram_parameter("pT", [DL, S], BF16, isOutput=False)
    wq_in = nc.declare_dram_parameter("wq", [DM, DL], BF16, isOutput=False)
    wk_in = nc.declare_dram_parameter("wk", [DM, DL], BF16, isOutput=False)
    wv_in = nc.declare_dram_parameter("wv", [DM, DL], BF16, isOutput=False)
    wc_in = nc.declare_dram_parameter("wc", [DL, S], BF16, isOutput=False)
    wqb_in = nc.declare_dram_parameter("wqb", [128, 2], F32, isOutput=False)
    wkb_in = nc.declare_dram_parameter("wkb", [128, 2], F32, isOutput=False)
    wvb_in = nc.declare_dram_parameter("wvb", [1, DL], BF16, isOutput=False)
    ones_in = nc.declare_dram_parameter("ones1", [1, 128], BF16, isOutput=False)
    mask_in = nc.declare_dram_parameter("maskLE", [128, 128], BF16, isOutput=False)
    inv_in = nc.declare_dram_parameter("invidx", [128, NB], F32, isOutput=False)
    out_d = nc.declare_dram_parameter("out", [S, DM], BF16, isOutput=True)
    dbg = {}
    if DEBUG:
        dbg["a1"] = nc.declare_dram_parameter("d_a1", [128, NB * S], F32, isOutput=True)
        dbg["den"] = nc.declare_dram_parameter("d_den", [128, NB], F32, isOutput=True)
        dbg["wtT"] = nc.declare_dram_parameter("d_wtT", [128, NB * S], F32, isOutput=True)
        dbg["nsb"] = nc.declare_dram_parameter("d_nsb", [128, NB * 64], F32, isOutput=True)
        dbg["msb"] = nc.declare_dram_parameter("d_msb", [128, 2 * 512], F32, isOutput=True)
        dbg["oT"] = nc.declare_dram_parameter("d_oT", [64, HG * S], F32, isOutput=True)

    with TileContext(nc) as tc:
        with tc.tile_pool(name="persist", bufs=1) as cp, \
             tc.tile_pool(name="pm", bufs=4, space="PSUM") as pm, \
             tc.tile_pool(name="scr", bufs=2) as sp:

            maskLE = cp.tile([128, 128], BF16)
            invidx = cp.tile([128, NB], F32)
            wqb = cp.tile([128, 2], F32)
            wkb = cp.tile([128, 2], F32)
            wvb = cp.tile([1, DL], BF16)
            ones1 = cp.tile([1, 128], BF16)
            pTt = cp.tile([128, 2, S], BF16)
            qhT = cp.tile([128, 2, S], BF16)
            khT = cp.tile([128, 2, S], BF16)
            vh = cp.tile([128, NB, DL], BF16)
            oT = cp.tile([128, 2, S], BF16)
            wct = cp.tile([128, 2, S], BF16)
            # double-buffered big per-head tensors
            a1s = [cp.tile([128, NB, S], BF16, name=f"a1_{x}") for x in range(2)]
            wtTs = [cp.tile([128, NB, S], BF16, name=f"wtT_{x}") for x in range(2)]

            # PSUM cumulative state (persists across the per-head loops);
            # N is accumulated in SBUF bf16 snapshots instead (value path)
            Mps = [pm.tile([128, 512], F32, tag=f"Mps{c}", bufs=1, name=f"Mps{c}")
                   for c in range(2)]

            # ---------------- projections ----------------
            vp_cm = tc.tile_pool(name="vproj", bufs=1)
            vp = vp_cm.__enter__()
            wvt = vp.tile([128, NB, DL], BF16)
            vTt = vp.tile([128, NB, S], BF16)
            with tc.tile_pool(name="proj", bufs=1) as jp:
                wqt = jp.tile([128, NB, DL], BF16)
                wkt = jp.tile([128, NB, DL], BF16)
                qTt = jp.tile([128, NB, S], BF16)
                kTt = jp.tile([128, NB, S], BF16)
                for wt_, wsrc, xt_, xsrc in ((wqt, wq_in, qTt, qT_in),
                                             (wkt, wk_in, kTt, kT_in),
                                             (wvt, wv_in, vTt, vT_in)):
                    for q4 in range(4):
                        kb = 2 * q4
                        nc.sync.dma_start(
                            out=wt_[:, kb:kb + 2, :],
                            in_=wsrc[kb * 128:(kb + 2) * 128, :].rearrange(
                                "(a p) d -> p a d", p=128))
                        nc.sync.dma_start(
                            out=xt_[:, kb:kb + 2, :],
                            in_=xsrc[kb * 128:(kb + 2) * 128, :].rearrange(
                                "(a p) t -> p a t", p=128))
                    if wt_ is wqt:
                        nc.sync.dma_start(
                            out=pTt[:], in_=pT_in.rearrange("(g p) t -> p g t", p=128))
                        nc.sync.dma_start(out=wqb[:], in_=wqb_in[:])
                        nc.sync.dma_start(out=invidx[:], in_=inv_in[:])
                    elif wt_ is wkt:
                        nc.sync.dma_start(out=maskLE[:], in_=mask_in[:])
                        nc.sync.dma_start(out=wkb[:], in_=wkb_in[:])
                    else:
                        nc.sync.dma_start(out=wvb[:], in_=wvb_in[:])
                        nc.sync.dma_start(out=ones1[:], in_=ones_in[:])
                        nc.sync.dma_start(
                            out=wct[:], in_=wc_in.rearrange("(a p) t -> p a t", p=128))

                # qhT[dm, t] = sum_c wq[c, dm] qT[c, t]  (+bias, * 1/sqrt(D))
                for wt_, xt_, dst, bias_t, scale in (
                    (wqt, qTt, qhT, wqb, NORM_D),
                    (wkt, kTt, khT, wkb, 1.0),
                ):
                    for g in range(2):
                        for n in range(2):
                            ps = pm.tile([128, 512], F32, tag="mm", name="ps_proj")
                            for kb in range(NB):
                                nc.tensor.matmul(
                                    ps[:], wt_[:, kb, g * 128:(g + 1) * 128],
                                    xt_[:, kb, n * 512:(n + 1) * 512],
                                    start=(kb == 0), stop=(kb == NB - 1))
                            nc.scalar.activation(
                                dst[:, g, n * 512:(n + 1) * 512], ps[:],
                                ACTF.Identity, bias=bias_t[:, g:g + 1], scale=scale)

            # ---------------- attention (4 heads, chunked) ----------------
            st_sq = {}      # (h,i) -> masked SqT_ii strip
            st_wb = {}      # (h,i) -> W' block (exp, unnormalized)
            st_gsc = {}     # (h,i) -> 1/((t+1) den) column
            st_a1t = {}     # (h,i) -> A1^T strip
            st_dsb = {}     # (h,i) -> masked D_ii^T
            st_nsb = {}     # (h,i) -> N snapshot through chunk i
            msbs = {}       # (c, i%2) -> M snapshot half
            khSs = {}       # h -> kh [s,d] strips
            oNs = {}

            def a1_gen(h):
                """A1 = elu(qh ph^T)+1 = min(exp(x),1) + relu(x); 16 units."""
                g, p0 = h // 2, (h % 2) * 64
                a1 = a1s[h % 2]
                for m in range(NB):
                    for c in range(2):
                        ps = pm.tile([128, 512], F32, tag="a1ps", bufs=2,
                                     name="ps_a1")
                        nc.tensor.matmul(
                            ps[:], qhT[p0:p0 + 64, g, m * 128:(m + 1) * 128],
                            pTt[p0:p0 + 64, g, c * 512:(c + 1) * 512],
                            start=True, stop=True)
                        e = sp.tile([128, 512], BF16, tag="e", bufs=6, name="e")
                        nc.scalar.activation(e[:], ps[:], ACTF.Exp)
                        e1 = sp.tile([128, 512], BF16, tag="e1", bufs=6, name="e1")
                        nc.gpsimd.tensor_scalar_min(e1[:], e[:], 1.0)
                        nc.vector.scalar_tensor_tensor(
                            a1[:, m, c * 512:(c + 1) * 512], ps[:], 0.0, e1[:],
                            ALU.max, ALU.add)
                        yield

            def emit_khS(h):
                g, p0 = h // 2, (h % 2) * 64
                khS = sp.tile([128, NB, 64], BF16, tag="khS", bufs=3, name="khS")
                nc.sync.dma_start_transpose(out=khS[:], in_=khT[p0:p0 + 64, g, :])
                khSs[h] = khS

            def emit_sq(h, i):
                # SqT strip [s in block si, t in block i]: si = i (masked diag)
                # plus si = i-1 (unmasked) for odd i, whose M snapshot lags.
                g, p0 = h // 2, (h % 2) * 64
                for si in ([i - 1, i] if i % 2 == 1 else [i]):
                    ps = pm.tile([128, 128], F32, tag="mm", name="ps_sq")
                    nc.tensor.matmul(
                        ps[:], khT[p0:p0 + 64, g, si * 128:(si + 1) * 128],
                        qhT[p0:p0 + 64, g, i * 128:(i + 1) * 128],
                        start=True, stop=True)
                    sq = sp.tile([128, 128], BF16, tag="sq", bufs=4, name="sq")
                    if si == i:
                        nc.vector.tensor_tensor(sq[:], ps[:], maskLE[:], ALU.mult)
                    else:
                        nc.vector.tensor_copy(sq[:], ps[:])
                    st_sq[(h, i, si)] = sq

            def emit_u(h, i):
                # U(i) = SqT_ii @ A1_i + qh_i @ M_{<i};  W' = exp(U/(t+1))
                # M[d,j] += kh_i^T A1_i afterwards (PSUM accum, snapshot to bf16)
                g, p0 = h // 2, (h % 2) * 64
                a1 = a1s[h % 2]
                wb = sp.tile([128, S], BF16, tag="wblk", bufs=4, name="wb")
                st_wb[(h, i)] = wb
                strips = [st_sq.pop(k) for k in
                          ([(h, i, i - 1), (h, i, i)] if i % 2 == 1
                           else [(h, i, i)])]
                mlag = 2 * (i // 2) - 1   # M snapshot (odd index) U(i) reads
                dps = []
                for c in range(2):
                    ps = pm.tile([128, 512], F32, tag="mm", name="ps_u")
                    for z, sq in enumerate(strips):
                        si = i - (len(strips) - 1 - z)
                        nc.tensor.matmul(ps[:], sq[:],
                                         a1[:, si, c * 512:(c + 1) * 512],
                                         start=(z == 0),
                                         stop=(z == len(strips) - 1 and mlag < 0))
                    if mlag >= 0:
                        nc.tensor.matmul(
                            ps[:], qhT[p0:p0 + 64, g, i * 128:(i + 1) * 128],
                            msbs[(c, (mlag // 2) % 2)][p0:p0 + 64, :],
                            start=False, stop=True)
                    dp = sp.tile([128, 1], F32, tag="dp", bufs=6, name="dp")
                    nc.scalar.activation(
                        wb[:, c * 512:(c + 1) * 512], ps[:], ACTF.Exp,
                        scale=invidx[:, i:i + 1], accum_out=dp[:])
                    dps.append(dp)
                # M update for chunk i (after U used M_{<i})
                for c in range(2):
                    nc.tensor.matmul(
                        Mps[c][p0:p0 + 64, :], khSs[h][:, i, :],
                        a1[:, i, c * 512:(c + 1) * 512],
                        start=(i == 0), stop=True)
                if i % 2 == 1 and i < NB - 1:
                    for c in range(2):
                        msb = sp.tile([128, 512], BF16, tag=f"msb{c}", bufs=3,
                                      name="msb")
                        nc.vector.tensor_copy(msb[p0:p0 + 64, :],
                                              Mps[c][p0:p0 + 64, :])
                        msbs[(c, (i // 2) % 2)] = msb
                # denominator -> gsc = 1/((t+1) den)
                dsum = sp.tile([128, 1], F32, tag="dsum", bufs=2, name="dsum")
                nc.vector.tensor_tensor(dsum[:], dps[0][:], dps[1][:], ALU.add)
                rec = sp.tile([128, 1], F32, tag="rec", bufs=2, name="rec")
                nc.vector.reciprocal(rec[:], dsum[:])
                gsc = sp.tile([128, 1], F32, tag="gsc", bufs=8, name="gsc")
                nc.vector.tensor_tensor(gsc[:], rec[:], invidx[:, i:i + 1],
                                        ALU.mult)
                st_gsc[(h, i)] = gsc
                if DEBUG and h == 0:
                    nc.vector.tensor_copy(dbg_den[:, i:i + 1], dsum[:])

            def emit_wt(h, i):
                wb = st_wb.pop((h, i))
                nc.sync.dma_start_transpose(
                    out=wtTs[h % 2][:, :, i * 128:(i + 1) * 128], in_=wb[:])

            def emit_a1t(h, i):
                a1 = a1s[h % 2]
                at = sp.tile([128, NB, 128], BF16, tag="a1T", bufs=5, name="a1T")
                nc.sync.dma_start_transpose(out=at[:], in_=a1[:, i, :])
                st_a1t[(h, i)] = at

            def emit_d(h, i):
                # D_ii^T[s',t'] = sum_j A1[s,j] W'[t,j], masked to s<=t
                at = st_a1t.pop((h, i))
                ps = pm.tile([128, 128], F32, tag="mm", name="ps_d")
                for k in range(NB):
                    nc.tensor.matmul(
                        ps[:], at[:, k, :],
                        wtTs[h % 2][:, k, i * 128:(i + 1) * 128],
                        start=(k == 0), stop=(k == NB - 1))
                dsb = sp.tile([128, 128], BF16, tag="dsb", bufs=4, name="dsb")
                nc.vector.tensor_tensor(dsb[:], ps[:], maskLE[:], ALU.mult)
                st_dsb[(h, i)] = dsb

            def emit_nupd(h, i):
                # N_i[j,d] = N_{i-1} + A1_i^T vh_i  (SBUF bf16 running sum)
                a1 = a1s[h % 2]
                d0 = h * 64
                ps = pm.tile([128, NB, 64], F32, tag="mm", name="ps_n")
                for k in range(NB):
                    nc.tensor.matmul(
                        ps[:, k, :], a1[:, i, k * 128:(k + 1) * 128],
                        vh[:, i, d0:d0 + 64], start=True, stop=True)
                nsb = sp.tile([128, NB, 64], BF16, tag="nsb", bufs=5, name="nsb")
                if i == 0:
                    nc.vector.tensor_copy(nsb[:], ps[:])
                else:
                    nc.vector.tensor_tensor(nsb[:], ps[:],
                                            st_nsb[(h, i - 1)][:], ALU.add)
                st_nsb[(h, i)] = nsb
                if DEBUG and h == 0 and i == NB - 1:
                    nc.sync.dma_start(out=dbg["nsb"].rearrange(
                        "p (a b) -> p a b", a=NB), in_=nsb[:])

            def emit_o2(h, i):
                # out2(i) = (W'_i @ N_{<i} + D^T-contract vh_i) * gsc
                d0 = h * 64
                if h % 2 == 0 and i == 0:
                    oNs[h // 2] = sp.tile([128, NB, 128], BF16, tag="oN",
                                          bufs=2, name="oN")
                oN = oNs[h // 2]
                ps = pm.tile([128, 64], F32, tag="mm", name="ps_o2")
                dsb = st_dsb.pop((h, i))
                if i > 0:
                    nsb = st_nsb[(h, i - 1)]
                    for k in range(NB):
                        nc.tensor.matmul(
                            ps[:], wtTs[h % 2][:, k, i * 128:(i + 1) * 128],
                            nsb[:, k, :], start=(k == 0), stop=False)
                    nc.tensor.matmul(ps[:], dsb[:], vh[:, i, d0:d0 + 64],
                                     start=False, stop=True)
                else:
                    nc.tensor.matmul(ps[:], dsb[:], vh[:, i, d0:d0 + 64],
                                     start=True, stop=True)
                if i >= 2:
                    st_nsb.pop((h, i - 2), None)
                nc.scalar.activation(
                    oN[:, i, (h % 2) * 64:(h % 2) * 64 + 64], ps[:],
                    ACTF.Copy, scale=st_gsc.pop((h, i))[:])
                if h % 2 == 1:
                    nc.sync.dma_start_transpose(
                        out=oT[:, h // 2, i * 128:(i + 1) * 128],
                        in_=oN[:, i, :])

            def emit_final_tile(i):
                for c in range(2):
                    ps = pm.tile([128, 512], F32, tag="mm", name="ps_fin")
                    for g2 in range(2):
                        nc.tensor.matmul(
                            ps[:], oT[:, g2, i * 128:(i + 1) * 128],
                            wct[:, g2, c * 512:(c + 1) * 512],
                            start=(g2 == 0), stop=(g2 == 1))
                    ot = sp.tile([128, 512], BF16, tag="ot", bufs=6, name="ot")
                    if (i + c) % 2 == 0:
                        nc.scalar.activation(ot[:], ps[:], ACTF.Copy)
                    else:
                        nc.vector.tensor_copy(ot[:], ps[:])
                    nc.sync.dma_start(
                        out=out_d[i * 128:(i + 1) * 128, c * 512:(c + 1) * 512],
                        in_=ot[:])

            if DEBUG:
                dbg_den = sp.tile([128, NB], F32, tag="dbgden", bufs=1,
                                  name="dbgden")

            # vh[s, d] = sum_c vT[c, s] wv[c, d] + wv_b[d], interleaved with
            # head 0's A1 so PE has work while vT streams in
            gens = {hh: a1_gen(hh) for hh in range(HG)}

            def pull(h, n):
                if h < HG:
                    for _ in range(n):
                        if next(gens[h], "done") == "done":
                            break

            for m in range(NB):
                ps = pm.tile([128, DL], F32, tag="mm", name="ps_vh")
                for kb in range(NB):
                    nc.tensor.matmul(
                        ps[:], vTt[:, kb, m * 128:(m + 1) * 128], wvt[:, kb, :],
                        start=(kb == 0), stop=False)
                nc.tensor.matmul(ps[:], ones1[:], wvb[:], start=False, stop=True)
                nc.scalar.activation(vh[:, m, :], ps[:], ACTF.Copy)
                pull(0, 2)
            pull(0, 16)
            if DEBUG:
                nc.sync.dma_start(out=dbg["a1"].rearrange("p (a b) -> p a b", a=NB),
                                  in_=a1s[0][:])

            def hi(tau):
                # map absolute pipeline time to (head, iter), None past the end
                h, i = divmod(tau, NB)
                return (h, i) if 0 <= h < HG else None

            emit_khS(0)
            emit_sq(0, 0)
            for tau in range(HG * NB + 4):
                cur = hi(tau)
                if cur:
                    nxt = hi(tau + 1)
                    if nxt:
                        if nxt[1] == 0:
                            emit_khS(nxt[0])
                        emit_sq(*nxt)
                    emit_u(*cur)
                    emit_wt(*cur)
                pull(tau // NB + 1, 1)
                if cur:
                    emit_a1t(*cur)
                if hi(tau - 2):
                    emit_nupd(*hi(tau - 2))
                pull(tau // NB + 1, 1)
                if hi(tau - 3):
                    emit_d(*hi(tau - 3))
                if hi(tau - 4):
                    emit_o2(*hi(tau - 4))
            st_nsb.clear()
            oNs.clear()
            for i in range(NB):
                emit_final_tile(i)
            if DEBUG:
                nc.sync.dma_start(
                    out=dbg["oT"].rearrange("p (a b) -> p a b", a=HG), in_=oT[:])

            vp_cm.__exit__(None, None, None)

    nc.finalize()
    return nc


_CACHE = {}


def _get_program():
    if "nc" not in _CACHE:
        _CACHE["nc"] = _build_program()
    return _CACHE["nc"]


def _consts():
    if "consts" not in _CACHE:
        p_ = np.arange(128, dtype=np.float32)[:, None]
        c_ = np.arange(128, dtype=np.float32)[None, :]
        maskLE = (p_ <= c_).astype(NPBF)
        blk = np.arange(NB, dtype=np.float32)[None, :]
        invidx = (1.0 / (blk * 128.0 + p_ + 1.0)).astype(np.float32)
        ones1 = np.ones((1, 128), NPBF)
        _CACHE["consts"] = (maskLE, invidx, ones1)
    return _CACHE["consts"]


PROFILE = False
LAST_RESULTS = None


def kernel(v, k, q, p, wq_k, wq_b, wk_k, wk_b, wv_k, wv_b, wc_k, wc_b):
    global LAST_RESULTS
    nc = _get_program()
    maskLE, invidx, ones1 = _consts()

    qT = [np.ascontiguousarray(q[b].T).astype(NPBF) for b in range(B)]
    kT = [np.ascontiguousarray(k[b].T).astype(NPBF) for b in range(B)]
    vT = [np.ascontiguousarray(v[b].T).astype(NPBF) for b in range(B)]
    pT = [np.ascontiguousarray(p[b].T).astype(NPBF) for b in range(B)]
    wqc = wq_k.astype(NPBF)
    wkc = wk_k.astype(NPBF)
    wvc = wv_k.astype(NPBF)
    wcc = wc_k.astype(NPBF)

    in_maps = []
    for c in range(8):
        b, hg = c // 4, c % 4
        c0 = hg * DL
        wqb = np.ascontiguousarray(
            (wq_b[c0:c0 + DL].reshape(2, 128).T * NORM_D).astype(np.float32))
        wkb = np.ascontiguousarray(wk_b[c0:c0 + DL].reshape(2, 128).T.astype(np.float32))
        in_maps.append({
            "qT": qT[b], "kT": kT[b], "vT": vT[b],
            "pT": np.ascontiguousarray(pT[b][c0:c0 + DL]),
            "wq": np.ascontiguousarray(wqc[:, c0:c0 + DL]),
            "wk": np.ascontiguousarray(wkc[:, c0:c0 + DL]),
            "wv": np.ascontiguousarray(wvc[:, c0:c0 + DL]),
            "wc": np.ascontiguousarray(wcc[c0:c0 + DL, :]),
            "wqb": wqb, "wkb": wkb,
            "wvb": np.ascontiguousarray(wv_b[c0:c0 + DL].reshape(1, DL).astype(NPBF)),
            "ones1": ones1, "maskLE": maskLE, "invidx": invidx,
        })

    res = run_bass_kernel_spmd(
        nc, in_maps, core_ids=list(range(8)), trace=PROFILE)
    LAST_RESULTS = res

    out = np.zeros((B, S, DM), np.float32)
    for c in range(8):
        out[c // 4] += res.results[c]["out"].astype(np.float32)
    out += wc_b[None, None, :].astype(np.float32)
    return out


# revision 4
# speedup vs baseline: 1.0213x; 1.0154x over previous
"""Trainium2 Bass kernel for nn_MultiHeadAttention_75548474736720.

Linear-attention-style MHA with two causal prefix-sum bilinear forms,
evaluated with a chunked (linear-attention) reformulation instead of the
naive O(S^2)-blocks triangular matmuls:
  qh/kh/vh = projections, ph = split_heads(p)
  A1 = elu(qh ph^T) + 1
  U[t,j] = sum_{s<=t} Sq[t,s] A1[s,j],  Sq = qh kh^T  (1/(t+1) in exp scale)
  W' = exp(U/(t+1)), den = sum_j W'
  out2[t,d] = (1/((t+1) den[t])) sum_{s<=t} (W'[t].A1[s]) vh[s,d]

Chunked evaluation (128-row chunks, 256-row state snapshots):
  U:  M[d,j] = cumsum_s kh[s,d] A1[s,j] held in f32 PSUM, snapshotted to
      bf16 SBUF every 256 rows; U(i) = SqT-strips @ A1 + qh_i @ M_snap.
  S2: N[j,d] = cumsum_s A1[s,j] vh[s,d] as a bf16 SBUF running sum;
      D_ii = masked(A1_i W'_i^T) via transposed strips;
      out2(i) = W'_i @ N_{<i} + D_ii-contract vh_i, ACT-scaled by
      1/((t+1) den) so W' is never normalized explicitly.

All [row,col]->[col,row] layout changes (W'^T, A1^T, kh, oN->oT) run on the
DMA crossbar (dma_start_transpose), costing no PE/ACT/DVE time.  The four
heads run through one software-pipelined loop (stages lag 0/2/3/4) so each
head's S2 drain overlaps the next head's U phase; A1 generation for head
h+1 is pulled two units per step into head h's loop.

Sharding: 8 cores = (batch b in 0..1) x (head-group hg in 0..3, 4 heads
each).  Each core computes its 4 heads end-to-end (wq/wk/wv column-sliced,
wc row-sliced) and returns a partial [S, Dm] output in bf16; the host sums
partials per batch and adds the wc bias.
"""

import sys

sys.path.insert(0, "/opt/trn_rl_repo")

import ml_dtypes
import numpy as np

import concourse.bass as bass  # noqa: F401  (registers AP machinery)
import concourse.mybir as mybir
from concourse import bacc
from concourse.tile import TileContext
from concourse.bass_utils import run_bass_kernel_spmd

F32 = mybir.dt.float32
BF16 = mybir.dt.bfloat16
ACTF = mybir.ActivationFunctionType
ALU = mybir.AluOpType
NPBF = ml_dtypes.bfloat16

B, S, DM, H = 2, 1024, 1024, 16
D = DM // H            # 64, head dim
HG = 4                 # heads per core
DL = HG * D            # 256, local dm slice
NB = S // 128          # 8 s-blocks
NORM_D = 0.125         # 1/sqrt(D)

DEBUG = False


def _build_program():
    nc = bacc.Bacc(None, target_bir_lowering=False)

    qT_in = nc.declare_dram_parameter("qT", [DM, S], BF16, isOutput=False)
    kT_in = nc.declare_dram_parameter("kT", [DM, S], BF16, isOutput=False)
    vT_in = nc.declare_dram_parameter("vT", [DM, S], BF16, isOutput=False)
    pT_in = nc.declare_dram_parameter("pT", [DL, S], BF16, isOutput=False)
    wq_in = nc.declare_dram_parameter("wq", [DM, DL], BF16, isOutput=False)
    wk_in = nc.declare_dram_parameter("wk", [DM, DL], BF16, isOutput=False)
    wv_in = nc.declare_dram_parameter("wv", [DM, DL], BF16, isOutput=False)
    wc_in = nc.declare_dram_parameter("wc", [DL, S], BF16, isOutput=False)
    wqb_in = nc.declare_dram_parameter("wqb", [128, 2], F32, isOutput=False)
    wkb_in = nc.declare_dram_parameter("wkb", [128, 2], F32, isOutput=False)
    wvb_in = nc.declare_dram_parameter("wvb", [1, DL], BF16, isOutput=False)
    ones_in = nc.declare_dram_parameter("ones1", [1, 128], BF16, isOutput=False)
    mask_in = nc.declare_dram_parameter("maskLE", [128, 128], BF16, isOutput=False)
    inv_in = nc.declare_dram_parameter("invidx", [128, NB], F32, isOutput=False)
    out_d = nc.declare_dram_parameter("out", [S, DM], BF16, isOutput=True)
    dbg = {}
    if DEBUG:
        dbg["a1"] = nc.declare_dram_parameter("d_a1", [128, NB * S], F32, isOutput=True)
        dbg["den"] = nc.declare_dram_parameter("d_den", [128, NB], F32, isOutput=True)
        dbg["wtT"] = nc.declare_dram_parameter("d_wtT", [128, NB * S], F32, isOutput=True)
        dbg["nsb"] = nc.declare_dram_parameter("d_nsb", [128, NB * 64], F32, isOutput=True)
        dbg["msb"] = nc.declare_dram_parameter("d_msb", [128, 2 * 512], F32, isOutput=True)
        dbg["oT"] = nc.declare_dram_parameter("d_oT", [64, HG * S], F32, isOutput=True)

    with TileContext(nc) as tc:
        with tc.tile_pool(name="persist", bufs=1) as cp, \
             tc.tile_pool(name="pm", bufs=4, space="PSUM") as pm, \
             tc.tile_pool(name="scr", bufs=2) as sp:

            maskLE = cp.tile([128, 128], BF16)
            invidx = cp.tile([128, NB], F32)
            wqb = cp.tile([128, 2], F32)
            wkb = cp.tile([128, 2], F32)
            wvb = cp.tile([1, DL], BF16)
            ones1 = cp.tile([1, 128], BF16)
            pTt = cp.tile([128, 2, S], BF16)
            qhT = cp.tile([128, 2, S], BF16)
            khT = cp.tile([128, 2, S], BF16)
            vh = cp.tile([128, NB, DL], BF16)
            oT = cp.tile([128, 2, S], BF16)
            wct = cp.tile([128, 2, S], BF16)
            # double-buffered big per-head tensors
            a1s = [cp.tile([128, NB, S], BF16, name=f"a1_{x}") for x in range(2)]
            wtTs = [cp.tile([128, NB, S], BF16, name=f"wtT_{x}") for x in range(2)]

            # PSUM cumulative state (persists across the per-head loops);
            # N is accumulated in SBUF bf16 snapshots instead (value path)
            Mps = [pm.tile([128, 512], F32, tag=f"Mps{c}", bufs=1, name=f"Mps{c}")
                   for c in range(2)]

            # ---------------- projections ----------------
            vp_cm = tc.tile_pool(name="vproj", bufs=1)
            vp = vp_cm.__enter__()
            wvt = vp.tile([128, NB, DL], BF16)
            vTt = vp.tile([128, NB, S], BF16)
            with tc.tile_pool(name="proj", bufs=1) as jp:
                wqt = jp.tile([128, NB, DL], BF16)
                wkt = jp.tile([128, NB, DL], BF16)
                qTt = jp.tile([128, NB, S], BF16)
                kTt = jp.tile([128, NB, S], BF16)
                for wt_, wsrc, xt_, xsrc in ((wqt, wq_in, qTt, qT_in),
                                             (wkt, wk_in, kTt, kT_in),
                                             (wvt, wv_in, vTt, vT_in)):
                    for q4 in range(4):
                        kb = 2 * q4
                        nc.sync.dma_start(
                            out=wt_[:, kb:kb + 2, :],
                            in_=wsrc[kb * 128:(kb + 2) * 128, :].rearrange(
                                "(a p) d -> p a d", p=128))
                        nc.sync.dma_start(
                            out=xt_[:, kb:kb + 2, :],
                            in_=xsrc[kb * 128:(kb + 2) * 128, :].rearrange(
                                "(a p) t -> p a t", p=128))
                    if wt_ is wqt:
                        nc.sync.dma_start(
                            out=pTt[:], in_=pT_in.rearrange("(g p) t -> p g t", p=128))
                        nc.sync.dma_start(out=wqb[:], in_=wqb_in[:])
                        nc.sync.dma_start(out=invidx[:], in_=inv_in[:])
                    elif wt_ is wkt:
                        nc.sync.dma_start(out=maskLE[:], in_=mask_in[:])
                        nc.sync.dma_start(out=wkb[:], in_=wkb_in[:])
                    else:
                        nc.sync.dma_start(out=wvb[:], in_=wvb_in[:])
                        nc.sync.dma_start(out=ones1[:], in_=ones_in[:])
                        nc.sync.dma_start(
                            out=wct[:], in_=wc_in.rearrange("(a p) t -> p a t", p=128))

                # qhT[dm, t] = sum_c wq[c, dm] qT[c, t]  (+bias, * 1/sqrt(D))
                for wt_, xt_, dst, bias_t, scale in (
                    (wqt, qTt, qhT, wqb, NORM_D),
                    (wkt, kTt, khT, wkb, 1.0),
                ):
                    for g in range(2):
                        for n in range(2):
                            ps = pm.tile([128, 512], F32, tag="mm", name="ps_proj")
                            for kb in range(NB):
                                nc.tensor.matmul(
                                    ps[:], wt_[:, kb, g * 128:(g + 1) * 128],
                                    xt_[:, kb, n * 512:(n + 1) * 512],
                                    start=(kb == 0), stop=(kb == NB - 1))
                            nc.scalar.activation(
                                dst[:, g, n * 512:(n + 1) * 512], ps[:],
                                ACTF.Identity, bias=bias_t[:, g:g + 1], scale=scale)

            # ---------------- attention (4 heads, chunked) ----------------
            st_sq = {}      # (h,i) -> masked SqT_ii strip
            st_wb = {}      # (h,i) -> W' block (exp, unnormalized)
            st_gsc = {}     # (h,i) -> 1/((t+1) den) column
            st_a1t = {}     # (h,i) -> A1^T strip
            st_dsb = {}     # (h,i) -> masked D_ii^T
            st_nsb = {}     # (h,i) -> N snapshot through chunk i
            msbs = {}       # (c, i%2) -> M snapshot half
            khSs = {}       # h -> kh [s,d] strips
            oNs = {}

            def a1_gen(h):
                """A1 = elu(qh ph^T)+1 = min(exp(x),1) + relu(x); 16 units."""
                g, p0 = h // 2, (h % 2) * 64
                a1 = a1s[h % 2]
                for m in range(NB):
                    for c in range(2):
                        ps = pm.tile([128, 512], F32, tag="a1ps", bufs=2,
                                     name="ps_a1")
                        nc.tensor.matmul(
                            ps[:], qhT[p0:p0 + 64, g, m * 128:(m + 1) * 128],
                            pTt[p0:p0 + 64, g, c * 512:(c + 1) * 512],
                            start=True, stop=True)
                        e = sp.tile([128, 512], BF16, tag="e", bufs=6, name="e")
                        nc.scalar.activation(e[:], ps[:], ACTF.Exp)
                        e1 = sp.tile([128, 512], BF16, tag="e1", bufs=6, name="e1")
                        nc.gpsimd.tensor_scalar_min(e1[:], e[:], 1.0)
                        nc.vector.scalar_tensor_tensor(
                            a1[:, m, c * 512:(c + 1) * 512], ps[:], 0.0, e1[:],
                            ALU.max, ALU.add)
                        yield

            def emit_khS(h):
                g, p0 = h // 2, (h % 2) * 64
                khS = sp.tile([128, NB, 64], BF16, tag="khS", bufs=3, name="khS")
                nc.sync.dma_start_transpose(out=khS[:], in_=khT[p0:p0 + 64, g, :])
                khSs[h] = khS

            def emit_sq(h, i):
                # SqT strip [s in block si, t in block i]: si = i (masked diag)
                # plus si = i-1 (unmasked) for odd i, whose M snapshot lags.
                g, p0 = h // 2, (h % 2) * 64
                for si in ([i - 1, i] if i % 2 == 1 else [i]):
                    ps = pm.tile([128, 128], F32, tag="mm", name="ps_sq")
                    nc.tensor.matmul(
                        ps[:], khT[p0:p0 + 64, g, si * 128:(si + 1) * 128],
                        qhT[p0:p0 + 64, g, i * 128:(i + 1) * 128],
                        start=True, stop=True)
                    sq = sp.tile([128, 128], BF16, tag="sq", bufs=4, name="sq")
                    if si == i:
                        nc.vector.tensor_tensor(sq[:], ps[:], maskLE[:], ALU.mult)
                    else:
                        nc.vector.tensor_copy(sq[:], ps[:])
                    st_sq[(h, i, si)] = sq

            def emit_u(h, i):
                # U(i) = SqT_ii @ A1_i + qh_i @ M_{<i};  W' = exp(U/(t+1))
                # M[d,j] += kh_i^T A1_i afterwards (PSUM accum, snapshot to bf16)
                g, p0 = h // 2, (h % 2) * 64
                a1 = a1s[h % 2]
                wb = sp.tile([128, S], BF16, tag="wblk", bufs=4, name="wb")
                st_wb[(h, i)] = wb
                strips = [st_sq.pop(k) for k in
                          ([(h, i, i - 1), (h, i, i)] if i % 2 == 1
                           else [(h, i, i)])]
                mlag = 2 * (i // 2) - 1   # M snapshot (odd index) U(i) reads
                dps = []
                for c in range(2):
                    ps = pm.tile([128, 512], F32, tag="mm", name="ps_u")
                    for z, sq in enumerate(strips):
                        si = i - (len(strips) - 1 - z)
                        nc.tensor.matmul(ps[:], sq[:],
                                         a1[:, si, c * 512:(c + 1) * 512],
                                         start=(z == 0),
                                         stop=(z == len(strips) - 1 and mlag < 0))
                    if mlag >= 0:
                        nc.tensor.matmul(
                            ps[:], qhT[p0:p0 + 64, g, i * 128:(i + 1) * 128],
                            msbs[(c, (mlag // 2) % 2)][p0:p0 + 64, :],
                            start=False, stop=True)
                    dp = sp.tile([128, 1], F32, tag="dp", bufs=6, name="dp")
                    nc.scalar.activation(
                        wb[:, c * 512:(c + 1) * 512], ps[:], ACTF.Exp,
                        scale=invidx[:, i:i + 1], accum_out=dp[:])
                    dps.append(dp)
                # M update for chunk i (after U used M_{<i})
                for c in range(2):
                    nc.tensor.matmul(
                        Mps[c][p0:p0 + 64, :], khSs[h][:, i, :],
                        a1[:, i, c * 512:(c + 1) * 512],
                        start=(i == 0), stop=True)
                if i % 2 == 1 and i < NB - 1:
                    for c in range(2):
                        msb = sp.tile([128, 512], BF16, tag=f"msb{c}", bufs=3,
                                      name="msb")
                        nc.vector.tensor_copy(msb[p0:p0 + 64, :],
                                              Mps[c][p0:p0 + 64, :])
                        msbs[(c, (i // 2) % 2)] = msb
                # denominator -> gsc = 1/((t+1) den)
                dsum = sp.tile([128, 1], F32, tag="dsum", bufs=2, name="dsum")
                nc.vector.tensor_tensor(dsum[:], dps[0][:], dps[1][:], ALU.add)
                rec = sp.tile([128, 1], F32, tag="rec", bufs=2, name="rec")
                nc.vector.reciprocal(rec[:], dsum[:])
                gsc = sp.tile([128, 1], F32, tag="gsc", bufs=8, name="gsc")
                nc.vector.tensor_tensor(gsc[:], rec[:], invidx[:, i:i + 1],
                                        ALU.mult)
                st_gsc[(h, i)] = gsc
                if DEBUG and h == 0:
                    nc.vector.tensor_copy(dbg_den[:, i:i + 1], dsum[:])

            def emit_wt(h, i):
                wb = st_wb.pop((h, i))
                nc.sync.dma_start_transpose(
                    out=wtTs[h % 2][:, :, i * 128:(i + 1) * 128], in_=wb[:])

            def emit_a1t(h, i):
                a1 = a1s[h % 2]
                at = sp.tile([128, NB, 128], BF16, tag="a1T", bufs=5, name="a1T")
                nc.sync.dma_start_transpose(out=at[:], in_=a1[:, i, :])
                st_a1t[(h, i)] = at

            def emit_d(h, i):
                # D_ii^T[s',t'] = sum_j A1[s,j] W'[t,j], masked to s<=t
                at = st_a1t.pop((h, i))
                ps = pm.tile([128, 128], F32, tag="mm", name="ps_d")
                for k in range(NB):
                    nc.tensor.matmul(
                        ps[:], at[:, k, :],
                        wtTs[h % 2][:, k, i * 128:(i + 1) * 128],
                        start=(k == 0), stop=(k == NB - 1))
                dsb = sp.tile([128, 128], BF16, tag="dsb", bufs=4, name="dsb")
                nc.vector.tensor_tensor(dsb[:], ps[:], maskLE[:], ALU.mult)
                st_dsb[(h, i)] = dsb

            def emit_nupd(h, i):
                # N_i[j,d] = N_{i-1} + A1_i^T vh_i  (SBUF bf16 running sum)
                a1 = a1s[h % 2]
                d0 = h * 64
                ps = pm.tile([128, NB, 64], F32, tag="mm", name="ps_n")
                for k in range(NB):
                    nc.tensor.matmul(
                        ps[:, k, :], a1[:, i, k * 128:(k + 1) * 128],
                        vh[:, i, d0:d0 + 64], start=True, stop=True)
                nsb = sp.tile([128, NB, 64], BF16, tag="nsb", bufs=5, name="nsb")
                if i == 0:
                    nc.vector.tensor_copy(nsb[:], ps[:])
                else:
                    nc.vector.tensor_tensor(nsb[:], ps[:],
                                            st_nsb[(h, i - 1)][:], ALU.add)
                st_nsb[(h, i)] = nsb
                if DEBUG and h == 0 and i == NB - 1:
                    nc.sync.dma_start(out=dbg["nsb"].rearrange(
                        "p (a b) -> p a b", a=NB), in_=nsb[:])

            def emit_o2(h, i):
                # out2(i) = (W'_i @ N_{<i} + D^T-contract vh_i) * gsc
                d0 = h * 64
                if h % 2 == 0 and i == 0:
                    oNs[h // 2] = sp.tile([128, NB, 128], BF16, tag="oN",
                                          bufs=2, name="oN")
                oN = oNs[h // 2]
                ps = pm.tile([128, 64], F32, tag="mm", name="ps_o2")
                dsb = st_dsb.pop((h, i))
                if i > 0:
                    nsb = st_nsb[(h, i - 1)]
                    for k in range(NB):
                        nc.tensor.matmul(
                            ps[:], wtTs[h % 2][:, k, i * 128:(i + 1) * 128],
                            nsb[:, k, :], start=(k == 0), stop=False)
                    nc.tensor.matmul(ps[:], dsb[:], vh[:, i, d0:d0 + 64],
                                     start=False, stop=True)
                else:
                    nc.tensor.matmul(ps[:], dsb[:], vh[:, i, d0:d0 + 64],
                                     start=True, stop=True)
                if i >= 2:
                    st_nsb.pop((h, i - 2), None)
                nc.scalar.activation(
                    oN[:, i, (h % 2) * 64:(h % 2) * 64 + 64], ps[:],
                    ACTF.Copy, scale=st_gsc.pop((h, i))[:])
                if h % 2 == 1:
                    nc.sync.dma_start_transpose(
                        out=oT[:, h // 2, i * 128:(i + 1) * 128],
                        in_=oN[:, i, :])

            def emit_final_tile(i):
                for c in range(2):
                    ps = pm.tile([128, 512], F32, tag="mm", name="ps_fin")
                    for g2 in range(2):
                        nc.tensor.matmul(
                            ps[:], oT[:, g2, i * 128:(i + 1) * 128],
                            wct[:, g2, c * 512:(c + 1) * 512],
                            start=(g2 == 0), stop=(g2 == 1))
                    ot = sp.tile([128, 512], BF16, tag="ot", bufs=6, name="ot")
                    if (i + c) % 2 == 0:
                        nc.scalar.activation(ot[:], ps[:], ACTF.Copy)
                    else:
                        nc.vector.tensor_copy(ot[:], ps[:])
                    nc.sync.dma_start(
                        out=out_d[i * 128:(i + 1) * 128, c * 512:(c + 1) * 512],
                        in_=ot[:])

            if DEBUG:
                dbg_den = sp.tile([128, NB], F32, tag="dbgden", bufs=1,
                                  name="dbgden")

            # vh[s, d] = sum_c vT[c, s] wv[c, d] + wv_b[d], interleaved with
            # head 0's A1 so PE has work while vT streams in
            gens = {hh: a1_gen(hh) for hh in range(HG)}

            def pull(h, n):
                if h < HG:
                    for _ in range(n):
                        if next(gens[h], "done") == "done":
                            break

            def emit_vh(m):
                ps = pm.tile([128, DL], F32, tag="mm", name="ps_vh")
                for kb in range(NB):
                    nc.tensor.matmul(
                        ps[:], vTt[:, kb, m * 128:(m + 1) * 128], wvt[:, kb, :],
                        start=(kb == 0), stop=False)
                nc.tensor.matmul(ps[:], ones1[:], wvb[:], start=False, stop=True)
                nc.scalar.activation(vh[:, m, :], ps[:], ACTF.Copy)

            pull(0, 32)
            if DEBUG:
                nc.sync.dma_start(out=dbg["a1"].rearrange("p (a b) -> p a b", a=NB),
                                  in_=a1s[0][:])

            def hi(tau):
                # map absolute pipeline time to (head, iter), None past the end
                h, i = divmod(tau, NB)
                return (h, i) if 0 <= h < HG else None

            emit_khS(0)
            emit_sq(0, 0)
            for tau in range(HG * NB + 4):
                cur = hi(tau)
                if tau <= NB - 1:
                    emit_vh(tau)
                if cur:
                    nxt = hi(tau + 1)
                    if nxt:
                        if nxt[1] == 0:
                            emit_khS(nxt[0])
                        emit_sq(*nxt)
                    emit_u(*cur)
                    emit_wt(*cur)
                pull(tau // NB + 1, 1)
                if cur:
                    emit_a1t(*cur)
                if hi(tau - 2):
                    emit_nupd(*hi(tau - 2))
                pull(tau // NB + 1, 1)
                if hi(tau - 3):
                    emit_d(*hi(tau - 3))
                if hi(tau - 4):
                    emit_o2(*hi(tau - 4))
            st_nsb.clear()
            oNs.clear()
            for i in range(NB):
                emit_final_tile(i)
            if DEBUG:
                nc.sync.dma_start(
                    out=dbg["oT"].rearrange("p (a b) -> p a b", a=HG), in_=oT[:])

            vp_cm.__exit__(None, None, None)

    nc.finalize()
    return nc


_CACHE = {}


def _get_program():
    if "nc" not in _CACHE:
        _CACHE["nc"] = _build_program()
    return _CACHE["nc"]


def _consts():
    if "consts" not in _CACHE:
        p_ = np.arange(128, dtype=np.float32)[:, None]
        c_ = np.arange(128, dtype=np.float32)[None, :]
        maskLE = (p_ <= c_).astype(NPBF)
        blk = np.arange(NB, dtype=np.float32)[None, :]
        invidx = (1.0 / (blk * 128.0 + p_ + 1.0)).astype(np.float32)
        ones1 = np.ones((1, 128), NPBF)
        _CACHE["consts"] = (maskLE, invidx, ones1)
    return _CACHE["consts"]


PROFILE = False
LAST_RESULTS = None


def kernel(v, k, q, p, wq_k, wq_b, wk_k, wk_b, wv_k, wv_b, wc_k, wc_b):
    global LAST_RESULTS
    nc = _get_program()
    maskLE, invidx, ones1 = _consts()

    qT = [np.ascontiguousarray(q[b].T).astype(NPBF) for b in range(B)]
    kT = [np.ascontiguousarray(k[b].T).astype(NPBF) for b in range(B)]
    vT = [np.ascontiguousarray(v[b].T).astype(NPBF) for b in range(B)]
    pT = [np.ascontiguousarray(p[b].T).astype(NPBF) for b in range(B)]
    wqc = wq_k.astype(NPBF)
    wkc = wk_k.astype(NPBF)
    wvc = wv_k.astype(NPBF)
    wcc = wc_k.astype(NPBF)

    in_maps = []
    for c in range(8):
        b, hg = c // 4, c % 4
        c0 = hg * DL
        wqb = np.ascontiguousarray(
            (wq_b[c0:c0 + DL].reshape(2, 128).T * NORM_D).astype(np.float32))
        wkb = np.ascontiguousarray(wk_b[c0:c0 + DL].reshape(2, 128).T.astype(np.float32))
        in_maps.append({
            "qT": qT[b], "kT": kT[b], "vT": vT[b],
            "pT": np.ascontiguousarray(pT[b][c0:c0 + DL]),
            "wq": np.ascontiguousarray(wqc[:, c0:c0 + DL]),
            "wk": np.ascontiguousarray(wkc[:, c0:c0 + DL]),
            "wv": np.ascontiguousarray(wvc[:, c0:c0 + DL]),
            "wc": np.ascontiguousarray(wcc[c0:c0 + DL, :]),
            "wqb": wqb, "wkb": wkb,
            "wvb": np.ascontiguousarray(wv_b[c0:c0 + DL].reshape(1, DL).astype(NPBF)),
            "ones1": ones1, "maskLE": maskLE, "invidx": invidx,
        })

    res = run_bass_kernel_spmd(
        nc, in_maps, core_ids=list(range(8)), trace=PROFILE)
    LAST_RESULTS = res

    out = np.zeros((B, S, DM), np.float32)
    for c in range(8):
        out[c // 4] += res.results[c]["out"].astype(np.float32)
    out += wc_b[None, None, :].astype(np.float32)
    return out


# revision 5
# speedup vs baseline: 1.0229x; 1.0016x over previous
"""Trainium2 Bass kernel for nn_MultiHeadAttention_75548474736720.

Linear-attention-style MHA with two causal prefix-sum bilinear forms,
evaluated with a chunked (linear-attention) reformulation instead of the
naive O(S^2)-blocks triangular matmuls:
  qh/kh/vh = projections, ph = split_heads(p)
  A1 = elu(qh ph^T) + 1
  U[t,j] = sum_{s<=t} Sq[t,s] A1[s,j],  Sq = qh kh^T  (1/(t+1) in exp scale)
  W' = exp(U/(t+1)), den = sum_j W'
  out2[t,d] = (1/((t+1) den[t])) sum_{s<=t} (W'[t].A1[s]) vh[s,d]

Chunked evaluation (128-row chunks, 256-row state snapshots):
  U:  M[d,j] = cumsum_s kh[s,d] A1[s,j] held in f32 PSUM, snapshotted to
      bf16 SBUF every 256 rows; U(i) = SqT-strips @ A1 + qh_i @ M_snap.
  S2: N[j,d] = cumsum_s A1[s,j] vh[s,d] as a bf16 SBUF running sum;
      D_ii = masked(A1_i W'_i^T) via transposed strips;
      out2(i) = W'_i @ N_{<i} + D_ii-contract vh_i, ACT-scaled by
      1/((t+1) den) so W' is never normalized explicitly.

All [row,col]->[col,row] layout changes (W'^T, A1^T, kh, oN->oT) run on the
DMA crossbar (dma_start_transpose), costing no PE/ACT/DVE time.  The four
heads run through one software-pipelined loop (stages lag 0/2/3/4) so each
head's S2 drain overlaps the next head's U phase; A1 generation for head
h+1 is pulled two units per step into head h's loop.

Sharding: 8 cores = (batch b in 0..1) x (head-group hg in 0..3, 4 heads
each).  Each core computes its 4 heads end-to-end (wq/wk/wv column-sliced,
wc row-sliced) and returns a partial [S, Dm] output in bf16; the host sums
partials per batch and adds the wc bias.
"""

import sys

sys.path.insert(0, "/opt/trn_rl_repo")

import ml_dtypes
import numpy as np

import concourse.bass as bass  # noqa: F401  (registers AP machinery)
import concourse.mybir as mybir
from concourse import bacc
from concourse.tile import TileContext
from concourse.bass_utils import run_bass_kernel_spmd

F32 = mybir.dt.float32
BF16 = mybir.dt.bfloat16
ACTF = mybir.ActivationFunctionType
ALU = mybir.AluOpType
NPBF = ml_dtypes.bfloat16

B, S, DM, H = 2, 1024, 1024, 16
D = DM // H            # 64, head dim
HG = 4                 # heads per core
DL = HG * D            # 256, local dm slice
NB = S // 128          # 8 s-blocks
NORM_D = 0.125         # 1/sqrt(D)

DEBUG = False


def _build_program():
    nc = bacc.Bacc(None, target_bir_lowering=False)

    qT_in = nc.declare_dram_parameter("qT", [DM, S], BF16, isOutput=False)
    kT_in = nc.declare_dram_parameter("kT", [DM, S], BF16, isOutput=False)
    vT_in = nc.declare_dram_parameter("vT", [DM, S], BF16, isOutput=False)
    pT_in = nc.declare_dram_parameter("pT", [DL, S], BF16, isOutput=False)
    wq_in = nc.declare_dram_parameter("wq", [DM, DL], BF16, isOutput=False)
    wk_in = nc.declare_dram_parameter("wk", [DM, DL], BF16, isOutput=False)
    wv_in = nc.declare_dram_parameter("wv", [DM, DL], BF16, isOutput=False)
    wc_in = nc.declare_dram_parameter("wc", [DL, S], BF16, isOutput=False)
    wqb_in = nc.declare_dram_parameter("wqb", [128, 2], F32, isOutput=False)
    wkb_in = nc.declare_dram_parameter("wkb", [128, 2], F32, isOutput=False)
    wvb_in = nc.declare_dram_parameter("wvb", [1, DL], BF16, isOutput=False)
    ones_in = nc.declare_dram_parameter("ones1", [1, 128], BF16, isOutput=False)
    mask_in = nc.declare_dram_parameter("maskLE", [128, 128], BF16, isOutput=False)
    inv_in = nc.declare_dram_parameter("invidx", [128, NB], F32, isOutput=False)
    out_d = nc.declare_dram_parameter("out", [S, DM], BF16, isOutput=True)
    dbg = {}
    if DEBUG:
        dbg["a1"] = nc.declare_dram_parameter("d_a1", [128, NB * S], F32, isOutput=True)
        dbg["den"] = nc.declare_dram_parameter("d_den", [128, NB], F32, isOutput=True)
        dbg["wtT"] = nc.declare_dram_parameter("d_wtT", [128, NB * S], F32, isOutput=True)
        dbg["nsb"] = nc.declare_dram_parameter("d_nsb", [128, NB * 64], F32, isOutput=True)
        dbg["msb"] = nc.declare_dram_parameter("d_msb", [128, 2 * 512], F32, isOutput=True)
        dbg["oT"] = nc.declare_dram_parameter("d_oT", [64, HG * S], F32, isOutput=True)

    with TileContext(nc) as tc:
        with tc.tile_pool(name="persist", bufs=1) as cp, \
             tc.tile_pool(name="pm", bufs=4, space="PSUM") as pm, \
             tc.tile_pool(name="scr", bufs=2) as sp:

            maskLE = cp.tile([128, 128], BF16)
            invidx = cp.tile([128, NB], F32)
            wqb = cp.tile([128, 2], F32)
            wkb = cp.tile([128, 2], F32)
            wvb = cp.tile([1, DL], BF16)
            ones1 = cp.tile([1, 128], BF16)
            pTt = cp.tile([128, 2, S], BF16)
            qhT = cp.tile([128, 2, S], BF16)
            khT = cp.tile([128, 2, S], BF16)
            vh = cp.tile([128, NB, DL], BF16)
            oT = cp.tile([128, 2, S], BF16)
            wct = cp.tile([128, 2, S], BF16)
            # double-buffered big per-head tensors
            a1s = [cp.tile([128, NB, S], BF16, name=f"a1_{x}") for x in range(2)]
            wtTs = [cp.tile([128, NB, S], BF16, name=f"wtT_{x}") for x in range(2)]

            # PSUM cumulative state (persists across the per-head loops);
            # N is accumulated in SBUF bf16 snapshots instead (value path)
            Mps = [pm.tile([128, 512], F32, tag=f"Mps{c}", bufs=1, name=f"Mps{c}")
                   for c in range(2)]

            # ---------------- projections ----------------
            vp_cm = tc.tile_pool(name="vproj", bufs=1)
            vp = vp_cm.__enter__()
            wvt = vp.tile([128, NB, DL], BF16)
            vTt = vp.tile([128, NB, S], BF16)
            with tc.tile_pool(name="proj", bufs=1) as jp:
                wqt = jp.tile([128, NB, DL], BF16)
                wkt = jp.tile([128, NB, DL], BF16)
                qTt = jp.tile([128, NB, S], BF16)
                kTt = jp.tile([128, NB, S], BF16)
                for wt_, wsrc, xt_, xsrc in ((wqt, wq_in, qTt, qT_in),
                                             (wkt, wk_in, kTt, kT_in),
                                             (wvt, wv_in, vTt, vT_in)):
                    for q4 in range(4):
                        kb = 2 * q4
                        nc.sync.dma_start(
                            out=wt_[:, kb:kb + 2, :],
                            in_=wsrc[kb * 128:(kb + 2) * 128, :].rearrange(
                                "(a p) d -> p a d", p=128))
                        nc.sync.dma_start(
                            out=xt_[:, kb:kb + 2, :],
                            in_=xsrc[kb * 128:(kb + 2) * 128, :].rearrange(
                                "(a p) t -> p a t", p=128))
                    if wt_ is wqt:
                        nc.sync.dma_start(
                            out=pTt[:], in_=pT_in.rearrange("(g p) t -> p g t", p=128))
                        nc.sync.dma_start(out=wqb[:], in_=wqb_in[:])
                        nc.sync.dma_start(out=invidx[:], in_=inv_in[:])
                    elif wt_ is wkt:
                        nc.sync.dma_start(out=maskLE[:], in_=mask_in[:])
                        nc.sync.dma_start(out=wkb[:], in_=wkb_in[:])
                    else:
                        nc.sync.dma_start(out=wvb[:], in_=wvb_in[:])
                        nc.sync.dma_start(out=ones1[:], in_=ones_in[:])
                        nc.sync.dma_start(
                            out=wct[:], in_=wc_in.rearrange("(a p) t -> p a t", p=128))

                # qhT[dm, t] = sum_c wq[c, dm] qT[c, t]  (+bias, * 1/sqrt(D))
                for wt_, xt_, dst, bias_t, scale in (
                    (wqt, qTt, qhT, wqb, NORM_D),
                    (wkt, kTt, khT, wkb, 1.0),
                ):
                    for g in range(2):
                        for n in range(2):
                            ps = pm.tile([128, 512], F32, tag="mm", name="ps_proj")
                            for kb in range(NB):
                                nc.tensor.matmul(
                                    ps[:], wt_[:, kb, g * 128:(g + 1) * 128],
                                    xt_[:, kb, n * 512:(n + 1) * 512],
                                    start=(kb == 0), stop=(kb == NB - 1))
                            nc.scalar.activation(
                                dst[:, g, n * 512:(n + 1) * 512], ps[:],
                                ACTF.Identity, bias=bias_t[:, g:g + 1], scale=scale)

            # ---------------- attention (4 heads, chunked) ----------------
            st_sq = {}      # (h,i) -> masked SqT_ii strip
            st_wb = {}      # (h,i) -> W' block (exp, unnormalized)
            st_gsc = {}     # (h,i) -> 1/((t+1) den) column
            st_a1t = {}     # (h,i) -> A1^T strip
            st_dsb = {}     # (h,i) -> masked D_ii^T
            st_nsb = {}     # (h,i) -> N snapshot through chunk i
            msbs = {}       # (c, i%2) -> M snapshot half
            khSs = {}       # h -> kh [s,d] strips
            oNs = {}

            def a1_gen(h):
                """A1 = elu(qh ph^T)+1 = min(exp(x),1) + relu(x); 16 units."""
                g, p0 = h // 2, (h % 2) * 64
                a1 = a1s[h % 2]
                for m in range(NB):
                    for c in range(2):
                        ps = pm.tile([128, 512], F32, tag="a1ps", bufs=2,
                                     name="ps_a1")
                        nc.tensor.matmul(
                            ps[:], qhT[p0:p0 + 64, g, m * 128:(m + 1) * 128],
                            pTt[p0:p0 + 64, g, c * 512:(c + 1) * 512],
                            start=True, stop=True)
                        e = sp.tile([128, 512], BF16, tag="e", bufs=6, name="e")
                        nc.scalar.activation(e[:], ps[:], ACTF.Exp)
                        e1 = sp.tile([128, 512], BF16, tag="e1", bufs=6, name="e1")
                        nc.gpsimd.tensor_scalar_min(e1[:], e[:], 1.0)
                        nc.vector.scalar_tensor_tensor(
                            a1[:, m, c * 512:(c + 1) * 512], ps[:], 0.0, e1[:],
                            ALU.max, ALU.add)
                        yield

            def emit_khS(h):
                g, p0 = h // 2, (h % 2) * 64
                khS = sp.tile([128, NB, 64], BF16, tag="khS", bufs=3, name="khS")
                nc.sync.dma_start_transpose(out=khS[:], in_=khT[p0:p0 + 64, g, :])
                khSs[h] = khS

            def emit_sq(h, i):
                # SqT strip [s in block si, t in block i]: si = i (masked diag)
                # plus si = i-1 (unmasked) for odd i, whose M snapshot lags.
                g, p0 = h // 2, (h % 2) * 64
                for si in ([i - 1, i] if i % 2 == 1 else [i]):
                    ps = pm.tile([128, 128], F32, tag="mm", name="ps_sq")
                    nc.tensor.matmul(
                        ps[:], khT[p0:p0 + 64, g, si * 128:(si + 1) * 128],
                        qhT[p0:p0 + 64, g, i * 128:(i + 1) * 128],
                        start=True, stop=True)
                    sq = sp.tile([128, 128], BF16, tag="sq", bufs=4, name="sq")
                    if si == i:
                        nc.vector.tensor_tensor(sq[:], ps[:], maskLE[:], ALU.mult)
                    else:
                        nc.vector.tensor_copy(sq[:], ps[:])
                    st_sq[(h, i, si)] = sq

            def emit_u(h, i):
                # U(i) = SqT_ii @ A1_i + qh_i @ M_{<i};  W' = exp(U/(t+1))
                # M[d,j] += kh_i^T A1_i afterwards (PSUM accum, snapshot to bf16)
                g, p0 = h // 2, (h % 2) * 64
                a1 = a1s[h % 2]
                wb = sp.tile([128, S], BF16, tag="wblk", bufs=4, name="wb")
                st_wb[(h, i)] = wb
                strips = [st_sq.pop(k) for k in
                          ([(h, i, i - 1), (h, i, i)] if i % 2 == 1
                           else [(h, i, i)])]
                mlag = 2 * (i // 2) - 1   # M snapshot (odd index) U(i) reads
                dps = []
                for c in range(2):
                    ps = pm.tile([128, 512], F32, tag="mm", name="ps_u")
                    for z, sq in enumerate(strips):
                        si = i - (len(strips) - 1 - z)
                        nc.tensor.matmul(ps[:], sq[:],
                                         a1[:, si, c * 512:(c + 1) * 512],
                                         start=(z == 0),
                                         stop=(z == len(strips) - 1 and mlag < 0))
                    if mlag >= 0:
                        nc.tensor.matmul(
                            ps[:], qhT[p0:p0 + 64, g, i * 128:(i + 1) * 128],
                            msbs[(c, (mlag // 2) % 2)][p0:p0 + 64, :],
                            start=False, stop=True)
                    dp = sp.tile([128, 1], F32, tag="dp", bufs=6, name="dp")
                    nc.scalar.activation(
                        wb[:, c * 512:(c + 1) * 512], ps[:], ACTF.Exp,
                        scale=invidx[:, i:i + 1], accum_out=dp[:])
                    dps.append(dp)
                # M update for chunk i (after U used M_{<i})
                for c in range(2):
                    nc.tensor.matmul(
                        Mps[c][p0:p0 + 64, :], khSs[h][:, i, :],
                        a1[:, i, c * 512:(c + 1) * 512],
                        start=(i == 0), stop=True)
                if i % 2 == 1 and i < NB - 1:
                    for c in range(2):
                        msb = sp.tile([128, 512], BF16, tag=f"msb{c}", bufs=3,
                                      name="msb")
                        nc.vector.tensor_copy(msb[p0:p0 + 64, :],
                                              Mps[c][p0:p0 + 64, :])
                        msbs[(c, (i // 2) % 2)] = msb
                # denominator -> gsc = 1/((t+1) den)
                dsum = sp.tile([128, 1], F32, tag="dsum", bufs=2, name="dsum")
                nc.vector.tensor_tensor(dsum[:], dps[0][:], dps[1][:], ALU.add)
                rec = sp.tile([128, 1], F32, tag="rec", bufs=2, name="rec")
                nc.vector.reciprocal(rec[:], dsum[:])
                gsc = sp.tile([128, 1], F32, tag="gsc", bufs=8, name="gsc")
                nc.vector.tensor_tensor(gsc[:], rec[:], invidx[:, i:i + 1],
                                        ALU.mult)
                st_gsc[(h, i)] = gsc
                if DEBUG and h == 0:
                    nc.vector.tensor_copy(dbg_den[:, i:i + 1], dsum[:])

            def emit_wt(h, i):
                wb = st_wb.pop((h, i))
                nc.sync.dma_start_transpose(
                    out=wtTs[h % 2][:, :, i * 128:(i + 1) * 128], in_=wb[:])

            def emit_a1t(h, i):
                a1 = a1s[h % 2]
                at = sp.tile([128, NB, 128], BF16, tag="a1T", bufs=5, name="a1T")
                nc.sync.dma_start_transpose(out=at[:], in_=a1[:, i, :])
                st_a1t[(h, i)] = at

            def emit_d(h, i):
                # D_ii^T[s',t'] = sum_j A1[s,j] W'[t,j], masked to s<=t
                at = st_a1t.pop((h, i))
                ps = pm.tile([128, 128], F32, tag="mm", name="ps_d")
                for k in range(NB):
                    nc.tensor.matmul(
                        ps[:], at[:, k, :],
                        wtTs[h % 2][:, k, i * 128:(i + 1) * 128],
                        start=(k == 0), stop=(k == NB - 1))
                dsb = sp.tile([128, 128], BF16, tag="dsb", bufs=4, name="dsb")
                nc.vector.tensor_tensor(dsb[:], ps[:], maskLE[:], ALU.mult)
                st_dsb[(h, i)] = dsb

            def emit_nupd(h, i):
                # N_i[j,d] = N_{i-1} + A1_i^T vh_i  (SBUF bf16 running sum)
                a1 = a1s[h % 2]
                d0 = h * 64
                ps = pm.tile([128, NB, 64], F32, tag="mm", name="ps_n")
                for k in range(NB):
                    nc.tensor.matmul(
                        ps[:, k, :], a1[:, i, k * 128:(k + 1) * 128],
                        vh[:, i, d0:d0 + 64], start=True, stop=True)
                nsb = sp.tile([128, NB, 64], BF16, tag="nsb", bufs=5, name="nsb")
                if i == 0:
                    nc.vector.tensor_copy(nsb[:], ps[:])
                else:
                    nc.vector.tensor_tensor(nsb[:], ps[:],
                                            st_nsb[(h, i - 1)][:], ALU.add)
                st_nsb[(h, i)] = nsb
                if DEBUG and h == 0 and i == NB - 1:
                    nc.sync.dma_start(out=dbg["nsb"].rearrange(
                        "p (a b) -> p a b", a=NB), in_=nsb[:])

            def emit_o2(h, i):
                # out2(i) = (W'_i @ N_{<i} + D^T-contract vh_i) * gsc
                d0 = h * 64
                if h % 2 == 0 and i == 0:
                    oNs[h // 2] = sp.tile([128, NB, 128], BF16, tag="oN",
                                          bufs=2, name="oN")
                oN = oNs[h // 2]
                ps = pm.tile([128, 64], F32, tag="mm", name="ps_o2")
                dsb = st_dsb.pop((h, i))
                if i > 0:
                    nsb = st_nsb[(h, i - 1)]
                    for k in range(NB):
                        nc.tensor.matmul(
                            ps[:], wtTs[h % 2][:, k, i * 128:(i + 1) * 128],
                            nsb[:, k, :], start=(k == 0), stop=False)
                    nc.tensor.matmul(ps[:], dsb[:], vh[:, i, d0:d0 + 64],
                                     start=False, stop=True)
                else:
                    nc.tensor.matmul(ps[:], dsb[:], vh[:, i, d0:d0 + 64],
                                     start=True, stop=True)
                if i >= 2:
                    st_nsb.pop((h, i - 2), None)
                nc.scalar.activation(
                    oN[:, i, (h % 2) * 64:(h % 2) * 64 + 64], ps[:],
                    ACTF.Copy, scale=st_gsc.pop((h, i))[:])
                if h % 2 == 1:
                    nc.sync.dma_start_transpose(
                        out=oT[:, h // 2, i * 128:(i + 1) * 128],
                        in_=oN[:, i, :])

            def emit_final_tile(i):
                # i covers row-blocks 2i, 2i+1; one DMA per 256 output rows
                ot = sp.tile([128, 2, S], BF16, tag="ot", bufs=2, name="ot")
                for z in range(2):
                    ib = 2 * i + z
                    for c in range(2):
                        ps = pm.tile([128, 512], F32, tag="mm", name="ps_fin")
                        for g2 in range(2):
                            nc.tensor.matmul(
                                ps[:], oT[:, g2, ib * 128:(ib + 1) * 128],
                                wct[:, g2, c * 512:(c + 1) * 512],
                                start=(g2 == 0), stop=(g2 == 1))
                        if (ib + c) % 2 == 0:
                            nc.scalar.activation(
                                ot[:, z, c * 512:(c + 1) * 512], ps[:], ACTF.Copy)
                        else:
                            nc.vector.tensor_copy(
                                ot[:, z, c * 512:(c + 1) * 512], ps[:])
                nc.sync.dma_start(
                    out=out_d[2 * i * 128:(2 * i + 2) * 128, :].rearrange(
                        "(a p) d -> p a d", p=128),
                    in_=ot[:])

            if DEBUG:
                dbg_den = sp.tile([128, NB], F32, tag="dbgden", bufs=1,
                                  name="dbgden")

            # vh[s, d] = sum_c vT[c, s] wv[c, d] + wv_b[d], interleaved with
            # head 0's A1 so PE has work while vT streams in
            gens = {hh: a1_gen(hh) for hh in range(HG)}

            def pull(h, n):
                if h < HG:
                    for _ in range(n):
                        if next(gens[h], "done") == "done":
                            break

            def emit_vh(m):
                ps = pm.tile([128, DL], F32, tag="mm", name="ps_vh")
                for kb in range(NB):
                    nc.tensor.matmul(
                        ps[:], vTt[:, kb, m * 128:(m + 1) * 128], wvt[:, kb, :],
                        start=(kb == 0), stop=False)
                nc.tensor.matmul(ps[:], ones1[:], wvb[:], start=False, stop=True)
                nc.scalar.activation(vh[:, m, :], ps[:], ACTF.Copy)

            pull(0, 32)
            if DEBUG:
                nc.sync.dma_start(out=dbg["a1"].rearrange("p (a b) -> p a b", a=NB),
                                  in_=a1s[0][:])

            def hi(tau):
                # map absolute pipeline time to (head, iter), None past the end
                h, i = divmod(tau, NB)
                return (h, i) if 0 <= h < HG else None

            emit_khS(0)
            emit_sq(0, 0)
            for tau in range(HG * NB + 4):
                cur = hi(tau)
                if tau <= NB - 1:
                    emit_vh(tau)
                if cur:
                    nxt = hi(tau + 1)
                    if nxt:
                        if nxt[1] == 0:
                            emit_khS(nxt[0])
                        emit_sq(*nxt)
                    emit_u(*cur)
                    emit_wt(*cur)
                pull(tau // NB + 1, 1)
                if cur:
                    emit_a1t(*cur)
                if hi(tau - 2):
                    emit_nupd(*hi(tau - 2))
                pull(tau // NB + 1, 1)
                if hi(tau - 3):
                    emit_d(*hi(tau - 3))
                if hi(tau - 4):
                    emit_o2(*hi(tau - 4))
            st_nsb.clear()
            oNs.clear()
            for i in range(NB // 2):
                emit_final_tile(i)
            if DEBUG:
                nc.sync.dma_start(
                    out=dbg["oT"].rearrange("p (a b) -> p a b", a=HG), in_=oT[:])

            vp_cm.__exit__(None, None, None)

    nc.finalize()
    return nc


_CACHE = {}


def _get_program():
    if "nc" not in _CACHE:
        _CACHE["nc"] = _build_program()
    return _CACHE["nc"]


def _consts():
    if "consts" not in _CACHE:
        p_ = np.arange(128, dtype=np.float32)[:, None]
        c_ = np.arange(128, dtype=np.float32)[None, :]
        maskLE = (p_ <= c_).astype(NPBF)
        blk = np.arange(NB, dtype=np.float32)[None, :]
        invidx = (1.0 / (blk * 128.0 + p_ + 1.0)).astype(np.float32)
        ones1 = np.ones((1, 128), NPBF)
        _CACHE["consts"] = (maskLE, invidx, ones1)
    return _CACHE["consts"]


PROFILE = False
LAST_RESULTS = None


def kernel(v, k, q, p, wq_k, wq_b, wk_k, wk_b, wv_k, wv_b, wc_k, wc_b):
    global LAST_RESULTS
    nc = _get_program()
    maskLE, invidx, ones1 = _consts()

    qT = [np.ascontiguousarray(q[b].T).astype(NPBF) for b in range(B)]
    kT = [np.ascontiguousarray(k[b].T).astype(NPBF) for b in range(B)]
    vT = [np.ascontiguousarray(v[b].T).astype(NPBF) for b in range(B)]
    pT = [np.ascontiguousarray(p[b].T).astype(NPBF) for b in range(B)]
    wqc = wq_k.astype(NPBF)
    wkc = wk_k.astype(NPBF)
    wvc = wv_k.astype(NPBF)
    wcc = wc_k.astype(NPBF)

    in_maps = []
    for c in range(8):
        b, hg = c // 4, c % 4
        c0 = hg * DL
        wqb = np.ascontiguousarray(
            (wq_b[c0:c0 + DL].reshape(2, 128).T * NORM_D).astype(np.float32))
        wkb = np.ascontiguousarray(wk_b[c0:c0 + DL].reshape(2, 128).T.astype(np.float32))
        in_maps.append({
            "qT": qT[b], "kT": kT[b], "vT": vT[b],
            "pT": np.ascontiguousarray(pT[b][c0:c0 + DL]),
            "wq": np.ascontiguousarray(wqc[:, c0:c0 + DL]),
            "wk": np.ascontiguousarray(wkc[:, c0:c0 + DL]),
            "wv": np.ascontiguousarray(wvc[:, c0:c0 + DL]),
            "wc": np.ascontiguousarray(wcc[c0:c0 + DL, :]),
            "wqb": wqb, "wkb": wkb,
            "wvb": np.ascontiguousarray(wv_b[c0:c0 + DL].reshape(1, DL).astype(NPBF)),
            "ones1": ones1, "maskLE": maskLE, "invidx": invidx,
        })

    res = run_bass_kernel_spmd(
        nc, in_maps, core_ids=list(range(8)), trace=PROFILE)
    LAST_RESULTS = res

    out = np.zeros((B, S, DM), np.float32)
    for c in range(8):
        out[c // 4] += res.results[c]["out"].astype(np.float32)
    out += wc_b[None, None, :].astype(np.float32)
    return out


# revision 6
# speedup vs baseline: 1.0410x; 1.0177x over previous
"""Trainium2 Bass kernel for nn_MultiHeadAttention_75548474736720.

Linear-attention-style MHA with two causal prefix-sum bilinear forms,
evaluated with a chunked (linear-attention) reformulation instead of the
naive O(S^2)-blocks triangular matmuls:
  qh/kh/vh = projections, ph = split_heads(p)
  A1 = elu(qh ph^T) + 1
  U[t,j] = sum_{s<=t} Sq[t,s] A1[s,j],  Sq = qh kh^T  (1/(t+1) in exp scale)
  W' = exp(U/(t+1)), den = sum_j W'
  out2[t,d] = (1/((t+1) den[t])) sum_{s<=t} (W'[t].A1[s]) vh[s,d]

Chunked evaluation (128-row chunks, 256-row state snapshots):
  U:  M[d,j] = cumsum_s kh[s,d] A1[s,j] held in f32 PSUM, snapshotted to
      bf16 SBUF every 256 rows; U(i) = SqT-strips @ A1 + qh_i @ M_snap.
  S2: N[j,d] = cumsum_s A1[s,j] vh[s,d] as a bf16 SBUF running sum;
      D_ii = masked(A1_i W'_i^T) via transposed strips;
      out2(i) = W'_i @ N_{<i} + D_ii-contract vh_i, ACT-scaled by
      1/((t+1) den) so W' is never normalized explicitly.

All [row,col]->[col,row] layout changes (W'^T, A1^T, kh, oN->oT) run on the
DMA crossbar (dma_start_transpose), costing no PE/ACT/DVE time.  The four
heads run through one software-pipelined loop (stages lag 0/2/3/4) so each
head's S2 drain overlaps the next head's U phase; A1 generation for head
h+1 is pulled two units per step into head h's loop.

Sharding: 8 cores = (batch b in 0..1) x (head-group hg in 0..3, 4 heads
each).  Each core computes its 4 heads end-to-end (wq/wk/wv column-sliced,
wc row-sliced) and returns a partial [S, Dm] output in bf16; the host sums
partials per batch and adds the wc bias.
"""

import sys

sys.path.insert(0, "/opt/trn_rl_repo")

import ml_dtypes
import numpy as np

import concourse.bass as bass  # noqa: F401  (registers AP machinery)
import concourse.mybir as mybir
from concourse import bacc
from concourse.tile import TileContext
from concourse.bass_utils import run_bass_kernel_spmd

F32 = mybir.dt.float32
BF16 = mybir.dt.bfloat16
ACTF = mybir.ActivationFunctionType
ALU = mybir.AluOpType
NPBF = ml_dtypes.bfloat16

B, S, DM, H = 2, 1024, 1024, 16
D = DM // H            # 64, head dim
HG = 4                 # heads per core
DL = HG * D            # 256, local dm slice
NB = S // 128          # 8 s-blocks
NORM_D = 0.125         # 1/sqrt(D)

DEBUG = False


def _build_program():
    nc = bacc.Bacc(None, target_bir_lowering=False)

    qT_in = nc.declare_dram_parameter("qT", [DM, S], BF16, isOutput=False)
    kT_in = nc.declare_dram_parameter("kT", [DM, S], BF16, isOutput=False)
    vT_in = nc.declare_dram_parameter("vT", [DM, S], BF16, isOutput=False)
    pT_in = nc.declare_dram_parameter("pT", [DL, S], BF16, isOutput=False)
    wq_in = nc.declare_dram_parameter("wq", [DM, DL], BF16, isOutput=False)
    wk_in = nc.declare_dram_parameter("wk", [DM, DL], BF16, isOutput=False)
    wv_in = nc.declare_dram_parameter("wv", [DM, DL], BF16, isOutput=False)
    wc_in = nc.declare_dram_parameter("wc", [DL, S], BF16, isOutput=False)
    wqb_in = nc.declare_dram_parameter("wqb", [128, 2], F32, isOutput=False)
    wkb_in = nc.declare_dram_parameter("wkb", [128, 2], F32, isOutput=False)
    wvb_in = nc.declare_dram_parameter("wvb", [1, DL], BF16, isOutput=False)
    ones_in = nc.declare_dram_parameter("ones1", [1, 128], BF16, isOutput=False)
    mask_in = nc.declare_dram_parameter("maskLE", [128, 128], BF16, isOutput=False)
    ident_in = nc.declare_dram_parameter("ident", [128, 128], BF16, isOutput=False)
    inv_in = nc.declare_dram_parameter("invidx", [128, NB], F32, isOutput=False)
    out_d = nc.declare_dram_parameter("out", [S, DM], BF16, isOutput=True)
    dbg = {}
    if DEBUG:
        dbg["a1"] = nc.declare_dram_parameter("d_a1", [128, NB * S], F32, isOutput=True)
        dbg["den"] = nc.declare_dram_parameter("d_den", [128, NB], F32, isOutput=True)
        dbg["wtT"] = nc.declare_dram_parameter("d_wtT", [128, NB * S], F32, isOutput=True)
        dbg["nsb"] = nc.declare_dram_parameter("d_nsb", [128, NB * 64], F32, isOutput=True)
        dbg["msb"] = nc.declare_dram_parameter("d_msb", [128, 2 * 512], F32, isOutput=True)
        dbg["oT"] = nc.declare_dram_parameter("d_oT", [64, HG * S], F32, isOutput=True)

    with TileContext(nc) as tc:
        with tc.tile_pool(name="persist", bufs=1) as cp, \
             tc.tile_pool(name="pm", bufs=4, space="PSUM") as pm, \
             tc.tile_pool(name="scr", bufs=2) as sp:

            maskLE = cp.tile([128, 128], BF16)
            ident = cp.tile([128, 128], BF16)
            invidx = cp.tile([128, NB], F32)
            wqb = cp.tile([128, 2], F32)
            wkb = cp.tile([128, 2], F32)
            wvb = cp.tile([1, DL], BF16)
            ones1 = cp.tile([1, 128], BF16)
            pTt = cp.tile([128, 2, S], BF16)
            qhT = cp.tile([128, 2, S], BF16)
            khT = cp.tile([128, 2, S], BF16)
            vh = cp.tile([128, NB, DL], BF16)
            oT = cp.tile([128, 2, S], BF16)
            wct = cp.tile([128, 2, S], BF16)
            # double-buffered big per-head tensors
            a1s = [cp.tile([128, NB, S], BF16, name=f"a1_{x}") for x in range(2)]
            wtTs = [cp.tile([128, NB, S], BF16, name=f"wtT_{x}") for x in range(2)]

            # PSUM cumulative state (persists across the per-head loops);
            # N is accumulated in SBUF bf16 snapshots instead (value path)
            Mps = [pm.tile([128, 512], F32, tag=f"Mps{c}", bufs=1, name=f"Mps{c}")
                   for c in range(2)]

            # ---------------- projections ----------------
            vp_cm = tc.tile_pool(name="vproj", bufs=1)
            vp = vp_cm.__enter__()
            wvt = vp.tile([128, NB, DL], BF16)
            vTt = vp.tile([128, NB, S], BF16)
            with tc.tile_pool(name="proj", bufs=1) as jp:
                wqt = jp.tile([128, NB, DL], BF16)
                wkt = jp.tile([128, NB, DL], BF16)
                qTt = jp.tile([128, NB, S], BF16)
                kTt = jp.tile([128, NB, S], BF16)
                for wt_, wsrc, xt_, xsrc in ((wqt, wq_in, qTt, qT_in),
                                             (wkt, wk_in, kTt, kT_in),
                                             (wvt, wv_in, vTt, vT_in)):
                    for q4 in range(4):
                        kb = 2 * q4
                        nc.sync.dma_start(
                            out=wt_[:, kb:kb + 2, :],
                            in_=wsrc[kb * 128:(kb + 2) * 128, :].rearrange(
                                "(a p) d -> p a d", p=128))
                        nc.sync.dma_start(
                            out=xt_[:, kb:kb + 2, :],
                            in_=xsrc[kb * 128:(kb + 2) * 128, :].rearrange(
                                "(a p) t -> p a t", p=128))
                    if wt_ is wqt:
                        nc.sync.dma_start(
                            out=pTt[:], in_=pT_in.rearrange("(g p) t -> p g t", p=128))
                        nc.sync.dma_start(out=wqb[:], in_=wqb_in[:])
                        nc.sync.dma_start(out=invidx[:], in_=inv_in[:])
                    elif wt_ is wkt:
                        nc.sync.dma_start(out=maskLE[:], in_=mask_in[:])
                        nc.sync.dma_start(out=ident[:], in_=ident_in[:])
                        nc.sync.dma_start(out=wkb[:], in_=wkb_in[:])
                    else:
                        nc.sync.dma_start(out=wvb[:], in_=wvb_in[:])
                        nc.sync.dma_start(out=ones1[:], in_=ones_in[:])
                        nc.sync.dma_start(
                            out=wct[:], in_=wc_in.rearrange("(a p) t -> p a t", p=128))

                # qhT[dm, t] = sum_c wq[c, dm] qT[c, t]  (+bias, * 1/sqrt(D))
                for wt_, xt_, dst, bias_t, scale in (
                    (wqt, qTt, qhT, wqb, NORM_D),
                    (wkt, kTt, khT, wkb, 1.0),
                ):
                    for g in range(2):
                        for n in range(2):
                            ps = pm.tile([128, 512], F32, tag="mm", name="ps_proj")
                            for kb in range(NB):
                                nc.tensor.matmul(
                                    ps[:], wt_[:, kb, g * 128:(g + 1) * 128],
                                    xt_[:, kb, n * 512:(n + 1) * 512],
                                    start=(kb == 0), stop=(kb == NB - 1))
                            nc.scalar.activation(
                                dst[:, g, n * 512:(n + 1) * 512], ps[:],
                                ACTF.Identity, bias=bias_t[:, g:g + 1], scale=scale)

            # ---------------- attention (4 heads, chunked) ----------------
            st_sq = {}      # (h,i) -> masked SqT_ii strip
            st_wb = {}      # (h,i) -> W' block (exp, unnormalized)
            st_gsc = {}     # (h,i) -> 1/((t+1) den) column
            st_a1t = {}     # (h,i) -> A1^T strip
            st_dsb = {}     # (h,i) -> masked D_ii^T
            st_nsb = {}     # (h,i) -> N snapshot through chunk i
            msbs = {}       # (c, i%2) -> M snapshot half
            khSs = {}       # h -> kh [s,d] strips
            oNs = {}

            def a1_gen(h):
                """A1 = elu(qh ph^T)+1 = min(exp(x),1) + relu(x); 16 units."""
                g, p0 = h // 2, (h % 2) * 64
                a1 = a1s[h % 2]
                for m in range(NB):
                    for c in range(2):
                        ps = pm.tile([128, 512], F32, tag="a1ps", bufs=2,
                                     name="ps_a1")
                        nc.tensor.matmul(
                            ps[:], qhT[p0:p0 + 64, g, m * 128:(m + 1) * 128],
                            pTt[p0:p0 + 64, g, c * 512:(c + 1) * 512],
                            start=True, stop=True)
                        e = sp.tile([128, 512], BF16, tag="e", bufs=5, name="e")
                        nc.scalar.activation(e[:], ps[:], ACTF.Exp)
                        e1 = sp.tile([128, 512], BF16, tag="e1", bufs=5, name="e1")
                        nc.gpsimd.tensor_scalar_min(e1[:], e[:], 1.0)
                        nc.vector.scalar_tensor_tensor(
                            a1[:, m, c * 512:(c + 1) * 512], ps[:], 0.0, e1[:],
                            ALU.max, ALU.add)
                        yield

            def emit_khS(h):
                g, p0 = h // 2, (h % 2) * 64
                khS = sp.tile([128, NB, 64], BF16, tag="khS", bufs=3, name="khS")
                nc.sync.dma_start_transpose(out=khS[:], in_=khT[p0:p0 + 64, g, :])
                khSs[h] = khS

            def emit_sq(h, i):
                # SqT strip [s in block si, t in block i]: si = i (masked diag)
                # plus si = i-1 (unmasked) for odd i, whose M snapshot lags.
                g, p0 = h // 2, (h % 2) * 64
                for si in ([i - 1, i] if i % 2 == 1 else [i]):
                    ps = pm.tile([128, 128], F32, tag="mm", name="ps_sq")
                    nc.tensor.matmul(
                        ps[:], khT[p0:p0 + 64, g, si * 128:(si + 1) * 128],
                        qhT[p0:p0 + 64, g, i * 128:(i + 1) * 128],
                        start=True, stop=True)
                    sq = sp.tile([128, 128], BF16, tag="sq", bufs=4, name="sq")
                    if si == i:
                        nc.vector.tensor_tensor(sq[:], ps[:], maskLE[:], ALU.mult)
                    else:
                        nc.vector.tensor_copy(sq[:], ps[:])
                    st_sq[(h, i, si)] = sq

            def emit_u(h, i):
                # U(i) = SqT_ii @ A1_i + qh_i @ M_{<i};  W' = exp(U/(t+1))
                # M[d,j] += kh_i^T A1_i afterwards (PSUM accum, snapshot to bf16)
                g, p0 = h // 2, (h % 2) * 64
                a1 = a1s[h % 2]
                wb = sp.tile([128, S], BF16, tag="wblk", bufs=4, name="wb")
                st_wb[(h, i)] = wb
                strips = [st_sq.pop(k) for k in
                          ([(h, i, i - 1), (h, i, i)] if i % 2 == 1
                           else [(h, i, i)])]
                mlag = 2 * (i // 2) - 1   # M snapshot (odd index) U(i) reads
                dps = []
                for c in range(2):
                    ps = pm.tile([128, 512], F32, tag="mm", name="ps_u")
                    for z, sq in enumerate(strips):
                        si = i - (len(strips) - 1 - z)
                        nc.tensor.matmul(ps[:], sq[:],
                                         a1[:, si, c * 512:(c + 1) * 512],
                                         start=(z == 0),
                                         stop=(z == len(strips) - 1 and mlag < 0))
                    if mlag >= 0:
                        nc.tensor.matmul(
                            ps[:], qhT[p0:p0 + 64, g, i * 128:(i + 1) * 128],
                            msbs[(c, (mlag // 2) % 2)][p0:p0 + 64, :],
                            start=False, stop=True)
                    dp = sp.tile([128, 1], F32, tag="dp", bufs=6, name="dp")
                    nc.scalar.activation(
                        wb[:, c * 512:(c + 1) * 512], ps[:], ACTF.Exp,
                        scale=invidx[:, i:i + 1], accum_out=dp[:])
                    dps.append(dp)
                # M update for chunk i (after U used M_{<i})
                for c in range(2):
                    nc.tensor.matmul(
                        Mps[c][p0:p0 + 64, :], khSs[h][:, i, :],
                        a1[:, i, c * 512:(c + 1) * 512],
                        start=(i == 0), stop=True)
                if i % 2 == 1 and i < NB - 1:
                    for c in range(2):
                        msb = sp.tile([128, 512], BF16, tag=f"msb{c}", bufs=3,
                                      name="msb")
                        nc.vector.tensor_copy(msb[p0:p0 + 64, :],
                                              Mps[c][p0:p0 + 64, :])
                        msbs[(c, (i // 2) % 2)] = msb
                # denominator -> gsc = 1/((t+1) den)
                dsum = sp.tile([128, 1], F32, tag="dsum", bufs=2, name="dsum")
                nc.vector.tensor_tensor(dsum[:], dps[0][:], dps[1][:], ALU.add)
                rec = sp.tile([128, 1], F32, tag="rec", bufs=2, name="rec")
                nc.vector.reciprocal(rec[:], dsum[:])
                gsc = sp.tile([128, 1], F32, tag="gsc", bufs=8, name="gsc")
                nc.vector.tensor_tensor(gsc[:], rec[:], invidx[:, i:i + 1],
                                        ALU.mult)
                st_gsc[(h, i)] = gsc
                if DEBUG and h == 0:
                    nc.vector.tensor_copy(dbg_den[:, i:i + 1], dsum[:])

            def emit_wt(h, i):
                wb = st_wb.pop((h, i))
                if h == HG - 1 and i == NB - 1:
                    # tail-critical: PE transpose dodges the xbar DMA latency
                    tps = pm.tile([128, S], BF16, tag="mm", name="tps")
                    for k in range(NB):
                        nc.tensor.transpose(
                            tps[:, k * 128:(k + 1) * 128],
                            wb[:, k * 128:(k + 1) * 128], ident[:])
                    nc.vector.tensor_copy(
                        wtTs[h % 2][:, :, i * 128:(i + 1) * 128],
                        tps[:].rearrange("p (a b) -> p a b", a=NB))
                else:
                    nc.sync.dma_start_transpose(
                        out=wtTs[h % 2][:, :, i * 128:(i + 1) * 128], in_=wb[:])

            def emit_a1t(h, i):
                a1 = a1s[h % 2]
                at = sp.tile([128, NB, 128], BF16, tag="a1T", bufs=5, name="a1T")
                nc.sync.dma_start_transpose(out=at[:], in_=a1[:, i, :])
                st_a1t[(h, i)] = at

            def emit_d(h, i):
                # D_ii^T[s',t'] = sum_j A1[s,j] W'[t,j], masked to s<=t
                at = st_a1t.pop((h, i))
                ps = pm.tile([128, 128], F32, tag="mm", name="ps_d")
                for k in range(NB):
                    nc.tensor.matmul(
                        ps[:], at[:, k, :],
                        wtTs[h % 2][:, k, i * 128:(i + 1) * 128],
                        start=(k == 0), stop=(k == NB - 1))
                dsb = sp.tile([128, 128], BF16, tag="dsb", bufs=4, name="dsb")
                nc.vector.tensor_tensor(dsb[:], ps[:], maskLE[:], ALU.mult)
                st_dsb[(h, i)] = dsb

            def emit_nupd(h, i):
                # N_i[j,d] = N_{i-1} + A1_i^T vh_i  (SBUF bf16 running sum)
                a1 = a1s[h % 2]
                d0 = h * 64
                ps = pm.tile([128, NB, 64], F32, tag="mm", name="ps_n")
                for k in range(NB):
                    nc.tensor.matmul(
                        ps[:, k, :], a1[:, i, k * 128:(k + 1) * 128],
                        vh[:, i, d0:d0 + 64], start=True, stop=True)
                nsb = sp.tile([128, NB, 64], BF16, tag="nsb", bufs=5, name="nsb")
                if i == 0:
                    nc.vector.tensor_copy(nsb[:], ps[:])
                else:
                    nc.vector.tensor_tensor(nsb[:], ps[:],
                                            st_nsb[(h, i - 1)][:], ALU.add)
                st_nsb[(h, i)] = nsb
                if DEBUG and h == 0 and i == NB - 1:
                    nc.sync.dma_start(out=dbg["nsb"].rearrange(
                        "p (a b) -> p a b", a=NB), in_=nsb[:])

            def emit_o2(h, i):
                # out2(i) = (W'_i @ N_{<i} + D^T-contract vh_i) * gsc
                d0 = h * 64
                if h % 2 == 0 and i == 0:
                    oNs[h // 2] = sp.tile([128, NB, 128], BF16, tag="oN",
                                          bufs=2, name="oN")
                oN = oNs[h // 2]
                ps = pm.tile([128, 64], F32, tag="mm", name="ps_o2")
                dsb = st_dsb.pop((h, i))
                if i > 0:
                    nsb = st_nsb[(h, i - 1)]
                    for k in range(NB):
                        nc.tensor.matmul(
                            ps[:], wtTs[h % 2][:, k, i * 128:(i + 1) * 128],
                            nsb[:, k, :], start=(k == 0), stop=False)
                    nc.tensor.matmul(ps[:], dsb[:], vh[:, i, d0:d0 + 64],
                                     start=False, stop=True)
                else:
                    nc.tensor.matmul(ps[:], dsb[:], vh[:, i, d0:d0 + 64],
                                     start=True, stop=True)
                if i >= 2:
                    st_nsb.pop((h, i - 2), None)
                nc.scalar.activation(
                    oN[:, i, (h % 2) * 64:(h % 2) * 64 + 64], ps[:],
                    ACTF.Copy, scale=st_gsc.pop((h, i))[:])
                if h % 2 == 1:
                    if h == HG - 1 and i == NB - 1:
                        tpo = pm.tile([128, 128], BF16, tag="mm", name="tpo")
                        nc.tensor.transpose(tpo[:], oN[:, i, :], ident[:])
                        nc.scalar.activation(
                            oT[:, h // 2, i * 128:(i + 1) * 128], tpo[:],
                            ACTF.Copy)
                    else:
                        nc.sync.dma_start_transpose(
                            out=oT[:, h // 2, i * 128:(i + 1) * 128],
                            in_=oN[:, i, :])

            def emit_final_tile(i):
                # i covers row-blocks 2i, 2i+1; one DMA per 256 output rows
                ot = sp.tile([128, 2, S], BF16, tag="ot", bufs=2, name="ot")
                for z in range(2):
                    ib = 2 * i + z
                    for c in range(2):
                        ps = pm.tile([128, 512], F32, tag="mm", name="ps_fin")
                        for g2 in range(2):
                            nc.tensor.matmul(
                                ps[:], oT[:, g2, ib * 128:(ib + 1) * 128],
                                wct[:, g2, c * 512:(c + 1) * 512],
                                start=(g2 == 0), stop=(g2 == 1))
                        if (ib + c) % 2 == 0:
                            nc.scalar.activation(
                                ot[:, z, c * 512:(c + 1) * 512], ps[:], ACTF.Copy)
                        else:
                            nc.vector.tensor_copy(
                                ot[:, z, c * 512:(c + 1) * 512], ps[:])
                nc.sync.dma_start(
                    out=out_d[2 * i * 128:(2 * i + 2) * 128, :].rearrange(
                        "(a p) d -> p a d", p=128),
                    in_=ot[:])

            if DEBUG:
                dbg_den = sp.tile([128, NB], F32, tag="dbgden", bufs=1,
                                  name="dbgden")

            # vh[s, d] = sum_c vT[c, s] wv[c, d] + wv_b[d], interleaved with
            # head 0's A1 so PE has work while vT streams in
            gens = {hh: a1_gen(hh) for hh in range(HG)}

            def pull(h, n):
                if h < HG:
                    for _ in range(n):
                        if next(gens[h], "done") == "done":
                            break

            def emit_vh(m):
                ps = pm.tile([128, DL], F32, tag="mm", name="ps_vh")
                for kb in range(NB):
                    nc.tensor.matmul(
                        ps[:], vTt[:, kb, m * 128:(m + 1) * 128], wvt[:, kb, :],
                        start=(kb == 0), stop=False)
                nc.tensor.matmul(ps[:], ones1[:], wvb[:], start=False, stop=True)
                nc.scalar.activation(vh[:, m, :], ps[:], ACTF.Copy)

            pull(0, 32)
            if DEBUG:
                nc.sync.dma_start(out=dbg["a1"].rearrange("p (a b) -> p a b", a=NB),
                                  in_=a1s[0][:])

            def hi(tau):
                # map absolute pipeline time to (head, iter), None past the end
                h, i = divmod(tau, NB)
                return (h, i) if 0 <= h < HG else None

            emit_khS(0)
            emit_sq(0, 0)
            for tau in range(HG * NB + 4):
                cur = hi(tau)
                if tau <= NB - 1:
                    emit_vh(tau)
                if cur:
                    nxt = hi(tau + 1)
                    if nxt:
                        if nxt[1] == 0:
                            emit_khS(nxt[0])
                        emit_sq(*nxt)
                    emit_u(*cur)
                    emit_wt(*cur)
                pull(tau // NB + 1, 1)
                if cur:
                    emit_a1t(*cur)
                if hi(tau - 2):
                    emit_nupd(*hi(tau - 2))
                pull(tau // NB + 1, 1)
                if hi(tau - 3):
                    emit_d(*hi(tau - 3))
                if hi(tau - 4):
                    emit_o2(*hi(tau - 4))
            st_nsb.clear()
            oNs.clear()
            for i in range(NB // 2):
                emit_final_tile(i)
            if DEBUG:
                nc.sync.dma_start(
                    out=dbg["oT"].rearrange("p (a b) -> p a b", a=HG), in_=oT[:])

            vp_cm.__exit__(None, None, None)

    nc.finalize()
    return nc


_CACHE = {}


def _get_program():
    if "nc" not in _CACHE:
        _CACHE["nc"] = _build_program()
    return _CACHE["nc"]


def _consts():
    if "consts" not in _CACHE:
        p_ = np.arange(128, dtype=np.float32)[:, None]
        c_ = np.arange(128, dtype=np.float32)[None, :]
        maskLE = (p_ <= c_).astype(NPBF)
        ident = np.eye(128, dtype=np.float32).astype(NPBF)
        blk = np.arange(NB, dtype=np.float32)[None, :]
        invidx = (1.0 / (blk * 128.0 + p_ + 1.0)).astype(np.float32)
        ones1 = np.ones((1, 128), NPBF)
        _CACHE["consts"] = (maskLE, ident, invidx, ones1)
    return _CACHE["consts"]


PROFILE = False
LAST_RESULTS = None


def kernel(v, k, q, p, wq_k, wq_b, wk_k, wk_b, wv_k, wv_b, wc_k, wc_b):
    global LAST_RESULTS
    nc = _get_program()
    maskLE, ident, invidx, ones1 = _consts()

    qT = [np.ascontiguousarray(q[b].T).astype(NPBF) for b in range(B)]
    kT = [np.ascontiguousarray(k[b].T).astype(NPBF) for b in range(B)]
    vT = [np.ascontiguousarray(v[b].T).astype(NPBF) for b in range(B)]
    pT = [np.ascontiguousarray(p[b].T).astype(NPBF) for b in range(B)]
    wqc = wq_k.astype(NPBF)
    wkc = wk_k.astype(NPBF)
    wvc = wv_k.astype(NPBF)
    wcc = wc_k.astype(NPBF)

    in_maps = []
    for c in range(8):
        b, hg = c // 4, c % 4
        c0 = hg * DL
        wqb = np.ascontiguousarray(
            (wq_b[c0:c0 + DL].reshape(2, 128).T * NORM_D).astype(np.float32))
        wkb = np.ascontiguousarray(wk_b[c0:c0 + DL].reshape(2, 128).T.astype(np.float32))
        in_maps.append({
            "qT": qT[b], "kT": kT[b], "vT": vT[b],
            "pT": np.ascontiguousarray(pT[b][c0:c0 + DL]),
            "wq": np.ascontiguousarray(wqc[:, c0:c0 + DL]),
            "wk": np.ascontiguousarray(wkc[:, c0:c0 + DL]),
            "wv": np.ascontiguousarray(wvc[:, c0:c0 + DL]),
            "wc": np.ascontiguousarray(wcc[c0:c0 + DL, :]),
            "wqb": wqb, "wkb": wkb,
            "wvb": np.ascontiguousarray(wv_b[c0:c0 + DL].reshape(1, DL).astype(NPBF)),
            "ones1": ones1, "maskLE": maskLE, "ident": ident,
            "invidx": invidx,
        })

    res = run_bass_kernel_spmd(
        nc, in_maps, core_ids=list(range(8)), trace=PROFILE)
    LAST_RESULTS = res

    out = np.zeros((B, S, DM), np.float32)
    for c in range(8):
        out[c // 4] += res.results[c]["out"].astype(np.float32)
    out += wc_b[None, None, :].astype(np.float32)
    return out


# revision 7
# speedup vs baseline: 1.0416x; 1.0006x over previous
"""Trainium2 Bass kernel for nn_MultiHeadAttention_75548474736720.

Linear-attention-style MHA with two causal prefix-sum bilinear forms,
evaluated with a chunked (linear-attention) reformulation instead of the
naive O(S^2)-blocks triangular matmuls:
  qh/kh/vh = projections, ph = split_heads(p)
  A1 = elu(qh ph^T) + 1
  U[t,j] = sum_{s<=t} Sq[t,s] A1[s,j],  Sq = qh kh^T  (1/(t+1) in exp scale)
  W' = exp(U/(t+1)), den = sum_j W'
  out2[t,d] = (1/((t+1) den[t])) sum_{s<=t} (W'[t].A1[s]) vh[s,d]

Chunked evaluation (128-row chunks, 256-row state snapshots):
  U:  M[d,j] = cumsum_s kh[s,d] A1[s,j] held in f32 PSUM, snapshotted to
      bf16 SBUF every 256 rows; U(i) = SqT-strips @ A1 + qh_i @ M_snap.
  S2: N[j,d] = cumsum_s A1[s,j] vh[s,d] as a bf16 SBUF running sum;
      D_ii = masked(A1_i W'_i^T) via transposed strips;
      out2(i) = W'_i @ N_{<i} + D_ii-contract vh_i, ACT-scaled by
      1/((t+1) den) so W' is never normalized explicitly.

All [row,col]->[col,row] layout changes (W'^T, A1^T, kh, oN->oT) run on the
DMA crossbar (dma_start_transpose), costing no PE/ACT/DVE time.  The four
heads run through one software-pipelined loop (stages lag 0/2/3/4) so each
head's S2 drain overlaps the next head's U phase; A1 generation for head
h+1 is pulled two units per step into head h's loop.

Sharding: 8 cores = (batch b in 0..1) x (head-group hg in 0..3, 4 heads
each).  Each core computes its 4 heads end-to-end (wq/wk/wv column-sliced,
wc row-sliced) and returns a partial [S, Dm] output in bf16; the host sums
partials per batch and adds the wc bias.
"""

import sys

sys.path.insert(0, "/opt/trn_rl_repo")

import ml_dtypes
import numpy as np

import concourse.bass as bass  # noqa: F401  (registers AP machinery)
import concourse.mybir as mybir
from concourse import bacc
from concourse.tile import TileContext
from concourse.bass_utils import run_bass_kernel_spmd

F32 = mybir.dt.float32
BF16 = mybir.dt.bfloat16
ACTF = mybir.ActivationFunctionType
ALU = mybir.AluOpType
NPBF = ml_dtypes.bfloat16

B, S, DM, H = 2, 1024, 1024, 16
D = DM // H            # 64, head dim
HG = 4                 # heads per core
DL = HG * D            # 256, local dm slice
NB = S // 128          # 8 s-blocks
NORM_D = 0.125         # 1/sqrt(D)

DEBUG = False


def _build_program():
    nc = bacc.Bacc(None, target_bir_lowering=False)

    qT_in = nc.declare_dram_parameter("qT", [DM, S], BF16, isOutput=False)
    kT_in = nc.declare_dram_parameter("kT", [DM, S], BF16, isOutput=False)
    vT_in = nc.declare_dram_parameter("vT", [DM, S], BF16, isOutput=False)
    pT_in = nc.declare_dram_parameter("pT", [DL, S], BF16, isOutput=False)
    wq_in = nc.declare_dram_parameter("wq", [DM, DL], BF16, isOutput=False)
    wk_in = nc.declare_dram_parameter("wk", [DM, DL], BF16, isOutput=False)
    wv_in = nc.declare_dram_parameter("wv", [DM, DL], BF16, isOutput=False)
    wc_in = nc.declare_dram_parameter("wc", [DL, S], BF16, isOutput=False)
    wqb_in = nc.declare_dram_parameter("wqb", [128, 2], F32, isOutput=False)
    wkb_in = nc.declare_dram_parameter("wkb", [128, 2], F32, isOutput=False)
    wvb_in = nc.declare_dram_parameter("wvb", [1, DL], BF16, isOutput=False)
    ones_in = nc.declare_dram_parameter("ones1", [1, 128], BF16, isOutput=False)
    mask_in = nc.declare_dram_parameter("maskLE", [128, 128], BF16, isOutput=False)
    ident_in = nc.declare_dram_parameter("ident", [128, 128], BF16, isOutput=False)
    inv_in = nc.declare_dram_parameter("invidx", [128, NB], F32, isOutput=False)
    out_d = nc.declare_dram_parameter("out", [S, DM], BF16, isOutput=True)
    dbg = {}
    if DEBUG:
        dbg["a1"] = nc.declare_dram_parameter("d_a1", [128, NB * S], F32, isOutput=True)
        dbg["den"] = nc.declare_dram_parameter("d_den", [128, NB], F32, isOutput=True)
        dbg["wtT"] = nc.declare_dram_parameter("d_wtT", [128, NB * S], F32, isOutput=True)
        dbg["nsb"] = nc.declare_dram_parameter("d_nsb", [128, NB * 64], F32, isOutput=True)
        dbg["msb"] = nc.declare_dram_parameter("d_msb", [128, 2 * 512], F32, isOutput=True)
        dbg["oT"] = nc.declare_dram_parameter("d_oT", [64, HG * S], F32, isOutput=True)

    with TileContext(nc) as tc:
        with tc.tile_pool(name="persist", bufs=1) as cp, \
             tc.tile_pool(name="pm", bufs=4, space="PSUM") as pm, \
             tc.tile_pool(name="scr", bufs=2) as sp:

            maskLE = cp.tile([128, 128], BF16)
            ident = cp.tile([128, 128], BF16)
            invidx = cp.tile([128, NB], F32)
            wqb = cp.tile([128, 2], F32)
            wkb = cp.tile([128, 2], F32)
            wvb = cp.tile([1, DL], BF16)
            ones1 = cp.tile([1, 128], BF16)
            pTt = cp.tile([128, 2, S], BF16)
            qhT = cp.tile([128, 2, S], BF16)
            khT = cp.tile([128, 2, S], BF16)
            vh = cp.tile([128, NB, DL], BF16)
            oT = cp.tile([128, 2, S], BF16)
            wct = cp.tile([128, 2, S], BF16)
            # double-buffered big per-head tensors
            a1s = [cp.tile([128, NB, S], BF16, name=f"a1_{x}") for x in range(2)]
            wtTs = [cp.tile([128, NB, S], BF16, name=f"wtT_{x}") for x in range(2)]

            # PSUM cumulative state (persists across the per-head loops);
            # N is accumulated in SBUF bf16 snapshots instead (value path)
            Mps = [pm.tile([128, 512], F32, tag=f"Mps{c}", bufs=1, name=f"Mps{c}")
                   for c in range(2)]

            # ---------------- projections ----------------
            vp_cm = tc.tile_pool(name="vproj", bufs=1)
            vp = vp_cm.__enter__()
            wvt = vp.tile([128, NB, DL], BF16)
            vTt = vp.tile([128, NB, S], BF16)
            with tc.tile_pool(name="proj", bufs=1) as jp:
                wqt = jp.tile([128, NB, DL], BF16)
                wkt = jp.tile([128, NB, DL], BF16)
                qTt = jp.tile([128, NB, S], BF16)
                kTt = jp.tile([128, NB, S], BF16)
                for wt_, wsrc, xt_, xsrc in ((wqt, wq_in, qTt, qT_in),
                                             (wkt, wk_in, kTt, kT_in),
                                             (wvt, wv_in, vTt, vT_in)):
                    for q4 in range(4):
                        kb = 2 * q4
                        nc.sync.dma_start(
                            out=wt_[:, kb:kb + 2, :],
                            in_=wsrc[kb * 128:(kb + 2) * 128, :].rearrange(
                                "(a p) d -> p a d", p=128))
                        nc.sync.dma_start(
                            out=xt_[:, kb:kb + 2, :],
                            in_=xsrc[kb * 128:(kb + 2) * 128, :].rearrange(
                                "(a p) t -> p a t", p=128))
                    if wt_ is wqt:
                        nc.sync.dma_start(
                            out=pTt[:], in_=pT_in.rearrange("(g p) t -> p g t", p=128))
                        nc.sync.dma_start(out=wqb[:], in_=wqb_in[:])
                        nc.sync.dma_start(out=invidx[:], in_=inv_in[:])
                    elif wt_ is wkt:
                        nc.sync.dma_start(out=maskLE[:], in_=mask_in[:])
                        nc.sync.dma_start(out=ident[:], in_=ident_in[:])
                        nc.sync.dma_start(out=wkb[:], in_=wkb_in[:])
                    else:
                        nc.sync.dma_start(out=wvb[:], in_=wvb_in[:])
                        nc.sync.dma_start(out=ones1[:], in_=ones_in[:])
                        nc.sync.dma_start(
                            out=wct[:], in_=wc_in.rearrange("(a p) t -> p a t", p=128))

                # qhT[dm, t] = sum_c wq[c, dm] qT[c, t]  (+bias, * 1/sqrt(D))
                for wt_, xt_, dst, bias_t, scale in (
                    (wqt, qTt, qhT, wqb, NORM_D),
                    (wkt, kTt, khT, wkb, 1.0),
                ):
                    for g in range(2):
                        for n in range(2):
                            ps = pm.tile([128, 512], F32, tag="mm", name="ps_proj")
                            for kb in range(NB):
                                nc.tensor.matmul(
                                    ps[:], wt_[:, kb, g * 128:(g + 1) * 128],
                                    xt_[:, kb, n * 512:(n + 1) * 512],
                                    start=(kb == 0), stop=(kb == NB - 1))
                            nc.scalar.activation(
                                dst[:, g, n * 512:(n + 1) * 512], ps[:],
                                ACTF.Identity, bias=bias_t[:, g:g + 1], scale=scale)

            # ---------------- attention (4 heads, chunked) ----------------
            st_sq = {}      # (h,i) -> masked SqT_ii strip
            st_wb = {}      # (h,i) -> W' block (exp, unnormalized)
            st_gsc = {}     # (h,i) -> 1/((t+1) den) column
            st_a1t = {}     # (h,i) -> A1^T strip
            st_dsb = {}     # (h,i) -> masked D_ii^T
            st_nsb = {}     # (h,i) -> N snapshot through chunk i
            msbs = {}       # (c, i%2) -> M snapshot half
            khSs = {}       # h -> kh [s,d] strips
            oNs = {}

            def a1_gen(h):
                """A1 = elu(qh ph^T)+1 = min(exp(x),1) + relu(x); 16 units."""
                g, p0 = h // 2, (h % 2) * 64
                a1 = a1s[h % 2]
                for m in range(NB):
                    for c in range(2):
                        ps = pm.tile([128, 512], F32, tag="a1ps", bufs=2,
                                     name="ps_a1")
                        nc.tensor.matmul(
                            ps[:], qhT[p0:p0 + 64, g, m * 128:(m + 1) * 128],
                            pTt[p0:p0 + 64, g, c * 512:(c + 1) * 512],
                            start=True, stop=True)
                        e = sp.tile([128, 512], BF16, tag="e", bufs=5, name="e")
                        nc.scalar.activation(e[:], ps[:], ACTF.Exp)
                        e1 = sp.tile([128, 512], BF16, tag="e1", bufs=5, name="e1")
                        nc.gpsimd.tensor_scalar_min(e1[:], e[:], 1.0)
                        nc.vector.scalar_tensor_tensor(
                            a1[:, m, c * 512:(c + 1) * 512], ps[:], 0.0, e1[:],
                            ALU.max, ALU.add)
                        yield

            def emit_khS(h):
                g, p0 = h // 2, (h % 2) * 64
                khS = sp.tile([128, NB, 64], BF16, tag="khS", bufs=3, name="khS")
                nc.sync.dma_start_transpose(out=khS[:], in_=khT[p0:p0 + 64, g, :])
                khSs[h] = khS

            def emit_sq(h, i):
                # SqT strip [s in block si, t in block i]: si = i (masked diag)
                # plus si = i-1 (unmasked) for odd i, whose M snapshot lags.
                g, p0 = h // 2, (h % 2) * 64
                for si in ([i - 1, i] if i % 2 == 1 else [i]):
                    ps = pm.tile([128, 128], F32, tag="mm", name="ps_sq")
                    nc.tensor.matmul(
                        ps[:], khT[p0:p0 + 64, g, si * 128:(si + 1) * 128],
                        qhT[p0:p0 + 64, g, i * 128:(i + 1) * 128],
                        start=True, stop=True)
                    sq = sp.tile([128, 128], BF16, tag="sq", bufs=4, name="sq")
                    if si == i:
                        nc.vector.tensor_tensor(sq[:], ps[:], maskLE[:], ALU.mult)
                    else:
                        nc.vector.tensor_copy(sq[:], ps[:])
                    st_sq[(h, i, si)] = sq

            def emit_u(h, i):
                # U(i) = SqT_ii @ A1_i + qh_i @ M_{<i};  W' = exp(U/(t+1))
                # M[d,j] += kh_i^T A1_i afterwards (PSUM accum, snapshot to bf16)
                g, p0 = h // 2, (h % 2) * 64
                a1 = a1s[h % 2]
                wb = sp.tile([128, S], BF16, tag="wblk", bufs=4, name="wb")
                st_wb[(h, i)] = wb
                strips = [st_sq.pop(k) for k in
                          ([(h, i, i - 1), (h, i, i)] if i % 2 == 1
                           else [(h, i, i)])]
                mlag = 2 * (i // 2) - 1   # M snapshot (odd index) U(i) reads
                dps = []
                for c in range(2):
                    ps = pm.tile([128, 512], F32, tag="mm", name="ps_u")
                    for z, sq in enumerate(strips):
                        si = i - (len(strips) - 1 - z)
                        nc.tensor.matmul(ps[:], sq[:],
                                         a1[:, si, c * 512:(c + 1) * 512],
                                         start=(z == 0),
                                         stop=(z == len(strips) - 1 and mlag < 0))
                    if mlag >= 0:
                        nc.tensor.matmul(
                            ps[:], qhT[p0:p0 + 64, g, i * 128:(i + 1) * 128],
                            msbs[(c, (mlag // 2) % 2)][p0:p0 + 64, :],
                            start=False, stop=True)
                    dp = sp.tile([128, 1], F32, tag="dp", bufs=6, name="dp")
                    nc.scalar.activation(
                        wb[:, c * 512:(c + 1) * 512], ps[:], ACTF.Exp,
                        scale=invidx[:, i:i + 1], accum_out=dp[:])
                    dps.append(dp)
                # M update for chunk i (after U used M_{<i}); chunks >=
                # NB-2 are past the last snapshot and never read
                if i < NB - 2:
                    for c in range(2):
                        nc.tensor.matmul(
                            Mps[c][p0:p0 + 64, :], khSs[h][:, i, :],
                            a1[:, i, c * 512:(c + 1) * 512],
                            start=(i == 0), stop=True)
                if i % 2 == 1 and i < NB - 1:
                    for c in range(2):
                        msb = sp.tile([128, 512], BF16, tag=f"msb{c}", bufs=3,
                                      name="msb")
                        nc.vector.tensor_copy(msb[p0:p0 + 64, :],
                                              Mps[c][p0:p0 + 64, :])
                        msbs[(c, (i // 2) % 2)] = msb
                # denominator -> gsc = 1/((t+1) den)
                dsum = sp.tile([128, 1], F32, tag="dsum", bufs=2, name="dsum")
                nc.vector.tensor_tensor(dsum[:], dps[0][:], dps[1][:], ALU.add)
                rec = sp.tile([128, 1], F32, tag="rec", bufs=2, name="rec")
                nc.vector.reciprocal(rec[:], dsum[:])
                gsc = sp.tile([128, 1], F32, tag="gsc", bufs=8, name="gsc")
                nc.vector.tensor_tensor(gsc[:], rec[:], invidx[:, i:i + 1],
                                        ALU.mult)
                st_gsc[(h, i)] = gsc
                if DEBUG and h == 0:
                    nc.vector.tensor_copy(dbg_den[:, i:i + 1], dsum[:])

            def emit_wt(h, i):
                wb = st_wb.pop((h, i))
                if h == HG - 1 and i == NB - 1:
                    # tail-critical: PE transpose dodges the xbar DMA latency
                    tps = pm.tile([128, S], BF16, tag="mm", name="tps")
                    for k in range(NB):
                        nc.tensor.transpose(
                            tps[:, k * 128:(k + 1) * 128],
                            wb[:, k * 128:(k + 1) * 128], ident[:])
                    nc.vector.tensor_copy(
                        wtTs[h % 2][:, :, i * 128:(i + 1) * 128],
                        tps[:].rearrange("p (a b) -> p a b", a=NB))
                else:
                    nc.sync.dma_start_transpose(
                        out=wtTs[h % 2][:, :, i * 128:(i + 1) * 128], in_=wb[:])

            def emit_a1t(h, i):
                a1 = a1s[h % 2]
                at = sp.tile([128, NB, 128], BF16, tag="a1T", bufs=5, name="a1T")
                nc.sync.dma_start_transpose(out=at[:], in_=a1[:, i, :])
                st_a1t[(h, i)] = at

            def emit_d(h, i):
                # D_ii^T[s',t'] = sum_j A1[s,j] W'[t,j], masked to s<=t
                at = st_a1t.pop((h, i))
                ps = pm.tile([128, 128], F32, tag="mm", name="ps_d")
                for k in range(NB):
                    nc.tensor.matmul(
                        ps[:], at[:, k, :],
                        wtTs[h % 2][:, k, i * 128:(i + 1) * 128],
                        start=(k == 0), stop=(k == NB - 1))
                dsb = sp.tile([128, 128], BF16, tag="dsb", bufs=4, name="dsb")
                nc.vector.tensor_tensor(dsb[:], ps[:], maskLE[:], ALU.mult)
                st_dsb[(h, i)] = dsb

            def emit_nupd(h, i):
                # N_i[j,d] = N_{i-1} + A1_i^T vh_i  (SBUF bf16 running sum);
                # the last chunk's update feeds nothing
                if i == NB - 1:
                    return
                a1 = a1s[h % 2]
                d0 = h * 64
                ps = pm.tile([128, NB, 64], F32, tag="mm", name="ps_n")
                for k in range(NB):
                    nc.tensor.matmul(
                        ps[:, k, :], a1[:, i, k * 128:(k + 1) * 128],
                        vh[:, i, d0:d0 + 64], start=True, stop=True)
                nsb = sp.tile([128, NB, 64], BF16, tag="nsb", bufs=5, name="nsb")
                if i == 0:
                    nc.vector.tensor_copy(nsb[:], ps[:])
                else:
                    nc.vector.tensor_tensor(nsb[:], ps[:],
                                            st_nsb[(h, i - 1)][:], ALU.add)
                st_nsb[(h, i)] = nsb
                if DEBUG and h == 0 and i == NB - 1:
                    nc.sync.dma_start(out=dbg["nsb"].rearrange(
                        "p (a b) -> p a b", a=NB), in_=nsb[:])

            def emit_o2(h, i):
                # out2(i) = (W'_i @ N_{<i} + D^T-contract vh_i) * gsc
                d0 = h * 64
                if h % 2 == 0 and i == 0:
                    oNs[h // 2] = sp.tile([128, NB, 128], BF16, tag="oN",
                                          bufs=2, name="oN")
                oN = oNs[h // 2]
                ps = pm.tile([128, 64], F32, tag="mm", name="ps_o2")
                dsb = st_dsb.pop((h, i))
                if i > 0:
                    nsb = st_nsb[(h, i - 1)]
                    for k in range(NB):
                        nc.tensor.matmul(
                            ps[:], wtTs[h % 2][:, k, i * 128:(i + 1) * 128],
                            nsb[:, k, :], start=(k == 0), stop=False)
                    nc.tensor.matmul(ps[:], dsb[:], vh[:, i, d0:d0 + 64],
                                     start=False, stop=True)
                else:
                    nc.tensor.matmul(ps[:], dsb[:], vh[:, i, d0:d0 + 64],
                                     start=True, stop=True)
                if i >= 2:
                    st_nsb.pop((h, i - 2), None)
                nc.scalar.activation(
                    oN[:, i, (h % 2) * 64:(h % 2) * 64 + 64], ps[:],
                    ACTF.Copy, scale=st_gsc.pop((h, i))[:])
                if h % 2 == 1:
                    if h == HG - 1 and i == NB - 1:
                        tpo = pm.tile([128, 128], BF16, tag="mm", name="tpo")
                        nc.tensor.transpose(tpo[:], oN[:, i, :], ident[:])
                        nc.scalar.activation(
                            oT[:, h // 2, i * 128:(i + 1) * 128], tpo[:],
                            ACTF.Copy)
                    else:
                        nc.sync.dma_start_transpose(
                            out=oT[:, h // 2, i * 128:(i + 1) * 128],
                            in_=oN[:, i, :])

            def emit_final_tile(i):
                # i covers row-blocks 2i, 2i+1; one DMA per 256 output rows
                ot = sp.tile([128, 2, S], BF16, tag="ot", bufs=2, name="ot")
                for z in range(2):
                    ib = 2 * i + z
                    for c in range(2):
                        ps = pm.tile([128, 512], F32, tag="mm", name="ps_fin")
                        for g2 in range(2):
                            nc.tensor.matmul(
                                ps[:], oT[:, g2, ib * 128:(ib + 1) * 128],
                                wct[:, g2, c * 512:(c + 1) * 512],
                                start=(g2 == 0), stop=(g2 == 1))
                        if (ib + c) % 2 == 0:
                            nc.scalar.activation(
                                ot[:, z, c * 512:(c + 1) * 512], ps[:], ACTF.Copy)
                        else:
                            nc.vector.tensor_copy(
                                ot[:, z, c * 512:(c + 1) * 512], ps[:])
                nc.sync.dma_start(
                    out=out_d[2 * i * 128:(2 * i + 2) * 128, :].rearrange(
                        "(a p) d -> p a d", p=128),
                    in_=ot[:])

            if DEBUG:
                dbg_den = sp.tile([128, NB], F32, tag="dbgden", bufs=1,
                                  name="dbgden")

            # vh[s, d] = sum_c vT[c, s] wv[c, d] + wv_b[d], interleaved with
            # head 0's A1 so PE has work while vT streams in
            gens = {hh: a1_gen(hh) for hh in range(HG)}

            def pull(h, n):
                if h < HG:
                    for _ in range(n):
                        if next(gens[h], "done") == "done":
                            break

            def emit_vh(m):
                ps = pm.tile([128, DL], F32, tag="mm", name="ps_vh")
                for kb in range(NB):
                    nc.tensor.matmul(
                        ps[:], vTt[:, kb, m * 128:(m + 1) * 128], wvt[:, kb, :],
                        start=(kb == 0), stop=False)
                nc.tensor.matmul(ps[:], ones1[:], wvb[:], start=False, stop=True)
                nc.scalar.activation(vh[:, m, :], ps[:], ACTF.Copy)

            pull(0, 32)
            if DEBUG:
                nc.sync.dma_start(out=dbg["a1"].rearrange("p (a b) -> p a b", a=NB),
                                  in_=a1s[0][:])

            def hi(tau):
                # map absolute pipeline time to (head, iter), None past the end
                h, i = divmod(tau, NB)
                return (h, i) if 0 <= h < HG else None

            emit_khS(0)
            emit_sq(0, 0)
            for tau in range(HG * NB + 4):
                cur = hi(tau)
                if tau <= NB - 1:
                    emit_vh(tau)
                if cur:
                    nxt = hi(tau + 1)
                    if nxt:
                        if nxt[1] == 0:
                            emit_khS(nxt[0])
                        emit_sq(*nxt)
                    emit_u(*cur)
                    emit_wt(*cur)
                pull(tau // NB + 1, 1)
                if cur:
                    emit_a1t(*cur)
                if hi(tau - 2):
                    emit_nupd(*hi(tau - 2))
                pull(tau // NB + 1, 1)
                if hi(tau - 3):
                    emit_d(*hi(tau - 3))
                if hi(tau - 4):
                    emit_o2(*hi(tau - 4))
            st_nsb.clear()
            oNs.clear()
            for i in range(NB // 2):
                emit_final_tile(i)
            if DEBUG:
                nc.sync.dma_start(
                    out=dbg["oT"].rearrange("p (a b) -> p a b", a=HG), in_=oT[:])

            vp_cm.__exit__(None, None, None)

    nc.finalize()
    return nc


_CACHE = {}


def _get_program():
    if "nc" not in _CACHE:
        _CACHE["nc"] = _build_program()
    return _CACHE["nc"]


def _consts():
    if "consts" not in _CACHE:
        p_ = np.arange(128, dtype=np.float32)[:, None]
        c_ = np.arange(128, dtype=np.float32)[None, :]
        maskLE = (p_ <= c_).astype(NPBF)
        ident = np.eye(128, dtype=np.float32).astype(NPBF)
        blk = np.arange(NB, dtype=np.float32)[None, :]
        invidx = (1.0 / (blk * 128.0 + p_ + 1.0)).astype(np.float32)
        ones1 = np.ones((1, 128), NPBF)
        _CACHE["consts"] = (maskLE, ident, invidx, ones1)
    return _CACHE["consts"]


PROFILE = False
LAST_RESULTS = None


def kernel(v, k, q, p, wq_k, wq_b, wk_k, wk_b, wv_k, wv_b, wc_k, wc_b):
    global LAST_RESULTS
    nc = _get_program()
    maskLE, ident, invidx, ones1 = _consts()

    qT = [np.ascontiguousarray(q[b].T).astype(NPBF) for b in range(B)]
    kT = [np.ascontiguousarray(k[b].T).astype(NPBF) for b in range(B)]
    vT = [np.ascontiguousarray(v[b].T).astype(NPBF) for b in range(B)]
    pT = [np.ascontiguousarray(p[b].T).astype(NPBF) for b in range(B)]
    wqc = wq_k.astype(NPBF)
    wkc = wk_k.astype(NPBF)
    wvc = wv_k.astype(NPBF)
    wcc = wc_k.astype(NPBF)

    in_maps = []
    for c in range(8):
        b, hg = c // 4, c % 4
        c0 = hg * DL
        wqb = np.ascontiguousarray(
            (wq_b[c0:c0 + DL].reshape(2, 128).T * NORM_D).astype(np.float32))
        wkb = np.ascontiguousarray(wk_b[c0:c0 + DL].reshape(2, 128).T.astype(np.float32))
        in_maps.append({
            "qT": qT[b], "kT": kT[b], "vT": vT[b],
            "pT": np.ascontiguousarray(pT[b][c0:c0 + DL]),
            "wq": np.ascontiguousarray(wqc[:, c0:c0 + DL]),
            "wk": np.ascontiguousarray(wkc[:, c0:c0 + DL]),
            "wv": np.ascontiguousarray(wvc[:, c0:c0 + DL]),
            "wc": np.ascontiguousarray(wcc[c0:c0 + DL, :]),
            "wqb": wqb, "wkb": wkb,
            "wvb": np.ascontiguousarray(wv_b[c0:c0 + DL].reshape(1, DL).astype(NPBF)),
            "ones1": ones1, "maskLE": maskLE, "ident": ident,
            "invidx": invidx,
        })

    res = run_bass_kernel_spmd(
        nc, in_maps, core_ids=list(range(8)), trace=PROFILE)
    LAST_RESULTS = res

    out = np.zeros((B, S, DM), np.float32)
    for c in range(8):
        out[c // 4] += res.results[c]["out"].astype(np.float32)
    out += wc_b[None, None, :].astype(np.float32)
    return out


# revision 8
# speedup vs baseline: 1.0433x; 1.0016x over previous
"""Trainium2 Bass kernel for nn_MultiHeadAttention_75548474736720.

Linear-attention-style MHA with two causal prefix-sum bilinear forms,
evaluated with a chunked (linear-attention) reformulation instead of the
naive O(S^2)-blocks triangular matmuls:
  qh/kh/vh = projections, ph = split_heads(p)
  A1 = elu(qh ph^T) + 1
  U[t,j] = sum_{s<=t} Sq[t,s] A1[s,j],  Sq = qh kh^T  (1/(t+1) in exp scale)
  W' = exp(U/(t+1)), den = sum_j W'
  out2[t,d] = (1/((t+1) den[t])) sum_{s<=t} (W'[t].A1[s]) vh[s,d]

Chunked evaluation (128-row chunks, 256-row state snapshots):
  U:  M[d,j] = cumsum_s kh[s,d] A1[s,j] held in f32 PSUM, snapshotted to
      bf16 SBUF every 256 rows; U(i) = SqT-strips @ A1 + qh_i @ M_snap.
  S2: N[j,d] = cumsum_s A1[s,j] vh[s,d] as a bf16 SBUF running sum;
      D_ii = masked(A1_i W'_i^T) via transposed strips;
      out2(i) = W'_i @ N_{<i} + D_ii-contract vh_i, ACT-scaled by
      1/((t+1) den) so W' is never normalized explicitly.

All [row,col]->[col,row] layout changes (W'^T, A1^T, kh, oN->oT) run on the
DMA crossbar (dma_start_transpose), costing no PE/ACT/DVE time.  The four
heads run through one software-pipelined loop (stages lag 0/2/3/4) so each
head's S2 drain overlaps the next head's U phase; A1 generation for head
h+1 is pulled two units per step into head h's loop.

Sharding: 8 cores = (batch b in 0..1) x (head-group hg in 0..3, 4 heads
each).  Each core computes its 4 heads end-to-end (wq/wk/wv column-sliced,
wc row-sliced) and returns a partial [S, Dm] output in bf16; the host sums
partials per batch and adds the wc bias.
"""

import sys

sys.path.insert(0, "/opt/trn_rl_repo")

import ml_dtypes
import numpy as np

import concourse.bass as bass  # noqa: F401  (registers AP machinery)
import concourse.mybir as mybir
from concourse import bacc
from concourse.tile import TileContext
from concourse.bass_utils import run_bass_kernel_spmd

F32 = mybir.dt.float32
BF16 = mybir.dt.bfloat16
ACTF = mybir.ActivationFunctionType
ALU = mybir.AluOpType
NPBF = ml_dtypes.bfloat16

B, S, DM, H = 2, 1024, 1024, 16
D = DM // H            # 64, head dim
HG = 4                 # heads per core
DL = HG * D            # 256, local dm slice
NB = S // 128          # 8 s-blocks
NORM_D = 0.125         # 1/sqrt(D)

DEBUG = False


def _build_program():
    nc = bacc.Bacc(None, target_bir_lowering=False)

    qT_in = nc.declare_dram_parameter("qT", [DM, S], BF16, isOutput=False)
    kT_in = nc.declare_dram_parameter("kT", [DM, S], BF16, isOutput=False)
    vT_in = nc.declare_dram_parameter("vT", [DM, S], BF16, isOutput=False)
    pT_in = nc.declare_dram_parameter("pT", [DL, S], BF16, isOutput=False)
    wq_in = nc.declare_dram_parameter("wq", [DM, DL], BF16, isOutput=False)
    wk_in = nc.declare_dram_parameter("wk", [DM, DL], BF16, isOutput=False)
    wv_in = nc.declare_dram_parameter("wv", [DM, DL], BF16, isOutput=False)
    wc_in = nc.declare_dram_parameter("wc", [DL, S], BF16, isOutput=False)
    wqb_in = nc.declare_dram_parameter("wqb", [128, 2], F32, isOutput=False)
    wkb_in = nc.declare_dram_parameter("wkb", [128, 2], F32, isOutput=False)
    wvb_in = nc.declare_dram_parameter("wvb", [1, DL], BF16, isOutput=False)
    ones_in = nc.declare_dram_parameter("ones1", [1, 128], BF16, isOutput=False)
    mask_in = nc.declare_dram_parameter("maskLE", [128, 128], BF16, isOutput=False)
    ident_in = nc.declare_dram_parameter("ident", [128, 128], BF16, isOutput=False)
    inv_in = nc.declare_dram_parameter("invidx", [128, NB], F32, isOutput=False)
    out_d = nc.declare_dram_parameter("out", [S, DM], BF16, isOutput=True)
    dbg = {}
    if DEBUG:
        dbg["a1"] = nc.declare_dram_parameter("d_a1", [128, NB * S], F32, isOutput=True)
        dbg["den"] = nc.declare_dram_parameter("d_den", [128, NB], F32, isOutput=True)
        dbg["wtT"] = nc.declare_dram_parameter("d_wtT", [128, NB * S], F32, isOutput=True)
        dbg["nsb"] = nc.declare_dram_parameter("d_nsb", [128, NB * 64], F32, isOutput=True)
        dbg["msb"] = nc.declare_dram_parameter("d_msb", [128, 2 * 512], F32, isOutput=True)
        dbg["oT"] = nc.declare_dram_parameter("d_oT", [64, HG * S], F32, isOutput=True)

    with TileContext(nc) as tc:
        with tc.tile_pool(name="persist", bufs=1) as cp, \
             tc.tile_pool(name="pm", bufs=4, space="PSUM") as pm, \
             tc.tile_pool(name="scr", bufs=2) as sp:

            maskLE = cp.tile([128, 128], BF16)
            ident = cp.tile([128, 128], BF16)
            invidx = cp.tile([128, NB], F32)
            wqb = cp.tile([128, 2], F32)
            wkb = cp.tile([128, 2], F32)
            wvb = cp.tile([1, DL], BF16)
            ones1 = cp.tile([1, 128], BF16)
            pTt = cp.tile([128, 2, S], BF16)
            qhT = cp.tile([128, 2, S], BF16)
            khT = cp.tile([128, 2, S], BF16)
            vh = cp.tile([128, NB, DL], BF16)
            oT = cp.tile([128, 2, S], BF16)
            wct = cp.tile([128, 2, S], BF16)
            # double-buffered big per-head tensors
            a1s = [cp.tile([128, NB, S], BF16, name=f"a1_{x}") for x in range(2)]
            wtTs = [cp.tile([128, NB, S], BF16, name=f"wtT_{x}") for x in range(2)]

            # PSUM cumulative state (persists across the per-head loops);
            # N is accumulated in SBUF bf16 snapshots instead (value path)
            Mps = [pm.tile([128, 512], F32, tag=f"Mps{c}", bufs=1, name=f"Mps{c}")
                   for c in range(2)]

            # ---------------- projections ----------------
            vp_cm = tc.tile_pool(name="vproj", bufs=1)
            vp = vp_cm.__enter__()
            wvt = vp.tile([128, NB, DL], BF16)
            vTt = vp.tile([128, NB, S], BF16)
            with tc.tile_pool(name="proj", bufs=1) as jp:
                wqt = jp.tile([128, NB, DL], BF16)
                wkt = jp.tile([128, NB, DL], BF16)
                qTt = jp.tile([128, NB, S], BF16)
                kTt = jp.tile([128, NB, S], BF16)
                for wt_, wsrc, xt_, xsrc in ((wqt, wq_in, qTt, qT_in),
                                             (wkt, wk_in, kTt, kT_in),
                                             (wvt, wv_in, vTt, vT_in)):
                    for q4 in range(4):
                        kb = 2 * q4
                        nc.sync.dma_start(
                            out=wt_[:, kb:kb + 2, :],
                            in_=wsrc[kb * 128:(kb + 2) * 128, :].rearrange(
                                "(a p) d -> p a d", p=128))
                        nc.sync.dma_start(
                            out=xt_[:, kb:kb + 2, :],
                            in_=xsrc[kb * 128:(kb + 2) * 128, :].rearrange(
                                "(a p) t -> p a t", p=128))
                    if wt_ is wqt:
                        nc.sync.dma_start(
                            out=pTt[:], in_=pT_in.rearrange("(g p) t -> p g t", p=128))
                        nc.sync.dma_start(out=wqb[:], in_=wqb_in[:])
                        nc.sync.dma_start(out=invidx[:], in_=inv_in[:])
                    elif wt_ is wkt:
                        nc.sync.dma_start(out=maskLE[:], in_=mask_in[:])
                        nc.sync.dma_start(out=ident[:], in_=ident_in[:])
                        nc.sync.dma_start(out=wkb[:], in_=wkb_in[:])
                    else:
                        nc.sync.dma_start(out=wvb[:], in_=wvb_in[:])
                        nc.sync.dma_start(out=ones1[:], in_=ones_in[:])
                        nc.sync.dma_start(
                            out=wct[:], in_=wc_in.rearrange("(a p) t -> p a t", p=128))

                # qhT[dm, t] = sum_c wq[c, dm] qT[c, t]  (+bias, * 1/sqrt(D))
                for wt_, xt_, dst, bias_t, scale in (
                    (wqt, qTt, qhT, wqb, NORM_D),
                    (wkt, kTt, khT, wkb, 1.0),
                ):
                    for g in range(2):
                        for n in range(2):
                            ps = pm.tile([128, 512], F32, tag="mm", name="ps_proj")
                            for kb in range(NB):
                                nc.tensor.matmul(
                                    ps[:], wt_[:, kb, g * 128:(g + 1) * 128],
                                    xt_[:, kb, n * 512:(n + 1) * 512],
                                    start=(kb == 0), stop=(kb == NB - 1))
                            nc.scalar.activation(
                                dst[:, g, n * 512:(n + 1) * 512], ps[:],
                                ACTF.Identity, bias=bias_t[:, g:g + 1], scale=scale)

            # ---------------- attention (4 heads, chunked) ----------------
            st_sq = {}      # (h,i) -> masked SqT_ii strip
            st_wb = {}      # (h,i) -> W' block (exp, unnormalized)
            st_gsc = {}     # (h,i) -> 1/((t+1) den) column
            st_a1t = {}     # (h,i) -> A1^T strip
            st_dsb = {}     # (h,i) -> masked D_ii^T
            st_nsb = {}     # (h,i) -> N snapshot through chunk i
            msbs = {}       # (c, i%2) -> M snapshot half
            khSs = {}       # h -> kh [s,d] strips
            oNs = {}

            def a1_gen(h):
                """A1 = elu(qh ph^T)+1 = min(exp(x),1) + relu(x); 16 units."""
                g, p0 = h // 2, (h % 2) * 64
                a1 = a1s[h % 2]
                for m in range(NB):
                    for c in range(2):
                        ps = pm.tile([128, 512], F32, tag="a1ps", bufs=2,
                                     name="ps_a1")
                        nc.tensor.matmul(
                            ps[:], qhT[p0:p0 + 64, g, m * 128:(m + 1) * 128],
                            pTt[p0:p0 + 64, g, c * 512:(c + 1) * 512],
                            start=True, stop=True)
                        e = sp.tile([128, 512], BF16, tag="e", bufs=5, name="e")
                        nc.scalar.activation(e[:], ps[:], ACTF.Exp)
                        e1 = sp.tile([128, 512], BF16, tag="e1", bufs=5, name="e1")
                        nc.gpsimd.tensor_scalar_min(e1[:], e[:], 1.0)
                        nc.vector.scalar_tensor_tensor(
                            a1[:, m, c * 512:(c + 1) * 512], ps[:], 0.0, e1[:],
                            ALU.max, ALU.add)
                        yield

            def emit_khS(h):
                g, p0 = h // 2, (h % 2) * 64
                khS = sp.tile([128, NB - 2, 64], BF16, tag="khS", bufs=3,
                              name="khS")
                nc.sync.dma_start_transpose(
                    out=khS[:], in_=khT[p0:p0 + 64, g, 0:(NB - 2) * 128])
                khSs[h] = khS

            def emit_sq(h, i):
                # SqT strip [s in block si, t in block i]: si = i (masked diag)
                # plus si = i-1 (unmasked) for odd i, whose M snapshot lags.
                g, p0 = h // 2, (h % 2) * 64
                for si in ([i - 1, i] if i % 2 == 1 else [i]):
                    ps = pm.tile([128, 128], F32, tag="mm", name="ps_sq")
                    nc.tensor.matmul(
                        ps[:], khT[p0:p0 + 64, g, si * 128:(si + 1) * 128],
                        qhT[p0:p0 + 64, g, i * 128:(i + 1) * 128],
                        start=True, stop=True)
                    sq = sp.tile([128, 128], BF16, tag="sq", bufs=4, name="sq")
                    if si == i:
                        nc.vector.tensor_tensor(sq[:], ps[:], maskLE[:], ALU.mult)
                    else:
                        nc.vector.tensor_copy(sq[:], ps[:])
                    st_sq[(h, i, si)] = sq

            def emit_u(h, i):
                # U(i) = SqT_ii @ A1_i + qh_i @ M_{<i};  W' = exp(U/(t+1))
                # M[d,j] += kh_i^T A1_i afterwards (PSUM accum, snapshot to bf16)
                g, p0 = h // 2, (h % 2) * 64
                a1 = a1s[h % 2]
                wb = sp.tile([128, S], BF16, tag="wblk", bufs=4, name="wb")
                st_wb[(h, i)] = wb
                strips = [st_sq.pop(k) for k in
                          ([(h, i, i - 1), (h, i, i)] if i % 2 == 1
                           else [(h, i, i)])]
                mlag = 2 * (i // 2) - 1   # M snapshot (odd index) U(i) reads
                dps = []
                for c in range(2):
                    ps = pm.tile([128, 512], F32, tag="mm", name="ps_u")
                    for z, sq in enumerate(strips):
                        si = i - (len(strips) - 1 - z)
                        nc.tensor.matmul(ps[:], sq[:],
                                         a1[:, si, c * 512:(c + 1) * 512],
                                         start=(z == 0),
                                         stop=(z == len(strips) - 1 and mlag < 0))
                    if mlag >= 0:
                        nc.tensor.matmul(
                            ps[:], qhT[p0:p0 + 64, g, i * 128:(i + 1) * 128],
                            msbs[(c, (mlag // 2) % 2)][p0:p0 + 64, :],
                            start=False, stop=True)
                    dp = sp.tile([128, 1], F32, tag="dp", bufs=6, name="dp")
                    nc.scalar.activation(
                        wb[:, c * 512:(c + 1) * 512], ps[:], ACTF.Exp,
                        scale=invidx[:, i:i + 1], accum_out=dp[:])
                    dps.append(dp)
                # M update for chunk i (after U used M_{<i}); chunks >=
                # NB-2 are past the last snapshot and never read
                if i < NB - 2:
                    for c in range(2):
                        nc.tensor.matmul(
                            Mps[c][p0:p0 + 64, :], khSs[h][:, i, :],
                            a1[:, i, c * 512:(c + 1) * 512],
                            start=(i == 0), stop=True)
                if i % 2 == 1 and i < NB - 1:
                    for c in range(2):
                        msb = sp.tile([128, 512], BF16, tag=f"msb{c}", bufs=3,
                                      name="msb")
                        nc.vector.tensor_copy(msb[p0:p0 + 64, :],
                                              Mps[c][p0:p0 + 64, :])
                        msbs[(c, (i // 2) % 2)] = msb
                # denominator -> gsc = 1/((t+1) den)
                dsum = sp.tile([128, 1], F32, tag="dsum", bufs=2, name="dsum")
                nc.vector.tensor_tensor(dsum[:], dps[0][:], dps[1][:], ALU.add)
                rec = sp.tile([128, 1], F32, tag="rec", bufs=2, name="rec")
                nc.vector.reciprocal(rec[:], dsum[:])
                gsc = sp.tile([128, 1], F32, tag="gsc", bufs=8, name="gsc")
                nc.vector.tensor_tensor(gsc[:], rec[:], invidx[:, i:i + 1],
                                        ALU.mult)
                st_gsc[(h, i)] = gsc
                if DEBUG and h == 0:
                    nc.vector.tensor_copy(dbg_den[:, i:i + 1], dsum[:])

            def emit_wt(h, i):
                wb = st_wb.pop((h, i))
                if h == HG - 1 and i == NB - 1:
                    # tail-critical: PE transpose dodges the xbar DMA latency
                    tps = pm.tile([128, S], BF16, tag="mm", name="tps")
                    for k in range(NB):
                        nc.tensor.transpose(
                            tps[:, k * 128:(k + 1) * 128],
                            wb[:, k * 128:(k + 1) * 128], ident[:])
                    nc.vector.tensor_copy(
                        wtTs[h % 2][:, :, i * 128:(i + 1) * 128],
                        tps[:].rearrange("p (a b) -> p a b", a=NB))
                else:
                    nc.sync.dma_start_transpose(
                        out=wtTs[h % 2][:, :, i * 128:(i + 1) * 128], in_=wb[:])

            def emit_a1t(h, i):
                a1 = a1s[h % 2]
                at = sp.tile([128, NB, 128], BF16, tag="a1T", bufs=5, name="a1T")
                nc.sync.dma_start_transpose(out=at[:], in_=a1[:, i, :])
                st_a1t[(h, i)] = at

            def emit_d(h, i):
                # D_ii^T[s',t'] = sum_j A1[s,j] W'[t,j], masked to s<=t
                at = st_a1t.pop((h, i))
                ps = pm.tile([128, 128], F32, tag="mm", name="ps_d")
                for k in range(NB):
                    nc.tensor.matmul(
                        ps[:], at[:, k, :],
                        wtTs[h % 2][:, k, i * 128:(i + 1) * 128],
                        start=(k == 0), stop=(k == NB - 1))
                dsb = sp.tile([128, 128], BF16, tag="dsb", bufs=4, name="dsb")
                nc.vector.tensor_tensor(dsb[:], ps[:], maskLE[:], ALU.mult)
                st_dsb[(h, i)] = dsb

            def emit_nupd(h, i):
                # N_i[j,d] = N_{i-1} + A1_i^T vh_i  (SBUF bf16 running sum);
                # the last chunk's update feeds nothing
                if i == NB - 1:
                    return
                a1 = a1s[h % 2]
                d0 = h * 64
                ps = pm.tile([128, NB, 64], F32, tag="mm", name="ps_n")
                for k in range(NB):
                    nc.tensor.matmul(
                        ps[:, k, :], a1[:, i, k * 128:(k + 1) * 128],
                        vh[:, i, d0:d0 + 64], start=True, stop=True)
                nsb = sp.tile([128, NB, 64], BF16, tag="nsb", bufs=5, name="nsb")
                if i == 0:
                    nc.vector.tensor_copy(nsb[:], ps[:])
                else:
                    nc.vector.tensor_tensor(nsb[:], ps[:],
                                            st_nsb[(h, i - 1)][:], ALU.add)
                st_nsb[(h, i)] = nsb
                if DEBUG and h == 0 and i == NB - 1:
                    nc.sync.dma_start(out=dbg["nsb"].rearrange(
                        "p (a b) -> p a b", a=NB), in_=nsb[:])

            def emit_o2(h, i):
                # out2(i) = (W'_i @ N_{<i} + D^T-contract vh_i) * gsc
                d0 = h * 64
                if h % 2 == 0 and i == 0:
                    oNs[h // 2] = sp.tile([128, NB, 128], BF16, tag="oN",
                                          bufs=2, name="oN")
                oN = oNs[h // 2]
                ps = pm.tile([128, 64], F32, tag="mm", name="ps_o2")
                dsb = st_dsb.pop((h, i))
                if i > 0:
                    nsb = st_nsb[(h, i - 1)]
                    for k in range(NB):
                        nc.tensor.matmul(
                            ps[:], wtTs[h % 2][:, k, i * 128:(i + 1) * 128],
                            nsb[:, k, :], start=(k == 0), stop=False)
                    nc.tensor.matmul(ps[:], dsb[:], vh[:, i, d0:d0 + 64],
                                     start=False, stop=True)
                else:
                    nc.tensor.matmul(ps[:], dsb[:], vh[:, i, d0:d0 + 64],
                                     start=True, stop=True)
                if i >= 2:
                    st_nsb.pop((h, i - 2), None)
                nc.scalar.activation(
                    oN[:, i, (h % 2) * 64:(h % 2) * 64 + 64], ps[:],
                    ACTF.Copy, scale=st_gsc.pop((h, i))[:])
                if h % 2 == 1:
                    if h == HG - 1 and i == NB - 1:
                        tpo = pm.tile([128, 128], BF16, tag="mm", name="tpo")
                        nc.tensor.transpose(tpo[:], oN[:, i, :], ident[:])
                        nc.scalar.activation(
                            oT[:, h // 2, i * 128:(i + 1) * 128], tpo[:],
                            ACTF.Copy)
                    else:
                        nc.sync.dma_start_transpose(
                            out=oT[:, h // 2, i * 128:(i + 1) * 128],
                            in_=oN[:, i, :])

            def emit_final_tile(i):
                # i covers row-blocks 2i, 2i+1; one DMA per 256 output rows
                ot = sp.tile([128, 2, S], BF16, tag="ot", bufs=2, name="ot")
                for z in range(2):
                    ib = 2 * i + z
                    for c in range(2):
                        ps = pm.tile([128, 512], F32, tag="mm", name="ps_fin")
                        for g2 in range(2):
                            nc.tensor.matmul(
                                ps[:], oT[:, g2, ib * 128:(ib + 1) * 128],
                                wct[:, g2, c * 512:(c + 1) * 512],
                                start=(g2 == 0), stop=(g2 == 1))
                        if (ib + c) % 2 == 0:
                            nc.scalar.activation(
                                ot[:, z, c * 512:(c + 1) * 512], ps[:], ACTF.Copy)
                        else:
                            nc.vector.tensor_copy(
                                ot[:, z, c * 512:(c + 1) * 512], ps[:])
                nc.sync.dma_start(
                    out=out_d[2 * i * 128:(2 * i + 2) * 128, :].rearrange(
                        "(a p) d -> p a d", p=128),
                    in_=ot[:])

            if DEBUG:
                dbg_den = sp.tile([128, NB], F32, tag="dbgden", bufs=1,
                                  name="dbgden")

            # vh[s, d] = sum_c vT[c, s] wv[c, d] + wv_b[d], interleaved with
            # head 0's A1 so PE has work while vT streams in
            gens = {hh: a1_gen(hh) for hh in range(HG)}

            def pull(h, n):
                if h < HG:
                    for _ in range(n):
                        if next(gens[h], "done") == "done":
                            break

            def emit_vh(m):
                ps = pm.tile([128, DL], F32, tag="mm", name="ps_vh")
                for kb in range(NB):
                    nc.tensor.matmul(
                        ps[:], vTt[:, kb, m * 128:(m + 1) * 128], wvt[:, kb, :],
                        start=(kb == 0), stop=False)
                nc.tensor.matmul(ps[:], ones1[:], wvb[:], start=False, stop=True)
                nc.scalar.activation(vh[:, m, :], ps[:], ACTF.Copy)

            pull(0, 32)
            if DEBUG:
                nc.sync.dma_start(out=dbg["a1"].rearrange("p (a b) -> p a b", a=NB),
                                  in_=a1s[0][:])

            def hi(tau):
                # map absolute pipeline time to (head, iter), None past the end
                h, i = divmod(tau, NB)
                return (h, i) if 0 <= h < HG else None

            emit_khS(0)
            emit_sq(0, 0)
            for tau in range(HG * NB + 4):
                cur = hi(tau)
                if tau <= NB - 1:
                    emit_vh(tau)
                if cur:
                    nxt = hi(tau + 1)
                    if nxt:
                        if nxt[1] == 0:
                            emit_khS(nxt[0])
                        emit_sq(*nxt)
                    emit_u(*cur)
                    emit_wt(*cur)
                pull(tau // NB + 1, 1)
                if cur:
                    emit_a1t(*cur)
                if hi(tau - 2):
                    emit_nupd(*hi(tau - 2))
                pull(tau // NB + 1, 1)
                for hh in range(HG):
                    ld = 3 if hh < HG - 1 else 2
                    for stage, lag in ((emit_d, ld), (emit_o2, ld + 1)):
                        ii = tau - NB * hh - lag
                        if 0 <= ii <= NB - 1:
                            stage(hh, ii)
            st_nsb.clear()
            oNs.clear()
            for i in range(NB // 2):
                emit_final_tile(i)
            if DEBUG:
                nc.sync.dma_start(
                    out=dbg["oT"].rearrange("p (a b) -> p a b", a=HG), in_=oT[:])

            vp_cm.__exit__(None, None, None)

    nc.finalize()
    return nc


_CACHE = {}


def _get_program():
    if "nc" not in _CACHE:
        _CACHE["nc"] = _build_program()
    return _CACHE["nc"]


def _consts():
    if "consts" not in _CACHE:
        p_ = np.arange(128, dtype=np.float32)[:, None]
        c_ = np.arange(128, dtype=np.float32)[None, :]
        maskLE = (p_ <= c_).astype(NPBF)
        ident = np.eye(128, dtype=np.float32).astype(NPBF)
        blk = np.arange(NB, dtype=np.float32)[None, :]
        invidx = (1.0 / (blk * 128.0 + p_ + 1.0)).astype(np.float32)
        ones1 = np.ones((1, 128), NPBF)
        _CACHE["consts"] = (maskLE, ident, invidx, ones1)
    return _CACHE["consts"]


PROFILE = False
LAST_RESULTS = None


def kernel(v, k, q, p, wq_k, wq_b, wk_k, wk_b, wv_k, wv_b, wc_k, wc_b):
    global LAST_RESULTS
    nc = _get_program()
    maskLE, ident, invidx, ones1 = _consts()

    qT = [np.ascontiguousarray(q[b].T).astype(NPBF) for b in range(B)]
    kT = [np.ascontiguousarray(k[b].T).astype(NPBF) for b in range(B)]
    vT = [np.ascontiguousarray(v[b].T).astype(NPBF) for b in range(B)]
    pT = [np.ascontiguousarray(p[b].T).astype(NPBF) for b in range(B)]
    wqc = wq_k.astype(NPBF)
    wkc = wk_k.astype(NPBF)
    wvc = wv_k.astype(NPBF)
    wcc = wc_k.astype(NPBF)

    in_maps = []
    for c in range(8):
        b, hg = c // 4, c % 4
        c0 = hg * DL
        wqb = np.ascontiguousarray(
            (wq_b[c0:c0 + DL].reshape(2, 128).T * NORM_D).astype(np.float32))
        wkb = np.ascontiguousarray(wk_b[c0:c0 + DL].reshape(2, 128).T.astype(np.float32))
        in_maps.append({
            "qT": qT[b], "kT": kT[b], "vT": vT[b],
            "pT": np.ascontiguousarray(pT[b][c0:c0 + DL]),
            "wq": np.ascontiguousarray(wqc[:, c0:c0 + DL]),
            "wk": np.ascontiguousarray(wkc[:, c0:c0 + DL]),
            "wv": np.ascontiguousarray(wvc[:, c0:c0 + DL]),
            "wc": np.ascontiguousarray(wcc[c0:c0 + DL, :]),
            "wqb": wqb, "wkb": wkb,
            "wvb": np.ascontiguousarray(wv_b[c0:c0 + DL].reshape(1, DL).astype(NPBF)),
            "ones1": ones1, "maskLE": maskLE, "ident": ident,
            "invidx": invidx,
        })

    res = run_bass_kernel_spmd(
        nc, in_maps, core_ids=list(range(8)), trace=PROFILE)
    LAST_RESULTS = res

    out = np.zeros((B, S, DM), np.float32)
    for c in range(8):
        out[c // 4] += res.results[c]["out"].astype(np.float32)
    out += wc_b[None, None, :].astype(np.float32)
    return out


# revision 9
# speedup vs baseline: 1.0436x; 1.0003x over previous
"""Trainium2 Bass kernel for nn_MultiHeadAttention_75548474736720.

Linear-attention-style MHA with two causal prefix-sum bilinear forms,
evaluated with a chunked (linear-attention) reformulation instead of the
naive O(S^2)-blocks triangular matmuls:
  qh/kh/vh = projections, ph = split_heads(p)
  A1 = elu(qh ph^T) + 1
  U[t,j] = sum_{s<=t} Sq[t,s] A1[s,j],  Sq = qh kh^T  (1/(t+1) in exp scale)
  W' = exp(U/(t+1)), den = sum_j W'
  out2[t,d] = (1/((t+1) den[t])) sum_{s<=t} (W'[t].A1[s]) vh[s,d]

Chunked evaluation (128-row chunks, 256-row state snapshots):
  U:  M[d,j] = cumsum_s kh[s,d] A1[s,j] held in f32 PSUM, snapshotted to
      bf16 SBUF every 256 rows; U(i) = SqT-strips @ A1 + qh_i @ M_snap.
  S2: N[j,d] = cumsum_s A1[s,j] vh[s,d] as a bf16 SBUF running sum;
      D_ii = masked(A1_i W'_i^T) via transposed strips;
      out2(i) = W'_i @ N_{<i} + D_ii-contract vh_i, ACT-scaled by
      1/((t+1) den) so W' is never normalized explicitly.

All [row,col]->[col,row] layout changes (W'^T, A1^T, kh, oN->oT) run on the
DMA crossbar (dma_start_transpose), costing no PE/ACT/DVE time.  The four
heads run through one software-pipelined loop (stages lag 0/2/3/4) so each
head's S2 drain overlaps the next head's U phase; A1 generation for head
h+1 is pulled two units per step into head h's loop.

Sharding: 8 cores = (batch b in 0..1) x (head-group hg in 0..3, 4 heads
each).  Each core computes its 4 heads end-to-end (wq/wk/wv column-sliced,
wc row-sliced) and returns a partial [S, Dm] output in bf16; the host sums
partials per batch and adds the wc bias.
"""

import sys

sys.path.insert(0, "/opt/trn_rl_repo")

import ml_dtypes
import numpy as np

import concourse.bass as bass  # noqa: F401  (registers AP machinery)
import concourse.mybir as mybir
from concourse import bacc
from concourse.tile import TileContext
from concourse.bass_utils import run_bass_kernel_spmd

F32 = mybir.dt.float32
BF16 = mybir.dt.bfloat16
ACTF = mybir.ActivationFunctionType
ALU = mybir.AluOpType
NPBF = ml_dtypes.bfloat16

B, S, DM, H = 2, 1024, 1024, 16
D = DM // H            # 64, head dim
HG = 4                 # heads per core
DL = HG * D            # 256, local dm slice
NB = S // 128          # 8 s-blocks
NORM_D = 0.125         # 1/sqrt(D)

DEBUG = False


def _build_program():
    nc = bacc.Bacc(None, target_bir_lowering=False)

    qT_in = nc.declare_dram_parameter("qT", [DM, S], BF16, isOutput=False)
    kT_in = nc.declare_dram_parameter("kT", [DM, S], BF16, isOutput=False)
    vT_in = nc.declare_dram_parameter("vT", [DM, S], BF16, isOutput=False)
    pT_in = nc.declare_dram_parameter("pT", [DL, S], BF16, isOutput=False)
    wq_in = nc.declare_dram_parameter("wq", [DM, DL], BF16, isOutput=False)
    wk_in = nc.declare_dram_parameter("wk", [DM, DL], BF16, isOutput=False)
    wv_in = nc.declare_dram_parameter("wv", [DM, DL], BF16, isOutput=False)
    wc_in = nc.declare_dram_parameter("wc", [DL, S], BF16, isOutput=False)
    wqb_in = nc.declare_dram_parameter("wqb", [128, 2], F32, isOutput=False)
    wkb_in = nc.declare_dram_parameter("wkb", [128, 2], F32, isOutput=False)
    wvb_in = nc.declare_dram_parameter("wvb", [1, DL], BF16, isOutput=False)
    ones_in = nc.declare_dram_parameter("ones1", [1, 128], BF16, isOutput=False)
    mask_in = nc.declare_dram_parameter("maskLE", [128, 128], BF16, isOutput=False)
    ident_in = nc.declare_dram_parameter("ident", [128, 128], BF16, isOutput=False)
    inv_in = nc.declare_dram_parameter("invidx", [128, NB], F32, isOutput=False)
    out_d = nc.declare_dram_parameter("out", [S, DM], BF16, isOutput=True)
    dbg = {}
    if DEBUG:
        dbg["a1"] = nc.declare_dram_parameter("d_a1", [128, NB * S], F32, isOutput=True)
        dbg["den"] = nc.declare_dram_parameter("d_den", [128, NB], F32, isOutput=True)
        dbg["wtT"] = nc.declare_dram_parameter("d_wtT", [128, NB * S], F32, isOutput=True)
        dbg["nsb"] = nc.declare_dram_parameter("d_nsb", [128, NB * 64], F32, isOutput=True)
        dbg["msb"] = nc.declare_dram_parameter("d_msb", [128, 2 * 512], F32, isOutput=True)
        dbg["oT"] = nc.declare_dram_parameter("d_oT", [64, HG * S], F32, isOutput=True)

    with TileContext(nc) as tc:
        with tc.tile_pool(name="persist", bufs=1) as cp, \
             tc.tile_pool(name="pm", bufs=4, space="PSUM") as pm, \
             tc.tile_pool(name="scr", bufs=2) as sp:

            maskLE = cp.tile([128, 128], BF16)
            ident = cp.tile([128, 128], BF16)
            invidx = cp.tile([128, NB], F32)
            wqb = cp.tile([128, 2], F32)
            wkb = cp.tile([128, 2], F32)
            wvb = cp.tile([1, DL], BF16)
            ones1 = cp.tile([1, 128], BF16)
            pTt = cp.tile([128, 2, S], BF16)
            qhT = cp.tile([128, 2, S], BF16)
            khT = cp.tile([128, 2, S], BF16)
            vh = cp.tile([128, NB, DL], BF16)
            oT = cp.tile([128, 2, S], BF16)
            wct = cp.tile([128, 2, S], BF16)
            # double-buffered big per-head tensors
            a1s = [cp.tile([128, NB, S], BF16, name=f"a1_{x}") for x in range(2)]
            wtTs = [cp.tile([128, NB, S], BF16, name=f"wtT_{x}") for x in range(2)]

            # PSUM cumulative state (persists across the per-head loops);
            # N is accumulated in SBUF bf16 snapshots instead (value path)
            Mps = [pm.tile([128, 512], F32, tag=f"Mps{c}", bufs=1, name=f"Mps{c}")
                   for c in range(2)]

            # ---------------- projections ----------------
            vp_cm = tc.tile_pool(name="vproj", bufs=1)
            vp = vp_cm.__enter__()
            wvt = vp.tile([128, NB, DL], BF16)
            vTt = vp.tile([128, NB, S], BF16)
            with tc.tile_pool(name="proj", bufs=1) as jp:
                wqt = jp.tile([128, NB, DL], BF16)
                wkt = jp.tile([128, NB, DL], BF16)
                qTt = jp.tile([128, NB, S], BF16)
                kTt = jp.tile([128, NB, S], BF16)
                for wt_, wsrc, xt_, xsrc in ((wqt, wq_in, qTt, qT_in),
                                             (wkt, wk_in, kTt, kT_in),
                                             (wvt, wv_in, vTt, vT_in)):
                    for q4 in range(4):
                        kb = 2 * q4
                        nc.sync.dma_start(
                            out=wt_[:, kb:kb + 2, :],
                            in_=wsrc[kb * 128:(kb + 2) * 128, :].rearrange(
                                "(a p) d -> p a d", p=128))
                        nc.sync.dma_start(
                            out=xt_[:, kb:kb + 2, :],
                            in_=xsrc[kb * 128:(kb + 2) * 128, :].rearrange(
                                "(a p) t -> p a t", p=128))
                    if wt_ is wqt:
                        nc.sync.dma_start(
                            out=pTt[:], in_=pT_in.rearrange("(g p) t -> p g t", p=128))
                        nc.sync.dma_start(out=wqb[:], in_=wqb_in[:])
                        nc.sync.dma_start(out=invidx[:], in_=inv_in[:])
                    elif wt_ is wkt:
                        nc.sync.dma_start(out=maskLE[:], in_=mask_in[:])
                        nc.sync.dma_start(out=ident[:], in_=ident_in[:])
                        nc.sync.dma_start(out=wkb[:], in_=wkb_in[:])
                    else:
                        nc.sync.dma_start(out=wvb[:], in_=wvb_in[:])
                        nc.sync.dma_start(out=ones1[:], in_=ones_in[:])
                        nc.sync.dma_start(
                            out=wct[:], in_=wc_in.rearrange("(a p) t -> p a t", p=128))

                # qhT[dm, t] = sum_c wq[c, dm] qT[c, t]  (+bias, * 1/sqrt(D))
                for wt_, xt_, dst, bias_t, scale in (
                    (wqt, qTt, qhT, wqb, NORM_D),
                    (wkt, kTt, khT, wkb, 1.0),
                ):
                    for g in range(2):
                        for n in range(2):
                            ps = pm.tile([128, 512], F32, tag="mm", name="ps_proj")
                            for kb in range(NB):
                                nc.tensor.matmul(
                                    ps[:], wt_[:, kb, g * 128:(g + 1) * 128],
                                    xt_[:, kb, n * 512:(n + 1) * 512],
                                    start=(kb == 0), stop=(kb == NB - 1))
                            nc.scalar.activation(
                                dst[:, g, n * 512:(n + 1) * 512], ps[:],
                                ACTF.Identity, bias=bias_t[:, g:g + 1], scale=scale)

            # ---------------- attention (4 heads, chunked) ----------------
            st_sq = {}      # (h,i) -> masked SqT_ii strip
            st_wb = {}      # (h,i) -> W' block (exp, unnormalized)
            st_gsc = {}     # (h,i) -> 1/((t+1) den) column
            st_a1t = {}     # (h,i) -> A1^T strip
            st_dsb = {}     # (h,i) -> masked D_ii^T
            st_nsb = {}     # (h,i) -> N snapshot through chunk i
            msbs = {}       # (c, i%2) -> M snapshot half
            khSs = {}       # h -> kh [s,d] strips
            oNs = {}

            def a1_gen(h):
                """A1 = elu(qh ph^T)+1 = min(exp(x),1) + relu(x); 16 units."""
                g, p0 = h // 2, (h % 2) * 64
                a1 = a1s[h % 2]
                for m in range(NB):
                    for c in range(2):
                        ps = pm.tile([128, 512], F32, tag="a1ps", bufs=2,
                                     name="ps_a1")
                        nc.tensor.matmul(
                            ps[:], qhT[p0:p0 + 64, g, m * 128:(m + 1) * 128],
                            pTt[p0:p0 + 64, g, c * 512:(c + 1) * 512],
                            start=True, stop=True)
                        e = sp.tile([128, 512], BF16, tag="e", bufs=5, name="e")
                        nc.scalar.activation(e[:], ps[:], ACTF.Exp)
                        e1 = sp.tile([128, 512], BF16, tag="e1", bufs=5, name="e1")
                        nc.gpsimd.tensor_scalar_min(e1[:], e[:], 1.0)
                        nc.vector.scalar_tensor_tensor(
                            a1[:, m, c * 512:(c + 1) * 512], ps[:], 0.0, e1[:],
                            ALU.max, ALU.add)
                        yield

            def emit_khS(h):
                g, p0 = h // 2, (h % 2) * 64
                khS = sp.tile([128, NB - 2, 64], BF16, tag="khS", bufs=3,
                              name="khS")
                nc.sync.dma_start_transpose(
                    out=khS[:], in_=khT[p0:p0 + 64, g, 0:(NB - 2) * 128])
                khSs[h] = khS

            def emit_sq(h, i):
                # SqT strip [s in block si, t in block i]: si = i (masked diag)
                # plus si = i-1 (unmasked) for odd i, whose M snapshot lags.
                g, p0 = h // 2, (h % 2) * 64
                for si in ([i - 1, i] if i % 2 == 1 else [i]):
                    ps = pm.tile([128, 128], F32, tag="mm", name="ps_sq")
                    nc.tensor.matmul(
                        ps[:], khT[p0:p0 + 64, g, si * 128:(si + 1) * 128],
                        qhT[p0:p0 + 64, g, i * 128:(i + 1) * 128],
                        start=True, stop=True)
                    sq = sp.tile([128, 128], BF16, tag="sq", bufs=4, name="sq")
                    if si == i:
                        nc.vector.tensor_tensor(sq[:], ps[:], maskLE[:], ALU.mult)
                    else:
                        nc.vector.tensor_copy(sq[:], ps[:])
                    st_sq[(h, i, si)] = sq

            def emit_u(h, i):
                # U(i) = SqT_ii @ A1_i + qh_i @ M_{<i};  W' = exp(U/(t+1))
                # M[d,j] += kh_i^T A1_i afterwards (PSUM accum, snapshot to bf16)
                g, p0 = h // 2, (h % 2) * 64
                a1 = a1s[h % 2]
                wb = sp.tile([128, S], BF16, tag="wblk", bufs=4, name="wb")
                st_wb[(h, i)] = wb
                strips = [st_sq.pop(k) for k in
                          ([(h, i, i - 1), (h, i, i)] if i % 2 == 1
                           else [(h, i, i)])]
                mlag = 2 * (i // 2) - 1   # M snapshot (odd index) U(i) reads
                dps = []
                for c in range(2):
                    ps = pm.tile([128, 512], F32, tag="mm", name="ps_u")
                    for z, sq in enumerate(strips):
                        si = i - (len(strips) - 1 - z)
                        nc.tensor.matmul(ps[:], sq[:],
                                         a1[:, si, c * 512:(c + 1) * 512],
                                         start=(z == 0),
                                         stop=(z == len(strips) - 1 and mlag < 0))
                    if mlag >= 0:
                        nc.tensor.matmul(
                            ps[:], qhT[p0:p0 + 64, g, i * 128:(i + 1) * 128],
                            msbs[(c, (mlag // 2) % 2)][p0:p0 + 64, :],
                            start=False, stop=True)
                    dp = sp.tile([128, 1], F32, tag="dp", bufs=6, name="dp")
                    nc.scalar.activation(
                        wb[:, c * 512:(c + 1) * 512], ps[:], ACTF.Exp,
                        scale=invidx[:, i:i + 1], accum_out=dp[:])
                    dps.append(dp)
                # M update for chunk i (after U used M_{<i}); chunks >=
                # NB-2 are past the last snapshot and never read
                if i < NB - 2:
                    for c in range(2):
                        nc.tensor.matmul(
                            Mps[c][p0:p0 + 64, :], khSs[h][:, i, :],
                            a1[:, i, c * 512:(c + 1) * 512],
                            start=(i == 0), stop=True)
                if i % 2 == 1 and i < NB - 1:
                    for c in range(2):
                        msb = sp.tile([128, 512], BF16, tag=f"msb{c}", bufs=3,
                                      name="msb")
                        nc.vector.tensor_copy(msb[p0:p0 + 64, :],
                                              Mps[c][p0:p0 + 64, :])
                        msbs[(c, (i // 2) % 2)] = msb
                # denominator -> gsc = 1/((t+1) den)
                dsum = sp.tile([128, 1], F32, tag="dsum", bufs=2, name="dsum")
                nc.vector.tensor_tensor(dsum[:], dps[0][:], dps[1][:], ALU.add)
                rec = sp.tile([128, 1], F32, tag="rec", bufs=2, name="rec")
                nc.vector.reciprocal(rec[:], dsum[:])
                gsc = sp.tile([128, 1], F32, tag="gsc", bufs=8, name="gsc")
                nc.vector.tensor_tensor(gsc[:], rec[:], invidx[:, i:i + 1],
                                        ALU.mult)
                st_gsc[(h, i)] = gsc
                if DEBUG and h == 0:
                    nc.vector.tensor_copy(dbg_den[:, i:i + 1], dsum[:])

            def emit_wt(h, i):
                wb = st_wb.pop((h, i))
                if h == HG - 1 and i == NB - 1:
                    # tail-critical: PE transpose dodges the xbar DMA latency
                    tps = pm.tile([128, S], BF16, tag="mm", name="tps")
                    for k in range(NB):
                        nc.tensor.transpose(
                            tps[:, k * 128:(k + 1) * 128],
                            wb[:, k * 128:(k + 1) * 128], ident[:])
                    nc.vector.tensor_copy(
                        wtTs[h % 2][:, :, i * 128:(i + 1) * 128],
                        tps[:].rearrange("p (a b) -> p a b", a=NB))
                else:
                    nc.sync.dma_start_transpose(
                        out=wtTs[h % 2][:, :, i * 128:(i + 1) * 128], in_=wb[:])

            def emit_a1t(h, i):
                a1 = a1s[h % 2]
                at = sp.tile([128, NB, 128], BF16, tag="a1T", bufs=5, name="a1T")
                nc.sync.dma_start_transpose(out=at[:], in_=a1[:, i, :])
                st_a1t[(h, i)] = at

            def emit_d(h, i):
                # D_ii^T[s',t'] = sum_j A1[s,j] W'[t,j], masked to s<=t
                at = st_a1t.pop((h, i))
                ps = pm.tile([128, 128], F32, tag="mm", name="ps_d")
                for k in range(NB):
                    nc.tensor.matmul(
                        ps[:], at[:, k, :],
                        wtTs[h % 2][:, k, i * 128:(i + 1) * 128],
                        start=(k == 0), stop=(k == NB - 1))
                dsb = sp.tile([128, 128], BF16, tag="dsb", bufs=4, name="dsb")
                nc.vector.tensor_tensor(dsb[:], ps[:], maskLE[:], ALU.mult)
                st_dsb[(h, i)] = dsb

            def emit_nupd(h, i):
                # N_i[j,d] = N_{i-1} + A1_i^T vh_i  (SBUF bf16 running sum);
                # the last chunk's update feeds nothing
                if i == NB - 1:
                    return
                a1 = a1s[h % 2]
                d0 = h * 64
                ps = pm.tile([128, NB, 64], F32, tag="mm", name="ps_n")
                for k in range(NB):
                    nc.tensor.matmul(
                        ps[:, k, :], a1[:, i, k * 128:(k + 1) * 128],
                        vh[:, i, d0:d0 + 64], start=True, stop=True)
                nsb = sp.tile([128, NB, 64], BF16, tag="nsb", bufs=5, name="nsb")
                if i == 0:
                    nc.vector.tensor_copy(nsb[:], ps[:])
                else:
                    nc.vector.tensor_tensor(nsb[:], ps[:],
                                            st_nsb[(h, i - 1)][:], ALU.add)
                st_nsb[(h, i)] = nsb
                if DEBUG and h == 0 and i == NB - 1:
                    nc.sync.dma_start(out=dbg["nsb"].rearrange(
                        "p (a b) -> p a b", a=NB), in_=nsb[:])

            def emit_o2(h, i):
                # out2(i) = (W'_i @ N_{<i} + D^T-contract vh_i) * gsc
                d0 = h * 64
                if h % 2 == 0 and i == 0:
                    oNs[h // 2] = sp.tile([128, NB, 128], BF16, tag="oN",
                                          bufs=2, name="oN")
                oN = oNs[h // 2]
                ps = pm.tile([128, 64], F32, tag="mm", name="ps_o2")
                dsb = st_dsb.pop((h, i))
                if i > 0:
                    nsb = st_nsb[(h, i - 1)]
                    for k in range(NB):
                        nc.tensor.matmul(
                            ps[:], wtTs[h % 2][:, k, i * 128:(i + 1) * 128],
                            nsb[:, k, :], start=(k == 0), stop=False)
                    nc.tensor.matmul(ps[:], dsb[:], vh[:, i, d0:d0 + 64],
                                     start=False, stop=True)
                else:
                    nc.tensor.matmul(ps[:], dsb[:], vh[:, i, d0:d0 + 64],
                                     start=True, stop=True)
                if i >= 2:
                    st_nsb.pop((h, i - 2), None)
                nc.scalar.activation(
                    oN[:, i, (h % 2) * 64:(h % 2) * 64 + 64], ps[:],
                    ACTF.Copy, scale=st_gsc.pop((h, i))[:])
                if h % 2 == 1:
                    if h == HG - 1 and i == NB - 1:
                        tpo = pm.tile([128, 128], BF16, tag="mm", name="tpo")
                        nc.tensor.transpose(tpo[:], oN[:, i, :], ident[:])
                        nc.scalar.activation(
                            oT[:, h // 2, i * 128:(i + 1) * 128], tpo[:],
                            ACTF.Copy)
                    else:
                        nc.sync.dma_start_transpose(
                            out=oT[:, h // 2, i * 128:(i + 1) * 128],
                            in_=oN[:, i, :])

            def emit_final_tile(i):
                # i covers row-blocks 2i, 2i+1; one DMA per 256 output rows
                ot = sp.tile([128, 2, S], BF16, tag="ot", bufs=2, name="ot")
                for z in range(2):
                    ib = 2 * i + z
                    for c in range(2):
                        ps = pm.tile([128, 512], F32, tag="mm", name="ps_fin")
                        for g2 in range(2):
                            nc.tensor.matmul(
                                ps[:], oT[:, g2, ib * 128:(ib + 1) * 128],
                                wct[:, g2, c * 512:(c + 1) * 512],
                                start=(g2 == 0), stop=(g2 == 1))
                        if (ib + c) % 2 == 0:
                            nc.scalar.activation(
                                ot[:, z, c * 512:(c + 1) * 512], ps[:], ACTF.Copy)
                        else:
                            nc.vector.tensor_copy(
                                ot[:, z, c * 512:(c + 1) * 512], ps[:])
                nc.sync.dma_start(
                    out=out_d[2 * i * 128:(2 * i + 2) * 128, :].rearrange(
                        "(a p) d -> p a d", p=128),
                    in_=ot[:])

            if DEBUG:
                dbg_den = sp.tile([128, NB], F32, tag="dbgden", bufs=1,
                                  name="dbgden")

            # vh[s, d] = sum_c vT[c, s] wv[c, d] + wv_b[d], interleaved with
            # head 0's A1 so PE has work while vT streams in
            gens = {hh: a1_gen(hh) for hh in range(HG)}

            def pull(h, n):
                if h < HG:
                    for _ in range(n):
                        if next(gens[h], "done") == "done":
                            break

            def emit_vh(m):
                ps = pm.tile([128, DL], F32, tag="mm", name="ps_vh")
                for kb in range(NB):
                    nc.tensor.matmul(
                        ps[:], vTt[:, kb, m * 128:(m + 1) * 128], wvt[:, kb, :],
                        start=(kb == 0), stop=False)
                nc.tensor.matmul(ps[:], ones1[:], wvb[:], start=False, stop=True)
                nc.scalar.activation(vh[:, m, :], ps[:], ACTF.Copy)

            pull(0, 32)
            if DEBUG:
                nc.sync.dma_start(out=dbg["a1"].rearrange("p (a b) -> p a b", a=NB),
                                  in_=a1s[0][:])

            def hi(tau):
                # map absolute pipeline time to (head, iter), None past the end
                h, i = divmod(tau, NB)
                return (h, i) if 0 <= h < HG else None

            emit_khS(0)
            emit_sq(0, 0)
            for tau in range(HG * NB + 4):
                cur = hi(tau)
                if tau <= NB - 1:
                    emit_vh(tau)
                if cur:
                    nxt = hi(tau + 1)
                    if nxt:
                        if nxt[1] == 0:
                            emit_khS(nxt[0])
                        emit_sq(*nxt)
                    emit_u(*cur)
                    emit_wt(*cur)
                pull(tau // NB + 1, 1)
                if cur:
                    emit_a1t(*cur)
                if hi(tau - 2):
                    emit_nupd(*hi(tau - 2))
                pull(tau // NB + 1, 1)
                for hh in range(HG):
                    ld = 2
                    for stage, lag in ((emit_d, ld), (emit_o2, ld + 1)):
                        ii = tau - NB * hh - lag
                        if 0 <= ii <= NB - 1:
                            stage(hh, ii)
            st_nsb.clear()
            oNs.clear()
            for i in range(NB // 2):
                emit_final_tile(i)
            if DEBUG:
                nc.sync.dma_start(
                    out=dbg["oT"].rearrange("p (a b) -> p a b", a=HG), in_=oT[:])

            vp_cm.__exit__(None, None, None)

    nc.finalize()
    return nc


_CACHE = {}


def _get_program():
    if "nc" not in _CACHE:
        _CACHE["nc"] = _build_program()
    return _CACHE["nc"]


def _consts():
    if "consts" not in _CACHE:
        p_ = np.arange(128, dtype=np.float32)[:, None]
        c_ = np.arange(128, dtype=np.float32)[None, :]
        maskLE = (p_ <= c_).astype(NPBF)
        ident = np.eye(128, dtype=np.float32).astype(NPBF)
        blk = np.arange(NB, dtype=np.float32)[None, :]
        invidx = (1.0 / (blk * 128.0 + p_ + 1.0)).astype(np.float32)
        ones1 = np.ones((1, 128), NPBF)
        _CACHE["consts"] = (maskLE, ident, invidx, ones1)
    return _CACHE["consts"]


PROFILE = False
LAST_RESULTS = None


def kernel(v, k, q, p, wq_k, wq_b, wk_k, wk_b, wv_k, wv_b, wc_k, wc_b):
    global LAST_RESULTS
    nc = _get_program()
    maskLE, ident, invidx, ones1 = _consts()

    qT = [np.ascontiguousarray(q[b].T).astype(NPBF) for b in range(B)]
    kT = [np.ascontiguousarray(k[b].T).astype(NPBF) for b in range(B)]
    vT = [np.ascontiguousarray(v[b].T).astype(NPBF) for b in range(B)]
    pT = [np.ascontiguousarray(p[b].T).astype(NPBF) for b in range(B)]
    wqc = wq_k.astype(NPBF)
    wkc = wk_k.astype(NPBF)
    wvc = wv_k.astype(NPBF)
    wcc = wc_k.astype(NPBF)

    in_maps = []
    for c in range(8):
        b, hg = c // 4, c % 4
        c0 = hg * DL
        wqb = np.ascontiguousarray(
            (wq_b[c0:c0 + DL].reshape(2, 128).T * NORM_D).astype(np.float32))
        wkb = np.ascontiguousarray(wk_b[c0:c0 + DL].reshape(2, 128).T.astype(np.float32))
        in_maps.append({
            "qT": qT[b], "kT": kT[b], "vT": vT[b],
            "pT": np.ascontiguousarray(pT[b][c0:c0 + DL]),
            "wq": np.ascontiguousarray(wqc[:, c0:c0 + DL]),
            "wk": np.ascontiguousarray(wkc[:, c0:c0 + DL]),
            "wv": np.ascontiguousarray(wvc[:, c0:c0 + DL]),
            "wc": np.ascontiguousarray(wcc[c0:c0 + DL, :]),
            "wqb": wqb, "wkb": wkb,
            "wvb": np.ascontiguousarray(wv_b[c0:c0 + DL].reshape(1, DL).astype(NPBF)),
            "ones1": ones1, "maskLE": maskLE, "ident": ident,
            "invidx": invidx,
        })

    res = run_bass_kernel_spmd(
        nc, in_maps, core_ids=list(range(8)), trace=PROFILE)
    LAST_RESULTS = res

    out = np.zeros((B, S, DM), np.float32)
    for c in range(8):
        out[c // 4] += res.results[c]["out"].astype(np.float32)
    out += wc_b[None, None, :].astype(np.float32)
    return out
